# revision 1
# baseline (speedup 1.0000x reference)
"""Channel-self-attention (LayerNorm + grouped-1x1-qkv + channel softmax attn
+ residual) on 8 TRN2 NeuronCores.

Strategy: shard the spatial axis (32^3 = 32768 -> 4096/core). Per core:
 - keep the x-shard [1024, 4096] f32 resident in SBUF
 - local stats (Sum x, Sum x^2) via DVE bn_stats/bn_aggr
 - u = gamma*x (ch 0..170 per batch) cast bf16, DMA-xbar-transpose SBUF->SBUF,
   PE Gram P^T[kap,a] = Sum_s u_{85+kap} u_a and t/g/h = u^T @ [gamma,beta,1]
 - ONE AllReduce (~132 KB) of (P^T, tgh, stats)
 - logits rebuilt from the Gram expansion of the LayerNorm algebra, softmax,
   apply att via one [89-row] matmul against [-gamma; beta; 1; gamma*x_V]
 - out = x + recip * PS  (softmax division folded into the epilogue)
"""
import sys

sys.path.insert(0, "/opt/trn_rl_repo")

import numpy as np
import ml_dtypes

B, C = 4, 256
S = 32 * 32 * 32          # 32768
NCORES = 8
SH = S // NCORES          # 4096 per-core spatial shard
NST = SH // 128           # 32 stiles
EPS = 1e-5
SCALE = float(S) ** -0.5

_BF = ml_dtypes.bfloat16

_cache = {}


def _build_program():
    """Trace the Bass/Tile program once; returns the compiled Bacc."""
    from contextlib import ExitStack
    import concourse.bass as bass
    import concourse.bacc as bacc
    import concourse.tile as tile
    from concourse import mybir, masks

    f32 = mybir.dt.float32
    bf16 = mybir.dt.bfloat16
    AF = mybir.ActivationFunctionType
    OP = mybir.AluOpType
    AX = mybir.AxisListType

    nc = bacc.Bacc(
        "TRN2",
        target_bir_lowering=False,
        debug=False,
        enable_asserts=False,
        num_devices=NCORES,
    )

    # ---------------- DRAM I/O ----------------
    xs_d = nc.dram_tensor("xs", [B * C, SH], f32, kind="ExternalInput")
    gl_d = nc.dram_tensor("gl", [1, SH], f32, kind="ExternalInput")
    gb1c_d = nc.dram_tensor("gb1c", [128, 3 * NST], bf16, kind="ExternalInput")
    gb1r_d = nc.dram_tensor("gb1r", [3, SH], bf16, kind="ExternalInput")
    eqt_d = nc.dram_tensor("eqt", [97, C], f32, kind="ExternalInput")
    ekt_d = nc.dram_tensor("ekt", [86, C], f32, kind="ExternalInput")
    w0_d = nc.dram_tensor("w0", [2 * 128, 87], bf16, kind="ExternalInput")
    bk_d = nc.dram_tensor("bk", [1, C], f32, kind="ExternalInput")
    sc_d = nc.dram_tensor("sc", [1, 8], f32, kind="ExternalInput")
    out_d = nc.dram_tensor("out", [B * C, SH], f32, kind="ExternalOutput")

    # Two bounce buffers so the first AllReduce (stats + batches 0,1)
    # overlaps stage-1 compute of batches 2,3.
    # Each PTK block is the [89, 89] merged matmul out:
    #   rows 0..2 = tghA^T (+3x3 junk corner), rows 3..88 = [P^T | tgh_K]
    PB = 89 * 89                   # 7921
    SX_OFF = 2 * PB                # in bncA
    SXX_OFF = SX_OFF + 1024
    TOT_A = SXX_OFF + 1024
    TOT_B = 2 * PB

    with tile.TileContext(nc) as tc, ExitStack() as ctx:
        const = ctx.enter_context(tc.tile_pool(name="const", bufs=1))
        xpool = ctx.enter_context(tc.tile_pool(name="xpool", bufs=1))
        upool = ctx.enter_context(tc.tile_pool(name="upool", bufs=1))
        utp = ctx.enter_context(tc.tile_pool(name="utp", bufs=4))
        rhsp = ctx.enter_context(tc.tile_pool(name="rhsp", bufs=2))
        osml = ctx.enter_context(tc.tile_pool(name="osml", bufs=2))
        small = ctx.enter_context(tc.tile_pool(name="small", bufs=2))
        dram = ctx.enter_context(tc.tile_pool(name="dram", bufs=1, space="DRAM"))

        # ------------- constants / inputs to SBUF -------------
        ident = const.tile([128, 128], f32)
        masks.make_identity(nc, ident[:])
        ident_bf = const.tile([128, 128], bf16)
        masks.make_identity(nc, ident_bf[:])
        gb1c_sb = const.tile([128, 3 * NST], bf16)
        nc.sync.dma_start(out=gb1c_sb[:], in_=gb1c_d.ap())
        eqt_sb = const.tile([97, C], f32)
        nc.sync.dma_start(out=eqt_sb[:], in_=eqt_d.ap())
        ekt_sb = const.tile([86, C], f32)
        nc.sync.dma_start(out=ekt_sb[:], in_=ekt_d.ap())
        w0_sb = const.tile([128, 2, 87], bf16)
        for jt in range(2):
            nc.sync.dma_start(out=w0_sb[:, jt, :], in_=w0_d[jt * 128:(jt + 1) * 128, :])
        def dram_bcast(dst, src_d, nparts, nfree):
            nc.gpsimd.dma_start(
                out=dst,
                in_=bass.AP(tensor=src_d, offset=0,
                            ap=[[0, nparts], [1, nfree]]))

        bk_bc = const.tile([128, C], f32)
        dram_bcast(bk_bc[:], bk_d, 128, C)
        sc_bc = const.tile([128, 8], f32)
        dram_bcast(sc_bc[:], sc_d, 128, 8)
        gam_bc = const.tile([128, SH], bf16)
        nc.gpsimd.dma_start(
            out=gam_bc[:],
            in_=bass.AP(tensor=gb1r_d, offset=SH,
                        ap=[[0, 128], [1, SH]]))
        nc.vector.tensor_scalar_mul(gam_bc[:], gam_bc[:], -1.0)

        # x resident: [128, 8, 4096] f32, tile t = rows t*128..t*128+127
        x_sb = xpool.tile([128, 8, SH], f32)
        for t in range(8):
            nc.sync.dma_start(out=x_sb[:, t, :], in_=xs_d[t * 128:(t + 1) * 128, :])

        # ------------- stage 1: stats via bn_stats -------------
        sums_sb = const.tile([128, 8], f32)
        sqs_sb = const.tile([128, 8], f32)
        for t in range(8):
            bnst = small.tile([128, 8, 6], f32, tag="bnst", bufs=1)
            for sub in range(8):
                nc.vector.bn_stats(
                    out=bnst[:, sub, :],
                    in_=x_sb[:, t, sub * 512:(sub + 1) * 512])
            aggr = small.tile([128, 2], f32, tag="aggr")
            nc.vector.bn_aggr(out=aggr[:], in_=bnst[:])
            nc.vector.tensor_scalar_mul(
                sums_sb[:, t:t + 1], aggr[:, 0:1], float(SH))
            tmp1 = small.tile([128, 1], f32, tag="tmp1")
            nc.vector.tensor_mul(tmp1[:], aggr[:, 0:1], aggr[:, 0:1])
            nc.vector.tensor_add(tmp1[:], tmp1[:], aggr[:, 1:2])
            nc.vector.tensor_scalar_mul(
                sqs_sb[:, t:t + 1], tmp1[:], float(SH))

        # ------------- stage 1: Gram + tgh per batch -------------
        bncA_in = dram.tile([TOT_A], f32)
        bncA_out = dram.tile([TOT_A], f32, addr_space="Shared")
        bncB_in = dram.tile([TOT_B], f32)
        bncB_out = dram.tile([TOT_B], f32, addr_space="Shared")

        u1s = []
        with tc.tile_pool(name="s1ps", bufs=2, space="PSUM") as stg1ps:
            for b in range(B):
                u0 = upool.tile([128, SH], bf16, tag="u0")
                # u1 covers the FULL second ctile (gamma*x, ch 128..255):
                # rows 0..42 feed the Gram transposes, rows 42..127 are
                # gamma*x_V reused for the M2 rhs (partition-shift DMA).
                u1 = upool.tile([128, SH], bf16, tag="u1", bufs=2)
                nc.vector.tensor_tensor(
                    out=u0[:], in0=x_sb[:, 2 * b, :], in1=gam_bc[:], op=OP.mult)
                nc.vector.tensor_tensor(
                    out=u1[:], in0=x_sb[:, 2 * b + 1, :],
                    in1=gam_bc[:], op=OP.mult)
                u1s.append(u1)

                ptk_ps = stg1ps.tile([89, 89], f32, tag="ptkps")
                for st in range(NST):
                    # PE transpose into PSUM bf16, copy back into the ut
                    # layout [A(86) | gamma beta 1 (3) | K(86)], then ONE
                    # matmul (lhsT = [gb1|K], rhs = [A|gb1]) produces
                    # tghA^T, P^T and tgh_K together in [89, 89].
                    tps = stg1ps.tile([128, 176], bf16, tag="tps", bufs=4)
                    ut = utp.tile([128, 175], bf16, name=f"ut_{b}_{st}", tag="ut")
                    sl = slice(st * 128, (st + 1) * 128)
                    nc.tensor.transpose(tps[:, 0:128], u0[:, sl], ident_bf[:])
                    nc.tensor.transpose(tps[:, 128:176], u1[0:48, sl],
                                        ident_bf[0:48, 0:48])
                    nc.scalar.copy(ut[:, 0:86], tps[:, 0:86])
                    nc.scalar.copy(ut[:, 89:175], tps[:, 85:171])
                    nc.vector.tensor_copy(ut[:, 86:89],
                                          gb1c_sb[:, 3 * st:3 * st + 3])
                    nc.tensor.matmul(
                        ptk_ps[:], lhsT=ut[:, 86:175], rhs=ut[:, 0:89],
                        start=(st == 0), stop=(st == NST - 1))

                ptk_sb = small.tile([89, 89], f32, tag="ptksb", bufs=1)
                nc.scalar.copy(ptk_sb[:], ptk_ps[:])

                bnc = bncA_in if b < 2 else bncB_in
                po = (b % 2) * PB
                nc.gpsimd.dma_start(
                    out=bnc[po:po + PB].rearrange("(p f) -> p f", f=89),
                    in_=ptk_sb[:])
                if b == 1:
                    nc.gpsimd.dma_start(
                        out=bncA_in[SX_OFF:SX_OFF + 1024].rearrange(
                            "(t p) -> p t", p=128),
                        in_=sums_sb[:])
                    nc.gpsimd.dma_start(
                        out=bncA_in[SXX_OFF:SXX_OFF + 1024].rearrange(
                            "(t p) -> p t", p=128),
                        in_=sqs_sb[:])
                    nc.gpsimd.collective_compute(
                        "AllReduce", OP.add,
                        replica_groups=[list(range(NCORES))],
                        ins=[bncA_in[:].opt()], outs=[bncA_out[:].opt()])

            nc.gpsimd.collective_compute(
                "AllReduce", OP.add,
                replica_groups=[list(range(NCORES))],
                ins=[bncB_in[:].opt()], outs=[bncB_out[:].opt()])

        # ------------- DMA back -------------
        pt_back = const.tile([86, B, 86], f32)
        tga_back = const.tile([86, B, 3], f32)   # A-side: ch 0..85
        tgk_back = const.tile([86, B, 3], f32)   # K-side: ch 85..170
        for b in range(B):
            bout = bncA_out if b < 2 else bncB_out
            po = (b % 2) * PB
            nc.sync.dma_start(
                out=pt_back[:, b, :],
                in_=bass.AP(tensor=bout.tensor,
                            offset=bout.offset + po + 3 * 89,
                            ap=[[89, 86], [1, 86]]))
            nc.sync.dma_start(
                out=tgk_back[:, b, :],
                in_=bass.AP(tensor=bout.tensor,
                            offset=bout.offset + po + 3 * 89 + 86,
                            ap=[[89, 86], [1, 3]]))
            nc.sync.dma_start(
                out=tga_back[:, b, :],
                in_=bass.AP(tensor=bout.tensor,
                            offset=bout.offset + po,
                            ap=[[1, 86], [89, 3]]))
        # stats columns, all at partitions 0..85: [p, {Sx,Sxx}, {A,K,V}, b]
        sAK = const.tile([86, 2, 3, B], f32)
        for k, koff in ((0, SX_OFF), (1, SXX_OFF)):
            for g, goff in ((0, 0), (1, 85), (2, 170)):
                nc.sync.dma_start(
                    out=sAK[:, k, g, :],
                    in_=bass.AP(tensor=bncA_out.tensor,
                                offset=bncA_out.offset + koff + goff,
                                ap=[[1, 86], [256, B]]))

        # ------------- stage 2/3 (phase-interleaved in pairs) -------------
        psA = ctx.enter_context(tc.tile_pool(name="psA", bufs=2, space="PSUM"))
        psB = ctx.enter_context(tc.tile_pool(name="psB", bufs=3, space="PSUM"))
        psC = ctx.enter_context(tc.tile_pool(name="psC", bufs=3, space="PSUM"))

        invS = 1.0 / float(S)
        st2 = [dict() for _ in range(B)]

        def phase_rhs(b):
            # rhs_M2 [128, SH] bf16: rows 0..85 = gamma*x_V from u1 via a
            # DRAM round-trip (a direct SBUF->SBUF DMA would deadlock
            # against concurrent xbar transposes), rows 86..88 =
            # [ones, -gamma, beta]
            rhs_m2 = rhsp.tile([128, SH], bf16, tag="rhsm2", name=f"rhs{b}")
            nc.gpsimd.dma_start(out=rhs_m2[0:86, :], in_=u1s[b][42:128, :])
            nc.gpsimd.dma_start(out=rhs_m2[86:89, :], in_=gb1r_d.ap())
            st2[b]["rhs"] = rhs_m2

        def phase_vec(b):
            s = st2[b]
            mAK = small.tile([86, 3], f32, tag="mAK", name=f"mAK{b}")
            nc.vector.tensor_scalar(
                out=mAK[:], in0=sAK[:, 0, :, b], scalar1=invS, scalar2=None,
                op0=OP.mult)
            vAK = small.tile([86, 3], f32, tag="vAK", name=f"vAK{b}")
            nc.vector.tensor_scalar(
                out=vAK[:], in0=sAK[:, 1, :, b], scalar1=invS, scalar2=EPS,
                op0=OP.mult, op1=OP.add)
            msq = small.tile([86, 3], f32, tag="msq", name=f"msq{b}")
            nc.vector.tensor_mul(msq[:], mAK[:], mAK[:])
            nc.vector.tensor_sub(vAK[:], vAK[:], msq[:])
            nc.scalar.activation(out=vAK[:], in_=vAK[:], func=AF.Sqrt)
            rAK = small.tile([86, 3], f32, tag="rAK", name=f"rAK{b}")
            nc.vector.reciprocal(rAK[:], vAK[:])
            invrV = small.tile([86, 1], f32, tag="invrV", name=f"invrV{b}")
            nc.vector.reciprocal(invrV[:], rAK[:, 2:3])
            mvinv_bf = small.tile([86, 2], bf16, tag="mvinv", name=f"mvinv{b}")
            nc.vector.tensor_copy(mvinv_bf[:, 0:1], mAK[:, 2:3])
            nc.vector.tensor_copy(mvinv_bf[:, 1:2], invrV[:])
            rv_ext = small.tile([128, 1], f32, tag="rvext", name=f"rvext{b}")
            nc.vector.memset(rv_ext[64:128, :], 1.0)
            nc.vector.tensor_copy(rv_ext[0:86, :], rAK[:, 2:3])
            s["mAK"], s["rAK"] = mAK, rAK
            s["mvinv"], s["rvext"] = mvinv_bf, rv_ext

            tA = tga_back[:, b, 0:1]
            gA = tga_back[:, b, 1:2]
            hA = tga_back[:, b, 2:3]
            tK = tgk_back[:, b, 0:1]
            gK = tgk_back[:, b, 1:2]
            hK = tgk_back[:, b, 2:3]
            mA, mK = mAK[:, 0:1], mAK[:, 1:2]
            rA, rK = rAK[:, 0:1], rAK[:, 1:2]
            scG1 = sc_bc[0:86, 0:1]
            scG2 = sc_bc[0:86, 1:2]
            scGb = sc_bc[0:86, 2:3]
            scB1 = sc_bc[0:86, 3:4]

            ntK = small.tile([86, 1], f32, tag="ntK", name=f"ntK{b}")
            nc.vector.tensor_scalar_mul(ntK[:], tK, -1.0)
            nmK = small.tile([86, 1], f32, tag="nmK", name=f"nmK{b}")
            nc.vector.tensor_scalar_mul(nmK[:], mK, -1.0)
            g2mK = small.tile([86, 1], f32, tag="g2mK", name=f"g2mK{b}")
            nc.vector.tensor_scalar(
                out=g2mK[:], in0=mK, scalar1=scG2, scalar2=None, op0=OP.mult)
            t3c = small.tile([86, 1], f32, tag="t3c", name=f"t3c{b}")
            nc.vector.tensor_scalar(
                out=t3c[:], in0=mK, scalar1=scGb, scalar2=None, op0=OP.mult)
            nc.vector.tensor_sub(t3c[:], gK, t3c[:])
            nc.vector.tensor_mul(t3c[:], rK, t3c[:])
            t2c = small.tile([86, 1], f32, tag="t2c", name=f"t2c{b}")
            nc.vector.tensor_scalar(
                out=t2c[:], in0=mA, scalar1=scGb, scalar2=None, op0=OP.mult)
            nc.vector.tensor_sub(t2c[:], gA, t2c[:])
            nc.vector.tensor_mul(t2c[:], rA, t2c[:])
            syA = small.tile([86, 1], f32, tag="syA", name=f"syA{b}")
            nc.vector.tensor_scalar(
                out=syA[:], in0=mA, scalar1=scG1, scalar2=None, op0=OP.mult)
            nc.vector.tensor_sub(syA[:], hA, syA[:])
            nc.vector.tensor_mul(syA[:], rA, syA[:])
            nc.vector.tensor_scalar(
                out=syA[:], in0=syA[:], scalar1=scB1, scalar2=None, op0=OP.add)
            syK = small.tile([86, 1], f32, tag="syK", name=f"syK{b}")
            nc.vector.tensor_scalar(
                out=syK[:], in0=mK, scalar1=scG1, scalar2=None, op0=OP.mult)
            nc.vector.tensor_sub(syK[:], hK, syK[:])
            nc.vector.tensor_mul(syK[:], rK, syK[:])
            nc.vector.tensor_scalar(
                out=syK[:], in0=syK[:], scalar1=scB1, scalar2=None, op0=OP.add)
            s["ntK"], s["nmK"], s["g2mK"] = ntK, nmK, g2mK
            s["t3c"], s["syA"], s["syK"] = t3c, syA, syK

            # rows (mA, tA, rA, term2) -> transpose -> DRAM -> one bcast DMA
            pack = small.tile([86, 4], f32, tag="pack", name=f"pack{b}")
            nc.vector.tensor_copy(pack[:, 0:1], mA)
            nc.vector.tensor_copy(pack[:, 1:2], tA)
            nc.vector.tensor_copy(pack[:, 2:3], rA)
            nc.vector.tensor_copy(pack[:, 3:4], t2c[:])
            packT_ps = psA.tile([4, 86], f32, tag="psA", name=f"pT{b}")
            nc.tensor.transpose(packT_ps[:], pack[:], ident[0:86, 0:86])
            packT = small.tile([4, 86], f32, tag="packT", name=f"packT{b}")
            nc.scalar.copy(packT[:], packT_ps[:])
            rows_d = dram.tile([4, 86], f32, name=f"rowsd{b}", tag="rowsd",
                               bufs=2)
            nc.gpsimd.dma_start(out=rows_d[:], in_=packT[:])
            bc4 = small.tile([86, 4, 86], f32, tag="bc4", name=f"bc4{b}")
            nc.gpsimd.dma_start(
                out=bc4[:],
                in_=bass.AP(tensor=rows_d.tensor, offset=rows_d.offset,
                            ap=[[0, 86], [86, 4], [1, 86]]))
            s["bc4"] = bc4

        def phase_syy(b):
            s = st2[b]
            bc4 = s["bc4"]
            rK = s["rAK"][:, 1:2]
            scBb = sc_bc[0:86, 4:5]
            syy = small.tile([86, 97], f32, tag="syy", name=f"syy{b}")
            nc.vector.memset(syy[:, 86:96], 0.0)
            nc.vector.scalar_tensor_tensor(
                out=syy[:, 0:86], in0=bc4[:, 0, :], scalar=s["ntK"][:],
                in1=pt_back[:, b, :], op0=OP.mult, op1=OP.add)
            nc.vector.scalar_tensor_tensor(
                out=syy[:, 0:86], in0=bc4[:, 1, :], scalar=s["nmK"][:],
                in1=syy[:, 0:86], op0=OP.mult, op1=OP.add)
            nc.vector.scalar_tensor_tensor(
                out=syy[:, 0:86], in0=bc4[:, 0, :], scalar=s["g2mK"][:],
                in1=syy[:, 0:86], op0=OP.mult, op1=OP.add)
            nc.vector.scalar_tensor_tensor(
                out=syy[:, 0:86], in0=bc4[:, 2, :], scalar=rK,
                in1=syy[:, 0:86], op0=OP.mult, op1=OP.mult)
            nc.vector.tensor_add(syy[:, 0:86], syy[:, 0:86], bc4[:, 3, :])
            nc.vector.tensor_scalar(
                out=syy[:, 0:86], in0=syy[:, 0:86], scalar1=s["t3c"][:],
                scalar2=scBb, op0=OP.add, op1=OP.add)
            nc.vector.tensor_copy(syy[:, 96:97], s["syK"][:])
            s["syy"] = syy

        def phase_logits(b):
            s = st2[b]
            u_ps = psA.tile([97, C], f32, tag="psA", name=f"ups{b}")
            nc.tensor.matmul(u_ps[:], lhsT=s["syy"][:], rhs=ekt_sb[:],
                             start=True, stop=True)
            u_ext = small.tile([128, C], f32, tag="uext", name=f"uext{b}")
            nc.vector.memset(u_ext[64:128, :], 0.0)
            nc.vector.scalar_tensor_tensor(
                out=u_ext[0:86, :], in0=bk_bc[0:86, :], scalar=s["syA"][:],
                in1=u_ps[0:86, :], op0=OP.mult, op1=OP.add)
            nc.vector.tensor_scalar_mul(
                u_ext[96:97, :], bk_bc[96:97, :], float(S))
            nc.vector.tensor_add(u_ext[96:97, :], u_ext[96:97, :],
                                 u_ps[96:97, :])

            att_sb = []
            recip2 = small.tile([128, 2], f32, tag="recip2", name=f"re{b}")
            z2 = small.tile([128, 2], f32, tag="z2", name=f"z2{b}")
            for it in range(2):
                log_ps = psB.tile([128, 512], f32, tag="psB", name=f"lg{b}{it}")
                nc.tensor.matmul(
                    log_ps[:, 0:C], lhsT=eqt_sb[:, it * 128:(it + 1) * 128],
                    rhs=u_ext[0:97, :], start=True, stop=True)
                rmax = small.tile([128, 1], f32, tag="rmax", name=f"rm{b}{it}")
                nc.vector.reduce_max(rmax[:], log_ps[:, 0:C], axis=AX.X)
                nbias = small.tile([128, 1], f32, tag="nbias",
                                   name=f"nb{b}{it}")
                nc.vector.tensor_scalar_mul(nbias[:], rmax[:], -SCALE)
                a_sb = small.tile([128, C], f32, tag=f"attsb{it}",
                                  name=f"att{b}{it}")
                nc.scalar.activation(
                    out=a_sb[:], in_=log_ps[:, 0:C], func=AF.Exp,
                    bias=nbias[:], scale=SCALE, accum_out=z2[:, it:it + 1])
                nc.vector.reciprocal(recip2[:, it:it + 1], z2[:, it:it + 1])
                att_sb.append(a_sb)
            s["att"], s["recip2"] = att_sb, recip2

        def phase_nt(b):
            s = st2[b]
            ntc_ps = psC.tile([128, C], f32, tag="psC", name=f"ntc{b}")
            for jt in range(2):
                at_ps = psC.tile([128, C], f32, tag="psC", name=f"atp{b}{jt}")
                for it in range(2):
                    nc.tensor.transpose(
                        at_ps[:, it * 128:(it + 1) * 128],
                        s["att"][it][:, jt * 128:(jt + 1) * 128],
                        ident[:])
                at_bf = small.tile([128, C], bf16, tag=f"atbf{jt}",
                                   name=f"atb{b}{jt}")
                nc.scalar.copy(at_bf[:], at_ps[:])
                nc.tensor.matmul(
                    ntc_ps[0:87, :], lhsT=w0_sb[:, jt, :], rhs=at_bf[:],
                    start=(jt == 0), stop=(jt == 1))

            # lhsT_M2 [128, C] bf16: rows 0..85=NR, 86=cv, 87=c1, 88=c2.
            # rv_ext has 1.0 at row 86 so cv copies through unscaled.
            lhs_m2 = small.tile([128, C], bf16, tag="lhsm2", name=f"lm{b}")
            rv = s["rvext"]
            nc.scalar.activation(
                out=lhs_m2[0:64, :], in_=ntc_ps[0:64, :], func=AF.Copy,
                scale=rv[0:64, :])
            nc.scalar.activation(
                out=lhs_m2[64:87, :], in_=ntc_ps[64:87, :], func=AF.Copy,
                scale=rv[64:87, :])
            nc.tensor.matmul(
                ntc_ps[64:66, :], lhsT=s["mvinv"][:],
                rhs=lhs_m2[0:86, :], start=True, stop=True)
            c12_sb = small.tile([128, C], bf16, tag="c12sb", name=f"c12{b}")
            nc.scalar.copy(c12_sb[64:66, :], ntc_ps[64:66, :])
            nc.gpsimd.dma_start(out=lhs_m2[87:89, :], in_=c12_sb[64:66, :])
            s["lhs_m2"] = lhs_m2

        def phase_m2(b):
            s = st2[b]
            lhs_m2, rhs_m2, recip2 = s["lhs_m2"], s["rhs"], s["recip2"]
            for it in range(2):
                for ch in range(8):
                    ostg = osml.tile([128, 512], f32, tag="ostg", bufs=3,
                                     name=f"o{b}{it}{ch}")
                    o_ps = psB.tile([128, 512], f32, tag="psB",
                                    name=f"op{b}{it}{ch}")
                    nc.tensor.matmul(
                        o_ps[:],
                        lhsT=lhs_m2[0:89, it * 128:(it + 1) * 128],
                        rhs=rhs_m2[0:89, ch * 512:(ch + 1) * 512],
                        start=True, stop=True)
                    nc.vector.scalar_tensor_tensor(
                        out=ostg[:], in0=o_ps[:],
                        scalar=recip2[:, it:it + 1],
                        in1=x_sb[:, 2 * b + it, ch * 512:(ch + 1) * 512],
                        op0=OP.mult, op1=OP.add)
                    nc.sync.dma_start(
                        out=out_d[(2 * b + it) * 128:(2 * b + it + 1) * 128,
                                  ch * 512:(ch + 1) * 512],
                        in_=ostg[:])

        phases = [phase_rhs, phase_vec, phase_syy, phase_logits, phase_nt,
                  phase_m2]
        for pair in ((0, 1), (2, 3)):
            for ph in phases:
                for b in pair:
                    ph(b)

    nc.compile()
    return nc


def _host_prep(x, gamma, beta, w_qkv, b_qkv):
    xf = np.ascontiguousarray(np.asarray(x, np.float32).reshape(B * C, S))
    gam = np.asarray(gamma, np.float32).reshape(-1)
    bet = np.asarray(beta, np.float32).reshape(-1)
    w_qkv = np.asarray(w_qkv, np.float32)
    b_qkv = np.asarray(b_qkv, np.float32)
    w_q, w_k, w_v = w_qkv[:C], w_qkv[C:2 * C], w_qkv[2 * C:]
    b_q, b_k, b_v = b_qkv[:C], b_qkv[C:2 * C], b_qkv[2 * C:]

    ii = np.arange(C)
    eqt = np.zeros((97, C), np.float32)
    eqt[ii // 3, ii] = w_q
    eqt[96] = b_q
    ekt = np.zeros((86, C), np.float32)
    ekt[(C + ii) // 3 - 85, ii] = w_k
    w0 = np.zeros((C, 87), np.float32)
    w0[ii, (2 * C + ii) // 3 - 170] = w_v
    w0[:, 86] = b_v
    w0 = w0.astype(_BF)

    sc = np.zeros((1, 8), np.float32)
    sc[0, :5] = [gam.sum(), (gam * gam).sum(), (gam * bet).sum(),
                 bet.sum(), (bet * bet).sum()]

    in_maps = []
    for r in range(NCORES):
        sl = slice(r * SH, (r + 1) * SH)
        gl = gam[sl]
        bl = bet[sl]
        gb1c = np.empty((128, 3 * NST), np.float32)
        for st in range(NST):
            gb1c[:, 3 * st] = gl[st * 128:(st + 1) * 128]
            gb1c[:, 3 * st + 1] = bl[st * 128:(st + 1) * 128]
            gb1c[:, 3 * st + 2] = 1.0
        gb1r = np.stack([np.ones(SH, np.float32), -gl, bl], 0)
        in_maps.append({
            "xs": np.ascontiguousarray(xf[:, sl]),
            "gl": gl.reshape(1, SH).copy(),
            "gb1c": gb1c.astype(_BF),
            "gb1r": gb1r.astype(_BF),
            "eqt": eqt,
            "ekt": ekt,
            "w0": w0,
            "bk": b_k.reshape(1, C).copy(),
            "sc": sc,
        })
    return in_maps


def kernel(x, gamma, beta, w_qkv, b_qkv):
    from concourse.bass_utils import run_bass_kernel_spmd

    if "nc" not in _cache:
        _cache["nc"] = _build_program()
    nc = _cache["nc"]

    in_maps = _host_prep(x, gamma, beta, w_qkv, b_qkv)
    res = run_bass_kernel_spmd(nc, in_maps, core_ids=list(range(NCORES)))
    out = np.empty((B * C, S), np.float32)
    for r in range(NCORES):
        out[:, r * SH:(r + 1) * SH] = res.results[r]["out"]
    return out.reshape(np.asarray(x).shape)


if __name__ == "__main__":
    rng = np.random.default_rng(0)
    inputs = {
        "x": rng.standard_normal((B, C, 32, 32, 32)).astype(np.float32),
        "gamma": (1 + 0.1 * rng.standard_normal((32, 32, 32))).astype(np.float32),
        "beta": (0.1 * rng.standard_normal((32, 32, 32))).astype(np.float32),
        "w_qkv": (0.5 * rng.standard_normal(3 * C)).astype(np.float32),
        "b_qkv": (0.05 * rng.standard_normal(3 * C)).astype(np.float32),
    }
    o = kernel(**inputs)
    print("out", o.shape, o.dtype, float(np.abs(o).mean()))



# revision 6
# speedup vs baseline: 1.1630x; 1.1630x over previous
"""Channel-self-attention (LayerNorm + grouped-1x1-qkv + channel softmax attn
+ residual) on 8 TRN2 NeuronCores.

Strategy: shard the spatial axis (32^3 = 32768 -> 4096/core). Per core:
 - keep the x-shard [1024, 4096] f32 resident in SBUF
 - local stats (Sum x, Sum x^2) via DVE bn_stats/bn_aggr
 - u = gamma*x (ch 0..170 per batch) cast bf16, DMA-xbar-transpose SBUF->SBUF,
   PE Gram P^T[kap,a] = Sum_s u_{85+kap} u_a and t/g/h = u^T @ [gamma,beta,1]
 - ONE AllReduce (~132 KB) of (P^T, tgh, stats)
 - logits rebuilt from the Gram expansion of the LayerNorm algebra, softmax,
   apply att via one [89-row] matmul against [-gamma; beta; 1; gamma*x_V]
 - out = x + recip * PS  (softmax division folded into the epilogue)
"""
import sys

sys.path.insert(0, "/opt/trn_rl_repo")

import numpy as np
import ml_dtypes

B, C = 4, 256
S = 32 * 32 * 32          # 32768
NCORES = 8
SH = S // NCORES          # 4096 per-core spatial shard
NST = SH // 128           # 32 stiles
EPS = 1e-5
SCALE = float(S) ** -0.5

_BF = ml_dtypes.bfloat16

_cache = {}


def _build_program():
    """Trace the Bass/Tile program once; returns the compiled Bacc."""
    from contextlib import ExitStack
    import concourse.bass as bass
    import concourse.bacc as bacc
    import concourse.tile as tile
    from concourse import mybir, masks

    f32 = mybir.dt.float32
    bf16 = mybir.dt.bfloat16
    AF = mybir.ActivationFunctionType
    OP = mybir.AluOpType
    AX = mybir.AxisListType

    nc = bacc.Bacc(
        "TRN2",
        target_bir_lowering=False,
        debug=False,
        enable_asserts=False,
        num_devices=NCORES,
    )

    # ---------------- DRAM I/O ----------------
    xs_d = nc.dram_tensor("xs", [B * C, SH], f32, kind="ExternalInput")
    gl_d = nc.dram_tensor("gl", [1, SH], f32, kind="ExternalInput")
    gb1c_d = nc.dram_tensor("gb1c", [128, 3 * NST], bf16, kind="ExternalInput")
    gb1r_d = nc.dram_tensor("gb1r", [3, SH], bf16, kind="ExternalInput")
    eqt_d = nc.dram_tensor("eqt", [97, C], f32, kind="ExternalInput")
    ekt_d = nc.dram_tensor("ekt", [86, C], f32, kind="ExternalInput")
    w0_d = nc.dram_tensor("w0", [2 * 128, 87], bf16, kind="ExternalInput")
    bk_d = nc.dram_tensor("bk", [1, C], f32, kind="ExternalInput")
    sc_d = nc.dram_tensor("sc", [1, 8], f32, kind="ExternalInput")
    out_d = nc.dram_tensor("out", [B * C, SH], f32, kind="ExternalOutput")

    # Five small AllReduces, each < 64 KB so the runtime picks the O(1)-hop
    # Mesh algorithm (empirically RDH kicks in above 64 KB and is ~7x
    # slower): stats (8 KB) fired right after the stats loop, then one
    # 31.7 KB AR per batch Gram, fired as soon as that Gram lands.
    # Each PTK block is the [89, 89] merged matmul out:
    #   rows 0..2 = tghA^T (+3x3 junk corner), rows 3..88 = [P^T | tgh_K]
    PB = 89 * 89                   # 7921
    SX_OFF = 0                     # in bncS
    SXX_OFF = 1024

    with tile.TileContext(nc) as tc, ExitStack() as ctx:
        const = ctx.enter_context(tc.tile_pool(name="const", bufs=1))
        xpool = ctx.enter_context(tc.tile_pool(name="xpool", bufs=1))
        upool = ctx.enter_context(tc.tile_pool(name="upool", bufs=1))
        utp = ctx.enter_context(tc.tile_pool(name="utp", bufs=4))
        rhsp = ctx.enter_context(tc.tile_pool(name="rhsp", bufs=2))
        osml = ctx.enter_context(tc.tile_pool(name="osml", bufs=2))
        small = ctx.enter_context(tc.tile_pool(name="small", bufs=2))
        dram = ctx.enter_context(tc.tile_pool(name="dram", bufs=1, space="DRAM"))

        # ------------- constants / inputs to SBUF -------------
        ident = const.tile([128, 128], f32)
        masks.make_identity(nc, ident[:])
        ident_bf = const.tile([128, 128], bf16)
        masks.make_identity(nc, ident_bf[:])
        gb1c_sb = const.tile([128, 3 * NST], bf16)
        nc.sync.dma_start(out=gb1c_sb[:], in_=gb1c_d.ap())
        eqt_sb = const.tile([97, C], f32)
        nc.sync.dma_start(out=eqt_sb[:], in_=eqt_d.ap())
        ekt_sb = const.tile([86, C], f32)
        nc.sync.dma_start(out=ekt_sb[:], in_=ekt_d.ap())
        w0_sb = const.tile([128, 2, 87], bf16)
        for jt in range(2):
            nc.sync.dma_start(out=w0_sb[:, jt, :], in_=w0_d[jt * 128:(jt + 1) * 128, :])
        def dram_bcast(dst, src_d, nparts, nfree):
            nc.gpsimd.dma_start(
                out=dst,
                in_=bass.AP(tensor=src_d, offset=0,
                            ap=[[0, nparts], [1, nfree]]))

        bk_bc = const.tile([128, C], f32)
        dram_bcast(bk_bc[:], bk_d, 128, C)
        sc_bc = const.tile([128, 8], f32)
        dram_bcast(sc_bc[:], sc_d, 128, 8)
        gam_bc = const.tile([128, SH], bf16)
        nc.gpsimd.dma_start(
            out=gam_bc[:],
            in_=bass.AP(tensor=gb1r_d, offset=SH,
                        ap=[[0, 128], [1, SH]]))
        nc.vector.tensor_scalar_mul(gam_bc[:], gam_bc[:], -1.0)

        # x resident: [128, 8, 4096] f32, tile t = rows t*128..t*128+127
        x_sb = xpool.tile([128, 8, SH], f32)
        for t in range(8):
            nc.sync.dma_start(out=x_sb[:, t, :], in_=xs_d[t * 128:(t + 1) * 128, :])

        # ------------- stage 1: stats via bn_stats -------------
        sums_sb = const.tile([128, 8], f32)
        sqs_sb = const.tile([128, 8], f32)
        for t in range(8):
            bnst = small.tile([128, 8, 6], f32, tag="bnst", bufs=1)
            for sub in range(8):
                nc.vector.bn_stats(
                    out=bnst[:, sub, :],
                    in_=x_sb[:, t, sub * 512:(sub + 1) * 512])
            aggr = small.tile([128, 2], f32, tag="aggr")
            nc.vector.bn_aggr(out=aggr[:], in_=bnst[:])
            nc.vector.tensor_scalar_mul(
                sums_sb[:, t:t + 1], aggr[:, 0:1], float(SH))
            tmp1 = small.tile([128, 1], f32, tag="tmp1")
            nc.vector.tensor_mul(tmp1[:], aggr[:, 0:1], aggr[:, 0:1])
            nc.vector.tensor_add(tmp1[:], tmp1[:], aggr[:, 1:2])
            nc.vector.tensor_scalar_mul(
                sqs_sb[:, t:t + 1], tmp1[:], float(SH))

        # ------------- stage 1: Gram + tgh per batch -------------
        bncS_in = dram.tile([2048], f32)
        bncS_out = dram.tile([2048], f32, addr_space="Shared")
        bncG_in = [dram.tile([PB], f32, name=f"gin{b}") for b in range(B)]
        bncG_out = [dram.tile([PB], f32, name=f"gout{b}", addr_space="Shared")
                    for b in range(B)]

        # stats AR: fired before any Gram work completes
        nc.gpsimd.dma_start(
            out=bncS_in[SX_OFF:SX_OFF + 1024].rearrange("(t p) -> p t", p=128),
            in_=sums_sb[:])
        nc.gpsimd.dma_start(
            out=bncS_in[SXX_OFF:SXX_OFF + 1024].rearrange(
                "(t p) -> p t", p=128),
            in_=sqs_sb[:])
        nc.gpsimd.collective_compute(
            "AllReduce", OP.add,
            replica_groups=[list(range(NCORES))],
            ins=[bncS_in[:].opt()], outs=[bncS_out[:].opt()])

        u1s = []
        with tc.tile_pool(name="s1ps", bufs=2, space="PSUM") as stg1ps:
            for b in range(B):
                u0 = upool.tile([128, SH], bf16, tag="u0")
                # u1 covers the FULL second ctile (gamma*x, ch 128..255):
                # rows 0..42 feed the Gram transposes, rows 42..127 are
                # gamma*x_V reused for the M2 rhs (partition-shift DMA).
                u1 = upool.tile([128, SH], bf16, tag="u1", bufs=2)
                nc.vector.tensor_tensor(
                    out=u0[:], in0=x_sb[:, 2 * b, :], in1=gam_bc[:], op=OP.mult)
                nc.vector.tensor_tensor(
                    out=u1[:], in0=x_sb[:, 2 * b + 1, :],
                    in1=gam_bc[:], op=OP.mult)
                u1s.append(u1)

                ptk_ps = stg1ps.tile([89, 89], f32, tag="ptkps")
                for st in range(NST):
                    # PE transpose into PSUM bf16, copy back into the ut
                    # layout [A(86) | gamma beta 1 (3) | K(86)], then ONE
                    # matmul (lhsT = [gb1|K], rhs = [A|gb1]) produces
                    # tghA^T, P^T and tgh_K together in [89, 89].
                    tps = stg1ps.tile([128, 176], bf16, tag="tps", bufs=4)
                    ut = utp.tile([128, 175], bf16, name=f"ut_{b}_{st}", tag="ut")
                    sl = slice(st * 128, (st + 1) * 128)
                    nc.tensor.transpose(tps[:, 0:128], u0[:, sl], ident_bf[:])
                    nc.tensor.transpose(tps[:, 128:176], u1[0:48, sl],
                                        ident_bf[0:48, 0:48])
                    nc.scalar.copy(ut[:, 0:86], tps[:, 0:86])
                    nc.scalar.copy(ut[:, 89:175], tps[:, 85:171])
                    nc.vector.tensor_copy(ut[:, 86:89],
                                          gb1c_sb[:, 3 * st:3 * st + 3])
                    nc.tensor.matmul(
                        ptk_ps[:], lhsT=ut[:, 86:175], rhs=ut[:, 0:89],
                        start=(st == 0), stop=(st == NST - 1))

                ptk_sb = small.tile([89, 89], f32, tag="ptksb", bufs=1)
                nc.scalar.copy(ptk_sb[:], ptk_ps[:])

                nc.gpsimd.dma_start(
                    out=bncG_in[b][:].rearrange("(p f) -> p f", f=89),
                    in_=ptk_sb[:])
                nc.gpsimd.collective_compute(
                    "AllReduce", OP.add,
                    replica_groups=[list(range(NCORES))],
                    ins=[bncG_in[b][:].opt()], outs=[bncG_out[b][:].opt()])

        # ------------- DMA back -------------
        pt_back = const.tile([86, B, 86], f32)
        tga_back = const.tile([86, B, 3], f32)   # A-side: ch 0..85
        tgk_back = const.tile([86, B, 3], f32)   # K-side: ch 85..170
        for b in range(B):
            bout = bncG_out[b]
            nc.sync.dma_start(
                out=pt_back[:, b, :],
                in_=bass.AP(tensor=bout.tensor,
                            offset=bout.offset + 3 * 89,
                            ap=[[89, 86], [1, 86]]))
            nc.sync.dma_start(
                out=tgk_back[:, b, :],
                in_=bass.AP(tensor=bout.tensor,
                            offset=bout.offset + 3 * 89 + 86,
                            ap=[[89, 86], [1, 3]]))
            nc.sync.dma_start(
                out=tga_back[:, b, :],
                in_=bass.AP(tensor=bout.tensor,
                            offset=bout.offset,
                            ap=[[1, 86], [89, 3]]))
        # stats columns, all at partitions 0..85: [p, {Sx,Sxx}, {A,K,V}, b]
        sAK = const.tile([86, 2, 3, B], f32)
        for k, koff in ((0, SX_OFF), (1, SXX_OFF)):
            for g, goff in ((0, 0), (1, 85), (2, 170)):
                nc.sync.dma_start(
                    out=sAK[:, k, g, :],
                    in_=bass.AP(tensor=bncS_out.tensor,
                                offset=bncS_out.offset + koff + goff,
                                ap=[[1, 86], [256, B]]))

        # ------------- stage 2/3 (phase-interleaved in pairs) -------------
        psA = ctx.enter_context(tc.tile_pool(name="psA", bufs=2, space="PSUM"))
        psB = ctx.enter_context(tc.tile_pool(name="psB", bufs=3, space="PSUM"))
        psC = ctx.enter_context(tc.tile_pool(name="psC", bufs=3, space="PSUM"))

        invS = 1.0 / float(S)
        st2 = [dict() for _ in range(B)]

        def phase_rhs(b):
            # rhs_M2 [128, SH] bf16: rows 0..85 = gamma*x_V from u1 via a
            # DRAM round-trip (a direct SBUF->SBUF DMA would deadlock
            # against concurrent xbar transposes), rows 86..88 =
            # [ones, -gamma, beta]
            rhs_m2 = rhsp.tile([128, SH], bf16, tag="rhsm2", name=f"rhs{b}")
            nc.gpsimd.dma_start(out=rhs_m2[0:86, :], in_=u1s[b][42:128, :])
            nc.gpsimd.dma_start(out=rhs_m2[86:89, :], in_=gb1r_d.ap())
            st2[b]["rhs"] = rhs_m2

        def phase_vec(b):
            s = st2[b]
            mAK = small.tile([86, 3], f32, tag="mAK", name=f"mAK{b}")
            nc.vector.tensor_scalar(
                out=mAK[:], in0=sAK[:, 0, :, b], scalar1=invS, scalar2=None,
                op0=OP.mult)
            vAK = small.tile([86, 3], f32, tag="vAK", name=f"vAK{b}")
            nc.vector.tensor_scalar(
                out=vAK[:], in0=sAK[:, 1, :, b], scalar1=invS, scalar2=EPS,
                op0=OP.mult, op1=OP.add)
            msq = small.tile([86, 3], f32, tag="msq", name=f"msq{b}")
            nc.vector.tensor_mul(msq[:], mAK[:], mAK[:])
            nc.vector.tensor_sub(vAK[:], vAK[:], msq[:])
            nc.scalar.activation(out=vAK[:], in_=vAK[:], func=AF.Sqrt)
            rAK = small.tile([86, 3], f32, tag="rAK", name=f"rAK{b}")
            nc.vector.reciprocal(rAK[:], vAK[:])
            invrV = small.tile([86, 1], f32, tag="invrV", name=f"invrV{b}")
            nc.vector.reciprocal(invrV[:], rAK[:, 2:3])
            mvinv_bf = small.tile([86, 2], bf16, tag="mvinv", name=f"mvinv{b}")
            nc.vector.tensor_copy(mvinv_bf[:, 0:1], mAK[:, 2:3])
            nc.vector.tensor_copy(mvinv_bf[:, 1:2], invrV[:])
            rv_ext = small.tile([128, 1], f32, tag="rvext", name=f"rvext{b}")
            nc.vector.memset(rv_ext[64:128, :], 1.0)
            nc.vector.tensor_copy(rv_ext[0:86, :], rAK[:, 2:3])
            s["mAK"], s["rAK"] = mAK, rAK
            s["mvinv"], s["rvext"] = mvinv_bf, rv_ext

            tA = tga_back[:, b, 0:1]
            gA = tga_back[:, b, 1:2]
            hA = tga_back[:, b, 2:3]
            tK = tgk_back[:, b, 0:1]
            gK = tgk_back[:, b, 1:2]
            hK = tgk_back[:, b, 2:3]
            mA, mK = mAK[:, 0:1], mAK[:, 1:2]
            rA, rK = rAK[:, 0:1], rAK[:, 1:2]
            scG1 = sc_bc[0:86, 0:1]
            scG2 = sc_bc[0:86, 1:2]
            scGb = sc_bc[0:86, 2:3]
            scB1 = sc_bc[0:86, 3:4]

            ntK = small.tile([86, 1], f32, tag="ntK", name=f"ntK{b}")
            nc.vector.tensor_scalar_mul(ntK[:], tK, -1.0)
            nmK = small.tile([86, 1], f32, tag="nmK", name=f"nmK{b}")
            nc.vector.tensor_scalar_mul(nmK[:], mK, -1.0)
            g2mK = small.tile([86, 1], f32, tag="g2mK", name=f"g2mK{b}")
            nc.vector.tensor_scalar(
                out=g2mK[:], in0=mK, scalar1=scG2, scalar2=None, op0=OP.mult)
            t3c = small.tile([86, 1], f32, tag="t3c", name=f"t3c{b}")
            nc.vector.tensor_scalar(
                out=t3c[:], in0=mK, scalar1=scGb, scalar2=None, op0=OP.mult)
            nc.vector.tensor_sub(t3c[:], gK, t3c[:])
            nc.vector.tensor_mul(t3c[:], rK, t3c[:])
            t2c = small.tile([86, 1], f32, tag="t2c", name=f"t2c{b}")
            nc.vector.tensor_scalar(
                out=t2c[:], in0=mA, scalar1=scGb, scalar2=None, op0=OP.mult)
            nc.vector.tensor_sub(t2c[:], gA, t2c[:])
            nc.vector.tensor_mul(t2c[:], rA, t2c[:])
            syA = small.tile([86, 1], f32, tag="syA", name=f"syA{b}")
            nc.vector.tensor_scalar(
                out=syA[:], in0=mA, scalar1=scG1, scalar2=None, op0=OP.mult)
            nc.vector.tensor_sub(syA[:], hA, syA[:])
            nc.vector.tensor_mul(syA[:], rA, syA[:])
            nc.vector.tensor_scalar(
                out=syA[:], in0=syA[:], scalar1=scB1, scalar2=None, op0=OP.add)
            syK = small.tile([86, 1], f32, tag="syK", name=f"syK{b}")
            nc.vector.tensor_scalar(
                out=syK[:], in0=mK, scalar1=scG1, scalar2=None, op0=OP.mult)
            nc.vector.tensor_sub(syK[:], hK, syK[:])
            nc.vector.tensor_mul(syK[:], rK, syK[:])
            nc.vector.tensor_scalar(
                out=syK[:], in0=syK[:], scalar1=scB1, scalar2=None, op0=OP.add)
            s["ntK"], s["nmK"], s["g2mK"] = ntK, nmK, g2mK
            s["t3c"], s["syA"], s["syK"] = t3c, syA, syK

            # rows (mA, tA, rA, term2) -> transpose -> DRAM -> one bcast DMA
            pack = small.tile([86, 4], f32, tag="pack", name=f"pack{b}")
            nc.vector.tensor_copy(pack[:, 0:1], mA)
            nc.vector.tensor_copy(pack[:, 1:2], tA)
            nc.vector.tensor_copy(pack[:, 2:3], rA)
            nc.vector.tensor_copy(pack[:, 3:4], t2c[:])
            packT_ps = psA.tile([4, 86], f32, tag="psA", name=f"pT{b}")
            nc.tensor.transpose(packT_ps[:], pack[:], ident[0:86, 0:86])
            packT = small.tile([4, 86], f32, tag="packT", name=f"packT{b}")
            nc.scalar.copy(packT[:], packT_ps[:])
            rows_d = dram.tile([4, 86], f32, name=f"rowsd{b}", tag="rowsd",
                               bufs=2)
            nc.gpsimd.dma_start(out=rows_d[:], in_=packT[:])
            bc4 = small.tile([86, 4, 86], f32, tag="bc4", name=f"bc4{b}")
            nc.gpsimd.dma_start(
                out=bc4[:],
                in_=bass.AP(tensor=rows_d.tensor, offset=rows_d.offset,
                            ap=[[0, 86], [86, 4], [1, 86]]))
            s["bc4"] = bc4

        def phase_syy(b):
            s = st2[b]
            bc4 = s["bc4"]
            rK = s["rAK"][:, 1:2]
            scBb = sc_bc[0:86, 4:5]
            syy = small.tile([86, 97], f32, tag="syy", name=f"syy{b}")
            nc.vector.memset(syy[:, 86:96], 0.0)
            nc.vector.scalar_tensor_tensor(
                out=syy[:, 0:86], in0=bc4[:, 0, :], scalar=s["ntK"][:],
                in1=pt_back[:, b, :], op0=OP.mult, op1=OP.add)
            nc.vector.scalar_tensor_tensor(
                out=syy[:, 0:86], in0=bc4[:, 1, :], scalar=s["nmK"][:],
                in1=syy[:, 0:86], op0=OP.mult, op1=OP.add)
            nc.vector.scalar_tensor_tensor(
                out=syy[:, 0:86], in0=bc4[:, 0, :], scalar=s["g2mK"][:],
                in1=syy[:, 0:86], op0=OP.mult, op1=OP.add)
            nc.vector.scalar_tensor_tensor(
                out=syy[:, 0:86], in0=bc4[:, 2, :], scalar=rK,
                in1=syy[:, 0:86], op0=OP.mult, op1=OP.mult)
            nc.vector.tensor_add(syy[:, 0:86], syy[:, 0:86], bc4[:, 3, :])
            nc.vector.tensor_scalar(
                out=syy[:, 0:86], in0=syy[:, 0:86], scalar1=s["t3c"][:],
                scalar2=scBb, op0=OP.add, op1=OP.add)
            nc.vector.tensor_copy(syy[:, 96:97], s["syK"][:])
            s["syy"] = syy

        def phase_logits(b):
            s = st2[b]
            u_ps = psA.tile([97, C], f32, tag="psA", name=f"ups{b}")
            nc.tensor.matmul(u_ps[:], lhsT=s["syy"][:], rhs=ekt_sb[:],
                             start=True, stop=True)
            u_ext = small.tile([128, C], f32, tag="uext", name=f"uext{b}")
            nc.vector.memset(u_ext[64:128, :], 0.0)
            nc.vector.scalar_tensor_tensor(
                out=u_ext[0:86, :], in0=bk_bc[0:86, :], scalar=s["syA"][:],
                in1=u_ps[0:86, :], op0=OP.mult, op1=OP.add)
            nc.vector.tensor_scalar_mul(
                u_ext[96:97, :], bk_bc[96:97, :], float(S))
            nc.vector.tensor_add(u_ext[96:97, :], u_ext[96:97, :],
                                 u_ps[96:97, :])

            att_sb = []
            recip2 = small.tile([128, 2], f32, tag="recip2", name=f"re{b}")
            z2 = small.tile([128, 2], f32, tag="z2", name=f"z2{b}")
            for it in range(2):
                log_ps = psB.tile([128, 512], f32, tag="psB", name=f"lg{b}{it}")
                nc.tensor.matmul(
                    log_ps[:, 0:C], lhsT=eqt_sb[:, it * 128:(it + 1) * 128],
                    rhs=u_ext[0:97, :], start=True, stop=True)
                rmax = small.tile([128, 1], f32, tag="rmax", name=f"rm{b}{it}")
                nc.vector.reduce_max(rmax[:], log_ps[:, 0:C], axis=AX.X)
                nbias = small.tile([128, 1], f32, tag="nbias",
                                   name=f"nb{b}{it}")
                nc.vector.tensor_scalar_mul(nbias[:], rmax[:], -SCALE)
                a_sb = small.tile([128, C], f32, tag=f"attsb{it}",
                                  name=f"att{b}{it}")
                nc.scalar.activation(
                    out=a_sb[:], in_=log_ps[:, 0:C], func=AF.Exp,
                    bias=nbias[:], scale=SCALE, accum_out=z2[:, it:it + 1])
                nc.vector.reciprocal(recip2[:, it:it + 1], z2[:, it:it + 1])
                att_sb.append(a_sb)
            s["att"], s["recip2"] = att_sb, recip2

        def phase_nt(b):
            s = st2[b]
            ntc_ps = psC.tile([128, C], f32, tag="psC", name=f"ntc{b}")
            for jt in range(2):
                at_ps = psC.tile([128, C], f32, tag="psC", name=f"atp{b}{jt}")
                for it in range(2):
                    nc.tensor.transpose(
                        at_ps[:, it * 128:(it + 1) * 128],
                        s["att"][it][:, jt * 128:(jt + 1) * 128],
                        ident[:])
                at_bf = small.tile([128, C], bf16, tag=f"atbf{jt}",
                                   name=f"atb{b}{jt}")
                nc.scalar.copy(at_bf[:], at_ps[:])
                nc.tensor.matmul(
                    ntc_ps[0:87, :], lhsT=w0_sb[:, jt, :], rhs=at_bf[:],
                    start=(jt == 0), stop=(jt == 1))

            # lhsT_M2 [128, C] bf16: rows 0..85=NR, 86=cv, 87=c1, 88=c2.
            # rv_ext has 1.0 at row 86 so cv copies through unscaled.
            lhs_m2 = small.tile([128, C], bf16, tag="lhsm2", name=f"lm{b}")
            rv = s["rvext"]
            nc.scalar.activation(
                out=lhs_m2[0:64, :], in_=ntc_ps[0:64, :], func=AF.Copy,
                scale=rv[0:64, :])
            nc.scalar.activation(
                out=lhs_m2[64:87, :], in_=ntc_ps[64:87, :], func=AF.Copy,
                scale=rv[64:87, :])
            nc.tensor.matmul(
                ntc_ps[64:66, :], lhsT=s["mvinv"][:],
                rhs=lhs_m2[0:86, :], start=True, stop=True)
            c12_sb = small.tile([128, C], bf16, tag="c12sb", name=f"c12{b}")
            nc.scalar.copy(c12_sb[64:66, :], ntc_ps[64:66, :])
            nc.gpsimd.dma_start(out=lhs_m2[87:89, :], in_=c12_sb[64:66, :])
            s["lhs_m2"] = lhs_m2

        def phase_m2(b):
            s = st2[b]
            lhs_m2, rhs_m2, recip2 = s["lhs_m2"], s["rhs"], s["recip2"]
            for it in range(2):
                for ch in range(8):
                    ostg = osml.tile([128, 512], f32, tag="ostg", bufs=3,
                                     name=f"o{b}{it}{ch}")
                    o_ps = psB.tile([128, 512], f32, tag="psB",
                                    name=f"op{b}{it}{ch}")
                    nc.tensor.matmul(
                        o_ps[:],
                        lhsT=lhs_m2[0:89, it * 128:(it + 1) * 128],
                        rhs=rhs_m2[0:89, ch * 512:(ch + 1) * 512],
                        start=True, stop=True)
                    nc.vector.scalar_tensor_tensor(
                        out=ostg[:], in0=o_ps[:],
                        scalar=recip2[:, it:it + 1],
                        in1=x_sb[:, 2 * b + it, ch * 512:(ch + 1) * 512],
                        op0=OP.mult, op1=OP.add)
                    nc.sync.dma_start(
                        out=out_d[(2 * b + it) * 128:(2 * b + it + 1) * 128,
                                  ch * 512:(ch + 1) * 512],
                        in_=ostg[:])

        phases = [phase_rhs, phase_vec, phase_syy, phase_logits, phase_nt,
                  phase_m2]
        for pair in ((0, 1), (2, 3)):
            for ph in phases:
                for b in pair:
                    ph(b)

    nc.compile()
    return nc


def _host_prep(x, gamma, beta, w_qkv, b_qkv):
    xf = np.ascontiguousarray(np.asarray(x, np.float32).reshape(B * C, S))
    gam = np.asarray(gamma, np.float32).reshape(-1)
    bet = np.asarray(beta, np.float32).reshape(-1)
    w_qkv = np.asarray(w_qkv, np.float32)
    b_qkv = np.asarray(b_qkv, np.float32)
    w_q, w_k, w_v = w_qkv[:C], w_qkv[C:2 * C], w_qkv[2 * C:]
    b_q, b_k, b_v = b_qkv[:C], b_qkv[C:2 * C], b_qkv[2 * C:]

    ii = np.arange(C)
    eqt = np.zeros((97, C), np.float32)
    eqt[ii // 3, ii] = w_q
    eqt[96] = b_q
    ekt = np.zeros((86, C), np.float32)
    ekt[(C + ii) // 3 - 85, ii] = w_k
    w0 = np.zeros((C, 87), np.float32)
    w0[ii, (2 * C + ii) // 3 - 170] = w_v
    w0[:, 86] = b_v
    w0 = w0.astype(_BF)

    sc = np.zeros((1, 8), np.float32)
    sc[0, :5] = [gam.sum(), (gam * gam).sum(), (gam * bet).sum(),
                 bet.sum(), (bet * bet).sum()]

    in_maps = []
    for r in range(NCORES):
        sl = slice(r * SH, (r + 1) * SH)
        gl = gam[sl]
        bl = bet[sl]
        gb1c = np.empty((128, 3 * NST), np.float32)
        for st in range(NST):
            gb1c[:, 3 * st] = gl[st * 128:(st + 1) * 128]
            gb1c[:, 3 * st + 1] = bl[st * 128:(st + 1) * 128]
            gb1c[:, 3 * st + 2] = 1.0
        gb1r = np.stack([np.ones(SH, np.float32), -gl, bl], 0)
        in_maps.append({
            "xs": np.ascontiguousarray(xf[:, sl]),
            "gl": gl.reshape(1, SH).copy(),
            "gb1c": gb1c.astype(_BF),
            "gb1r": gb1r.astype(_BF),
            "eqt": eqt,
            "ekt": ekt,
            "w0": w0,
            "bk": b_k.reshape(1, C).copy(),
            "sc": sc,
        })
    return in_maps


def kernel(x, gamma, beta, w_qkv, b_qkv):
    from concourse.bass_utils import run_bass_kernel_spmd

    if "nc" not in _cache:
        _cache["nc"] = _build_program()
    nc = _cache["nc"]

    in_maps = _host_prep(x, gamma, beta, w_qkv, b_qkv)
    res = run_bass_kernel_spmd(nc, in_maps, core_ids=list(range(NCORES)))
    out = np.empty((B * C, S), np.float32)
    for r in range(NCORES):
        out[:, r * SH:(r + 1) * SH] = res.results[r]["out"]
    return out.reshape(np.asarray(x).shape)


if __name__ == "__main__":
    rng = np.random.default_rng(0)
    inputs = {
        "x": rng.standard_normal((B, C, 32, 32, 32)).astype(np.float32),
        "gamma": (1 + 0.1 * rng.standard_normal((32, 32, 32))).astype(np.float32),
        "beta": (0.1 * rng.standard_normal((32, 32, 32))).astype(np.float32),
        "w_qkv": (0.5 * rng.standard_normal(3 * C)).astype(np.float32),
        "b_qkv": (0.05 * rng.standard_normal(3 * C)).astype(np.float32),
    }
    o = kernel(**inputs)
    print("out", o.shape, o.dtype, float(np.abs(o).mean()))



# revision 10
# speedup vs baseline: 1.3328x; 1.1459x over previous
"""Channel-self-attention (LayerNorm + grouped-1x1-qkv + channel softmax attn
+ residual) on 8 TRN2 NeuronCores.

Strategy (v2): pair-sharding — 2 cores per batch, each core owns one
spatial half (16384 of 32768). One ~34 KB 2-rank AllReduce per core.

Per core:
 - x half-shard [256, 16384] bf16 resident in SBUF (channel-major)
 - host also sends x TRANSPOSED (spatial-major, bf16) packed per 128-row
   stile as [x_A(86) | gamma^2, gamma*beta, gamma (3) | x_K(86)] so the
   Gram matmul needs NO on-chip transposes:
     lhsT = [g2 gb g | g2*x_K]  (one DVE col-scale + one 3-col copy/stile)
     rhs  = the raw packed stile
     out  = [89,175]: rows 0..2 x cols 0..85 = tgh_A, rows 3..88 = P^T,
            rows 0..2 x cols 89..174 = tgh_K
 - stats (Sum x, Sum x^2) via bn_stats on the channel-major copy
 - ONE AllReduce (Gram + tgh + stats, 33.7 KB) within the batch pair
 - logits from the Gram expansion of the LayerNorm algebra, softmax
   normalized IN att (recip folded before the att transpose), so the
   epilogue is a plain  out = x + att_norm @ v  residual add
"""
import sys

sys.path.insert(0, "/opt/trn_rl_repo")

import numpy as np
import ml_dtypes

B, C = 4, 256
S = 32 * 32 * 32          # 32768 global spatial
NCORES = 8
SH = S // 2               # 16384 per-core spatial half
NST = SH // 128           # 128 stiles
EPS = 1e-5
SCALE = float(S) ** -0.5

_BF = ml_dtypes.bfloat16

_cache = {}


def _build_program():
    from contextlib import ExitStack
    import concourse.bass as bass
    import concourse.bacc as bacc
    import concourse.tile as tile
    from concourse import mybir, masks

    f32 = mybir.dt.float32
    bf16 = mybir.dt.bfloat16
    AF = mybir.ActivationFunctionType
    OP = mybir.AluOpType
    AX = mybir.AxisListType

    nc = bacc.Bacc(
        "TRN2",
        target_bir_lowering=False,
        debug=False,
        enable_asserts=False,
        num_devices=NCORES,
    )

    # ---------------- DRAM I/O ----------------
    xs_d = nc.dram_tensor("xs", [C, SH], bf16, kind="ExternalInput")
    xt_d = nc.dram_tensor("xt", [128, NST * 175], bf16, kind="ExternalInput")
    g2c_d = nc.dram_tensor("g2c", [128, NST], f32, kind="ExternalInput")
    gb1r_d = nc.dram_tensor("gb1r", [3, SH], bf16, kind="ExternalInput")
    eqt_d = nc.dram_tensor("eqt", [97, C], f32, kind="ExternalInput")
    ekt_d = nc.dram_tensor("ekt", [86, C], f32, kind="ExternalInput")
    w0_d = nc.dram_tensor("w0", [2 * 128, 87], bf16, kind="ExternalInput")
    bk_d = nc.dram_tensor("bk", [1, C], f32, kind="ExternalInput")
    sc_d = nc.dram_tensor("sc", [1, 8], f32, kind="ExternalInput")
    out_d = nc.dram_tensor("out", [C, SH], f32, kind="ExternalOutput")

    # AR payload layout (f32 words):
    #   [0 : 7654)        M[0:89, 0:86] row-major  (tgh_A rows 0..2, P^T 3..88)
    #   [7654 : 7912)     M[0:3, 89:175] row-major (tgh_K)
    #   [7912 : 8168)     Sum x   per channel (flat idx = channel)
    #   [8168 : 8424)     Sum x^2 per channel
    PTOT = 8424

    with tile.TileContext(nc) as tc, ExitStack() as ctx:
        const = ctx.enter_context(tc.tile_pool(name="const", bufs=1))
        xpool = ctx.enter_context(tc.tile_pool(name="xpool", bufs=1))
        xtp = ctx.enter_context(tc.tile_pool(name="xtp", bufs=2))
        utp = ctx.enter_context(tc.tile_pool(name="utp", bufs=4))
        rhsp = ctx.enter_context(tc.tile_pool(name="rhsp", bufs=1))
        osml = ctx.enter_context(tc.tile_pool(name="osml", bufs=2))
        small = ctx.enter_context(tc.tile_pool(name="small", bufs=2))
        dram = ctx.enter_context(tc.tile_pool(name="dram", bufs=1, space="DRAM"))

        # ------------- constants / inputs to SBUF -------------
        ident = const.tile([128, 128], f32)
        masks.make_identity(nc, ident[:])
        ident_bf = const.tile([128, 128], bf16)
        masks.make_identity(nc, ident_bf[:])
        eqt_sb = const.tile([97, C], f32)
        nc.sync.dma_start(out=eqt_sb[:], in_=eqt_d.ap())
        ekt_sb = const.tile([86, C], f32)
        nc.sync.dma_start(out=ekt_sb[:], in_=ekt_d.ap())
        w0_sb = const.tile([128, 2, 87], bf16)
        for jt in range(2):
            nc.sync.dma_start(out=w0_sb[:, jt, :], in_=w0_d[jt * 128:(jt + 1) * 128, :])
        g2c_sb = const.tile([128, NST], f32)
        nc.sync.dma_start(out=g2c_sb[:], in_=g2c_d.ap())

        def dram_bcast(dst, src_d, nparts, nfree, off=0):
            nc.gpsimd.dma_start(
                out=dst,
                in_=bass.AP(tensor=src_d, offset=off,
                            ap=[[0, nparts], [1, nfree]]))

        bk_bc = const.tile([128, C], f32)
        dram_bcast(bk_bc[:], bk_d, 128, C)
        sc_bc = const.tile([128, 8], f32)
        dram_bcast(sc_bc[:], sc_d, 128, 8)
        gam_bc = const.tile([128, SH], bf16)
        dram_bcast(gam_bc[:], gb1r_d, 128, SH, off=SH)
        nc.vector.tensor_scalar_mul(gam_bc[:], gam_bc[:], -1.0)

        # xt stream: 8 chunks of 16 stiles (gpsimd queue, ahead of x)
        XCH = 16 * 175
        xt_sb = []
        for cchunk in range(8):
            t = xtp.tile([128, XCH], bf16, tag="xt", name=f"xt{cchunk}")
            nc.gpsimd.dma_start(
                out=t[:], in_=xt_d[:, cchunk * XCH:(cchunk + 1) * XCH])
            xt_sb.append(t)

        # x resident bf16 [128, 2, 16384], loaded in 4096-col chunks
        x_sb = xpool.tile([128, 2, SH], bf16)
        for ct in range(2):
            for cc in range(4):
                nc.sync.dma_start(
                    out=x_sb[:, ct, cc * 4096:(cc + 1) * 4096],
                    in_=xs_d[ct * 128:(ct + 1) * 128,
                             cc * 4096:(cc + 1) * 4096])

        # ------------- stats via bn_stats -------------
        sums_sb = const.tile([128, 2], f32)
        sqs_sb = const.tile([128, 2], f32)
        bnst = const.tile([128, 2, 32, 6], f32)
        for ct in range(2):
            for sub in range(32):
                nc.vector.bn_stats(
                    out=bnst[:, ct, sub, :],
                    in_=x_sb[:, ct, sub * 512:(sub + 1) * 512])
        for ct in range(2):
            aggr = small.tile([128, 2], f32, tag="aggr")
            nc.vector.bn_aggr(out=aggr[:], in_=bnst[:, ct, :, :])
            nc.vector.tensor_scalar_mul(
                sums_sb[:, ct:ct + 1], aggr[:, 0:1], float(SH))
            tmp1 = small.tile([128, 1], f32, tag="tmp1")
            nc.vector.tensor_mul(tmp1[:], aggr[:, 0:1], aggr[:, 0:1])
            nc.vector.tensor_add(tmp1[:], tmp1[:], aggr[:, 1:2])
            nc.vector.tensor_scalar_mul(
                sqs_sb[:, ct:ct + 1], tmp1[:], float(SH))

        # ------------- rhs for M2 (independent of AR) -------------
        # rows 0..85 = gamma*x_V (ch 170..255), 86..88 = [ones, -gamma, beta]
        rhs_m2 = rhsp.tile([128, SH], bf16)
        nc.gpsimd.dma_start(out=rhs_m2[0:86, :], in_=x_sb[42:128, 1, :])
        nc.vector.tensor_tensor(
            out=rhs_m2[0:86, :], in0=rhs_m2[0:86, :], in1=gam_bc[0:86, :],
            op=OP.mult)
        nc.gpsimd.dma_start(out=rhs_m2[86:89, :], in_=gb1r_d.ap())

        # ------------- Gram over 128 stiles -------------
        bncP_in = dram.tile([PTOT], f32)
        bncP_out = dram.tile([PTOT], f32)

        with tc.tile_pool(name="s1ps", bufs=1, space="PSUM") as stg1ps:
            ptk_ps = stg1ps.tile([89, 175], f32)
            for st in range(NST):
                blk = xt_sb[st // 16][:, (st % 16) * 175:(st % 16) * 175 + 175]
                ut2 = utp.tile([128, 89], bf16, tag="ut2", name=f"ut{st}")
                nc.scalar.copy(ut2[:, 0:3], blk[:, 86:89])
                nc.vector.tensor_scalar(
                    out=ut2[:, 3:89], in0=blk[:, 89:175],
                    scalar1=g2c_sb[:, st:st + 1], scalar2=None, op0=OP.mult)
                nc.tensor.matmul(
                    ptk_ps[:], lhsT=ut2[:], rhs=blk[:],
                    start=(st == 0), stop=(st == NST - 1))

            ptk_sb = small.tile([89, 86], f32, tag="ptksb", bufs=1)
            nc.scalar.copy(ptk_sb[:], ptk_ps[0:89, 0:86])
            ptk3_sb = small.tile([3, 86], f32, tag="ptk3sb", bufs=1)
            nc.scalar.copy(ptk3_sb[:], ptk_ps[0:3, 89:175])

        nc.gpsimd.dma_start(
            out=bncP_in[0:7654].rearrange("(p f) -> p f", f=86),
            in_=ptk_sb[:])
        nc.gpsimd.dma_start(
            out=bncP_in[7654:7912].rearrange("(p f) -> p f", f=86),
            in_=ptk3_sb[:])
        nc.gpsimd.dma_start(
            out=bncP_in[7912:8168].rearrange("(t p) -> p t", p=128),
            in_=sums_sb[:])
        nc.gpsimd.dma_start(
            out=bncP_in[8168:8424].rearrange("(t p) -> p t", p=128),
            in_=sqs_sb[:])
        nc.gpsimd.collective_compute(
            "AllReduce", OP.add,
            replica_groups=[[0, 1], [2, 3], [4, 5], [6, 7]],
            ins=[bncP_in[:].opt()], outs=[bncP_out[:].opt()])

        # ------------- DMA back -------------
        pt_back = const.tile([86, 86], f32)
        nc.sync.dma_start(
            out=pt_back[:],
            in_=bass.AP(tensor=bncP_out.tensor,
                        offset=bncP_out.offset + 3 * 86,
                        ap=[[86, 86], [1, 86]]))
        tga = const.tile([86, 3], f32)
        nc.sync.dma_start(
            out=tga[:],
            in_=bass.AP(tensor=bncP_out.tensor, offset=bncP_out.offset,
                        ap=[[1, 86], [86, 3]]))
        tgk = const.tile([86, 3], f32)
        nc.sync.dma_start(
            out=tgk[:],
            in_=bass.AP(tensor=bncP_out.tensor,
                        offset=bncP_out.offset + 7654,
                        ap=[[1, 86], [86, 3]]))
        # stats cols: [p, {Sx,Sxx} x {A,K,V}]
        sAK = const.tile([86, 6], f32)
        for k, koff in ((0, 7912), (1, 8168)):
            for g, goff in ((0, 0), (1, 85), (2, 170)):
                nc.sync.dma_start(
                    out=sAK[:, 3 * k + g:3 * k + g + 1],
                    in_=bass.AP(tensor=bncP_out.tensor,
                                offset=bncP_out.offset + koff + goff,
                                ap=[[1, 86], [1, 1]]))

        # ------------- stage 2 (one batch per core) -------------
        psA = ctx.enter_context(tc.tile_pool(name="psA", bufs=2, space="PSUM"))
        psB = ctx.enter_context(tc.tile_pool(name="psB", bufs=3, space="PSUM"))
        psC = ctx.enter_context(tc.tile_pool(name="psC", bufs=1, space="PSUM"))
        psD = ctx.enter_context(tc.tile_pool(name="psD", bufs=2, space="PSUM"))

        invS = 1.0 / float(S)

        # --- per-channel LayerNorm scalars ---
        mAK = small.tile([86, 3], f32, tag="mAK")
        nc.vector.tensor_scalar(
            out=mAK[:], in0=sAK[:, 0:3], scalar1=invS, scalar2=None,
            op0=OP.mult)
        vAK = small.tile([86, 3], f32, tag="vAK")
        nc.vector.tensor_scalar(
            out=vAK[:], in0=sAK[:, 3:6], scalar1=invS, scalar2=EPS,
            op0=OP.mult, op1=OP.add)
        msq = small.tile([86, 3], f32, tag="msq")
        nc.vector.tensor_mul(msq[:], mAK[:], mAK[:])
        nc.vector.tensor_sub(vAK[:], vAK[:], msq[:])
        nc.scalar.activation(out=vAK[:], in_=vAK[:], func=AF.Sqrt)
        rAK = small.tile([86, 3], f32, tag="rAK")
        nc.vector.reciprocal(rAK[:], vAK[:])
        invrV = small.tile([86, 1], f32, tag="invrV")
        nc.vector.reciprocal(invrV[:], rAK[:, 2:3])
        mvinv_bf = small.tile([86, 2], bf16, tag="mvinv")
        nc.vector.tensor_copy(mvinv_bf[:, 0:1], mAK[:, 2:3])
        nc.vector.tensor_copy(mvinv_bf[:, 1:2], invrV[:])
        rv_ext = small.tile([128, 1], f32, tag="rvext")
        nc.vector.memset(rv_ext[64:128, :], 1.0)
        nc.vector.tensor_copy(rv_ext[0:86, :], rAK[:, 2:3])

        tA, gA, hA = tga[:, 0:1], tga[:, 1:2], tga[:, 2:3]
        tK, gK, hK = tgk[:, 0:1], tgk[:, 1:2], tgk[:, 2:3]
        mA, mK = mAK[:, 0:1], mAK[:, 1:2]
        rA, rK = rAK[:, 0:1], rAK[:, 1:2]
        scG1 = sc_bc[0:86, 0:1]
        scG2 = sc_bc[0:86, 1:2]
        scGb = sc_bc[0:86, 2:3]
        scB1 = sc_bc[0:86, 3:4]
        scBb = sc_bc[0:86, 4:5]

        ntK = small.tile([86, 1], f32, tag="ntK")
        nc.vector.tensor_scalar_mul(ntK[:], tK, -1.0)
        nmK = small.tile([86, 1], f32, tag="nmK")
        nc.vector.tensor_scalar_mul(nmK[:], mK, -1.0)
        g2mK = small.tile([86, 1], f32, tag="g2mK")
        nc.vector.tensor_scalar(
            out=g2mK[:], in0=mK, scalar1=scG2, scalar2=None, op0=OP.mult)
        t3c = small.tile([86, 1], f32, tag="t3c")
        nc.vector.tensor_scalar(
            out=t3c[:], in0=mK, scalar1=scGb, scalar2=None, op0=OP.mult)
        nc.vector.tensor_sub(t3c[:], gK, t3c[:])
        nc.vector.tensor_mul(t3c[:], rK, t3c[:])
        t2c = small.tile([86, 1], f32, tag="t2c")
        nc.vector.tensor_scalar(
            out=t2c[:], in0=mA, scalar1=scGb, scalar2=None, op0=OP.mult)
        nc.vector.tensor_sub(t2c[:], gA, t2c[:])
        nc.vector.tensor_mul(t2c[:], rA, t2c[:])
        syA = small.tile([86, 1], f32, tag="syA")
        nc.vector.tensor_scalar(
            out=syA[:], in0=mA, scalar1=scG1, scalar2=None, op0=OP.mult)
        nc.vector.tensor_sub(syA[:], hA, syA[:])
        nc.vector.tensor_mul(syA[:], rA, syA[:])
        nc.vector.tensor_scalar(
            out=syA[:], in0=syA[:], scalar1=scB1, scalar2=None, op0=OP.add)
        syK = small.tile([86, 1], f32, tag="syK")
        nc.vector.tensor_scalar(
            out=syK[:], in0=mK, scalar1=scG1, scalar2=None, op0=OP.mult)
        nc.vector.tensor_sub(syK[:], hK, syK[:])
        nc.vector.tensor_mul(syK[:], rK, syK[:])
        nc.vector.tensor_scalar(
            out=syK[:], in0=syK[:], scalar1=scB1, scalar2=None, op0=OP.add)

        # rows (mA, tA, rA, term2) -> transpose -> DRAM -> one bcast DMA
        pack = small.tile([86, 4], f32, tag="pack")
        nc.vector.tensor_copy(pack[:, 0:1], mA)
        nc.vector.tensor_copy(pack[:, 1:2], tA)
        nc.vector.tensor_copy(pack[:, 2:3], rA)
        nc.vector.tensor_copy(pack[:, 3:4], t2c[:])
        packT_ps = psA.tile([4, 86], f32, tag="psA")
        nc.tensor.transpose(packT_ps[:], pack[:], ident[0:86, 0:86])
        packT = small.tile([4, 86], f32, tag="packT")
        nc.scalar.copy(packT[:], packT_ps[:])
        rows_d = dram.tile([4, 86], f32, tag="rowsd")
        nc.gpsimd.dma_start(out=rows_d[:], in_=packT[:])
        bc4 = small.tile([86, 4, 86], f32, tag="bc4")
        nc.gpsimd.dma_start(
            out=bc4[:],
            in_=bass.AP(tensor=rows_d.tensor, offset=rows_d.offset,
                        ap=[[0, 86], [86, 4], [1, 86]]))

        # --- syy ---
        syy = small.tile([86, 97], f32, tag="syy")
        nc.vector.memset(syy[:, 86:96], 0.0)
        nc.vector.scalar_tensor_tensor(
            out=syy[:, 0:86], in0=bc4[:, 0, :], scalar=ntK[:],
            in1=pt_back[:], op0=OP.mult, op1=OP.add)
        nc.vector.scalar_tensor_tensor(
            out=syy[:, 0:86], in0=bc4[:, 1, :], scalar=nmK[:],
            in1=syy[:, 0:86], op0=OP.mult, op1=OP.add)
        nc.vector.scalar_tensor_tensor(
            out=syy[:, 0:86], in0=bc4[:, 0, :], scalar=g2mK[:],
            in1=syy[:, 0:86], op0=OP.mult, op1=OP.add)
        nc.vector.scalar_tensor_tensor(
            out=syy[:, 0:86], in0=bc4[:, 2, :], scalar=rK,
            in1=syy[:, 0:86], op0=OP.mult, op1=OP.mult)
        nc.vector.tensor_add(syy[:, 0:86], syy[:, 0:86], bc4[:, 3, :])
        nc.vector.tensor_scalar(
            out=syy[:, 0:86], in0=syy[:, 0:86], scalar1=t3c[:],
            scalar2=scBb, op0=OP.add, op1=OP.add)
        nc.vector.tensor_copy(syy[:, 96:97], syK[:])

        # --- logits + softmax (recip folded into att) ---
        u_ps = psA.tile([97, C], f32, tag="psA")
        nc.tensor.matmul(u_ps[:], lhsT=syy[:], rhs=ekt_sb[:],
                         start=True, stop=True)
        u_ext = small.tile([128, C], f32, tag="uext")
        nc.vector.memset(u_ext[64:128, :], 0.0)
        nc.vector.scalar_tensor_tensor(
            out=u_ext[0:86, :], in0=bk_bc[0:86, :], scalar=syA[:],
            in1=u_ps[0:86, :], op0=OP.mult, op1=OP.add)
        nc.vector.tensor_scalar_mul(
            u_ext[96:97, :], bk_bc[96:97, :], float(S))
        nc.vector.tensor_add(u_ext[96:97, :], u_ext[96:97, :],
                             u_ps[96:97, :])

        att_nrm = []
        recip2 = small.tile([128, 2], f32, tag="recip2")
        z2 = small.tile([128, 2], f32, tag="z2")
        for it in range(2):
            log_ps = psB.tile([128, 512], f32, tag="psB", name=f"lg{it}")
            nc.tensor.matmul(
                log_ps[:, 0:C], lhsT=eqt_sb[:, it * 128:(it + 1) * 128],
                rhs=u_ext[0:97, :], start=True, stop=True)
            rmax = small.tile([128, 1], f32, tag="rmax", name=f"rm{it}")
            nc.vector.reduce_max(rmax[:], log_ps[:, 0:C], axis=AX.X)
            nbias = small.tile([128, 1], f32, tag="nbias", name=f"nb{it}")
            nc.vector.tensor_scalar_mul(nbias[:], rmax[:], -SCALE)
            a_sb = small.tile([128, C], f32, tag=f"attsb{it}", name=f"att{it}")
            nc.scalar.activation(
                out=a_sb[:], in_=log_ps[:, 0:C], func=AF.Exp,
                bias=nbias[:], scale=SCALE, accum_out=z2[:, it:it + 1])
            nc.vector.reciprocal(recip2[:, it:it + 1], z2[:, it:it + 1])
            a_nr = small.tile([128, C], bf16, tag=f"anrm{it}", name=f"an{it}")
            nc.scalar.activation(
                out=a_nr[:], in_=a_sb[:], func=AF.Copy,
                scale=recip2[:, it:it + 1])
            att_nrm.append(a_nr)

        # --- NT: lhs_m2 [89 rows, 256 q-ch] ---
        ntc_ps = psC.tile([128, C], f32, tag="psC", name="ntc")
        for jt in range(2):
            at_ps = psD.tile([128, C], bf16, tag="psD", name=f"atp{jt}")
            for it in range(2):
                nc.tensor.transpose(
                    at_ps[:, it * 128:(it + 1) * 128],
                    att_nrm[it][:, jt * 128:(jt + 1) * 128],
                    ident_bf[:])
            at_bf = small.tile([128, C], bf16, tag=f"atbf{jt}", name=f"atb{jt}")
            nc.scalar.copy(at_bf[:], at_ps[:])
            nc.tensor.matmul(
                ntc_ps[0:87, :], lhsT=w0_sb[:, jt, :], rhs=at_bf[:],
                start=(jt == 0), stop=(jt == 1))

        lhs_m2 = small.tile([128, C], bf16, tag="lhsm2")
        rv = rv_ext
        nc.scalar.activation(
            out=lhs_m2[0:64, :], in_=ntc_ps[0:64, :], func=AF.Copy,
            scale=rv[0:64, :])
        nc.scalar.activation(
            out=lhs_m2[64:87, :], in_=ntc_ps[64:87, :], func=AF.Copy,
            scale=rv[64:87, :])
        nc.tensor.matmul(
            ntc_ps[64:66, :], lhsT=mvinv_bf[:],
            rhs=lhs_m2[0:86, :], start=True, stop=True)
        c12_sb = small.tile([128, C], bf16, tag="c12sb")
        nc.scalar.copy(c12_sb[64:66, :], ntc_ps[64:66, :])
        nc.gpsimd.dma_start(out=lhs_m2[87:89, :], in_=c12_sb[64:66, :])

        # --- M2: out = x + att_nrm @ v ---
        for it in range(2):
            for ch in range(8):
                ostg = osml.tile([128, 2048], f32, tag="ostg",
                                 name=f"o{it}{ch}")
                for j in range(4):
                    off = ch * 2048 + j * 512
                    o_ps = psB.tile([128, 512], f32, tag="psB",
                                    name=f"op{it}{ch}{j}")
                    nc.tensor.matmul(
                        o_ps[:],
                        lhsT=lhs_m2[0:89, it * 128:(it + 1) * 128],
                        rhs=rhs_m2[0:89, off:off + 512],
                        start=True, stop=True)
                    nc.vector.tensor_tensor(
                        out=ostg[:, j * 512:(j + 1) * 512], in0=o_ps[:],
                        in1=x_sb[:, it, off:off + 512], op=OP.add)
                nc.sync.dma_start(
                    out=out_d[it * 128:(it + 1) * 128,
                              ch * 2048:(ch + 1) * 2048],
                    in_=ostg[:])

    nc.compile()
    return nc


def _host_prep(x, gamma, beta, w_qkv, b_qkv):
    xf = np.asarray(x, np.float32).reshape(B, C, S)
    gam = np.asarray(gamma, np.float32).reshape(-1)
    bet = np.asarray(beta, np.float32).reshape(-1)
    w_qkv = np.asarray(w_qkv, np.float32)
    b_qkv = np.asarray(b_qkv, np.float32)
    w_q, w_k, w_v = w_qkv[:C], w_qkv[C:2 * C], w_qkv[2 * C:]
    b_q, b_k, b_v = b_qkv[:C], b_qkv[C:2 * C], b_qkv[2 * C:]

    ii = np.arange(C)
    eqt = np.zeros((97, C), np.float32)
    eqt[ii // 3, ii] = w_q
    eqt[96] = b_q
    ekt = np.zeros((86, C), np.float32)
    ekt[(C + ii) // 3 - 85, ii] = w_k
    w0 = np.zeros((C, 87), np.float32)
    w0[ii, (2 * C + ii) // 3 - 170] = w_v
    w0[:, 86] = b_v
    w0 = w0.astype(_BF)

    sc = np.zeros((1, 8), np.float32)
    sc[0, :5] = [gam.sum(), (gam * gam).sum(), (gam * bet).sum(),
                 bet.sum(), (bet * bet).sum()]

    in_maps = []
    for r in range(NCORES):
        b, half = r // 2, r % 2
        sl = slice(half * SH, (half + 1) * SH)
        gl = gam[sl]
        bl = bet[sl]
        gb1r = np.stack([np.ones(SH, np.float32), -gl, bl], 0)

        xl = xf[b][:, sl]                       # [256, 16384]
        xtl = np.ascontiguousarray(xl.T)        # [16384, 256]
        blocks = np.empty((SH, 175), np.float32)
        blocks[:, 0:86] = xtl[:, 0:86]
        blocks[:, 86] = gl * gl
        blocks[:, 87] = gl * bl
        blocks[:, 88] = gl
        blocks[:, 89:175] = xtl[:, 85:171]
        xt = blocks.reshape(NST, 128, 175).transpose(1, 0, 2)
        xt = np.ascontiguousarray(xt.reshape(128, NST * 175)).astype(_BF)

        g2c = np.ascontiguousarray(
            (gl * gl).reshape(NST, 128).T).astype(np.float32)

        in_maps.append({
            "xs": np.ascontiguousarray(xl).astype(_BF),
            "xt": xt,
            "g2c": g2c,
            "gb1r": gb1r.astype(_BF),
            "eqt": eqt,
            "ekt": ekt,
            "w0": w0,
            "bk": b_k.reshape(1, C).copy(),
            "sc": sc,
        })
    return in_maps


def kernel(x, gamma, beta, w_qkv, b_qkv):
    from concourse.bass_utils import run_bass_kernel_spmd

    if "nc" not in _cache:
        _cache["nc"] = _build_program()
    nc = _cache["nc"]

    in_maps = _host_prep(x, gamma, beta, w_qkv, b_qkv)
    res = run_bass_kernel_spmd(nc, in_maps, core_ids=list(range(NCORES)))
    out = np.empty((B, C, S), np.float32)
    for r in range(NCORES):
        b, half = r // 2, r % 2
        out[b][:, half * SH:(half + 1) * SH] = res.results[r]["out"]
    return out.reshape(np.asarray(x).shape)


if __name__ == "__main__":
    rng = np.random.default_rng(0)
    inputs = {
        "x": rng.standard_normal((B, C, 32, 32, 32)).astype(np.float32),
        "gamma": (1 + 0.1 * rng.standard_normal((32, 32, 32))).astype(np.float32),
        "beta": (0.1 * rng.standard_normal((32, 32, 32))).astype(np.float32),
        "w_qkv": (0.5 * rng.standard_normal(3 * C)).astype(np.float32),
        "b_qkv": (0.05 * rng.standard_normal(3 * C)).astype(np.float32),
    }
    o = kernel(**inputs)
    print("out", o.shape, o.dtype, float(np.abs(o).mean()))


# revision 13
# speedup vs baseline: 1.4426x; 1.0824x over previous
"""Channel-self-attention (LayerNorm + grouped-1x1-qkv + channel softmax attn
+ residual) on 8 TRN2 NeuronCores.

Strategy (v3): pair-sharding — 2 cores per batch, each core owns one
spatial half (16384 of 32768). One ~34 KB 2-rank Mesh AllReduce per core.

Per core:
 - x half-shard [256, 16384] bf16 resident in SBUF (channel-major)
 - host also sends x TRANSPOSED (spatial-major, bf16) packed per 128-row
   stile as [x_A(86) | g2 gb g (3) | x_K(86)] so the Gram matmul needs NO
   on-chip transposes:
     lhsT = [g2 gb g | g2*x_K]   (g2*x_K built by 8 bulk chunk DVE mults
                                  against a host-replicated gamma^2 tile)
     rhs  = the raw packed stile
     out  = [89,175]: rows 0..2 x cols 0..85 = tgh_A, rows 3..88 = P^T,
            rows 0..2 x cols 89..174 = tgh_K
 - stats: Sum x via DVE reduce, Sum x^2 via Scalar Square+accum (idle
   engine), replacing bn_stats
 - ONE AllReduce (Gram + tgh + stats, 33.7 KB) within the batch pair
 - logits from the Gram expansion of the LayerNorm algebra; softmax
   normalization folded into att before the transpose, so the epilogue is
   a plain  out = x + att_nrm @ v  residual add (split DVE/GpSimd), with
   bf16 output upcast on host
"""
import sys

sys.path.insert(0, "/opt/trn_rl_repo")

import numpy as np
import ml_dtypes

B, C = 4, 256
S = 32 * 32 * 32          # 32768 global spatial
NCORES = 8
SH = S // 2               # 16384 per-core spatial half
NST = SH // 128           # 128 stiles
NCH = 8                   # Gram stream chunks
CST = NST // NCH          # 16 stiles per chunk
EPS = 1e-5
SCALE = float(S) ** -0.5

_BF = ml_dtypes.bfloat16

_cache = {}


def _build_program():
    from contextlib import ExitStack
    import concourse.bass as bass
    import concourse.bacc as bacc
    import concourse.tile as tile
    from concourse import mybir, masks

    f32 = mybir.dt.float32
    bf16 = mybir.dt.bfloat16
    AF = mybir.ActivationFunctionType
    OP = mybir.AluOpType
    AX = mybir.AxisListType

    nc = bacc.Bacc(
        "TRN2",
        target_bir_lowering=False,
        debug=False,
        enable_asserts=False,
        num_devices=NCORES,
    )

    # ---------------- DRAM I/O ----------------
    xs_d = nc.dram_tensor("xs", [C, SH], bf16, kind="ExternalInput")
    xt_d = nc.dram_tensor("xt", [128, NST * 175], bf16, kind="ExternalInput")
    u2i_d = nc.dram_tensor("u2i", [128, NST * 3], bf16, kind="ExternalInput")
    g2r_d = nc.dram_tensor("g2r", [128, NST * 86], bf16, kind="ExternalInput")
    gb1r_d = nc.dram_tensor("gb1r", [3, SH], bf16, kind="ExternalInput")
    eqt_d = nc.dram_tensor("eqt", [97, C], f32, kind="ExternalInput")
    ekt_d = nc.dram_tensor("ekt", [86, C], f32, kind="ExternalInput")
    w0_d = nc.dram_tensor("w0", [2 * 128, 87], bf16, kind="ExternalInput")
    bk_d = nc.dram_tensor("bk", [1, C], f32, kind="ExternalInput")
    sc_d = nc.dram_tensor("sc", [1, 8], f32, kind="ExternalInput")
    out_d = nc.dram_tensor("out", [C, SH], bf16, kind="ExternalOutput")

    # AR payload layout (f32 words):
    #   [0 : 7654)        M[0:89, 0:86] row-major  (tgh_A rows 0..2, P^T 3..88)
    #   [7654 : 7912)     M[0:3, 89:175] row-major (tgh_K)
    #   [7912 : 8168)     Sum x   per channel (flat idx = channel)
    #   [8168 : 8424)     Sum x^2 per channel
    PTOT = 8424

    with tile.TileContext(nc) as tc, ExitStack() as ctx:
        const = ctx.enter_context(tc.tile_pool(name="const", bufs=1))
        xpool = ctx.enter_context(tc.tile_pool(name="xpool", bufs=1))
        xtp = ctx.enter_context(tc.tile_pool(name="xtp", bufs=2))
        utp = ctx.enter_context(tc.tile_pool(name="utp", bufs=2))
        g2p = ctx.enter_context(tc.tile_pool(name="g2p", bufs=2))
        rhsp = ctx.enter_context(tc.tile_pool(name="rhsp", bufs=1))
        osml = ctx.enter_context(tc.tile_pool(name="osml", bufs=2))
        small = ctx.enter_context(tc.tile_pool(name="small", bufs=2))
        dram = ctx.enter_context(tc.tile_pool(name="dram", bufs=1, space="DRAM"))

        # ------------- constants / inputs to SBUF -------------
        ident = const.tile([128, 128], f32)
        masks.make_identity(nc, ident[:])
        ident_bf = const.tile([128, 128], bf16)
        masks.make_identity(nc, ident_bf[:])
        eqt_sb = const.tile([97, C], f32)
        nc.sync.dma_start(out=eqt_sb[:], in_=eqt_d.ap())
        ekt_sb = const.tile([86, C], f32)
        nc.sync.dma_start(out=ekt_sb[:], in_=ekt_d.ap())
        w0_sb = const.tile([128, 2, 87], bf16)
        for jt in range(2):
            nc.sync.dma_start(out=w0_sb[:, jt, :], in_=w0_d[jt * 128:(jt + 1) * 128, :])

        def dram_bcast(dst, src_d, nparts, nfree, off=0):
            nc.gpsimd.dma_start(
                out=dst,
                in_=bass.AP(tensor=src_d, offset=off,
                            ap=[[0, nparts], [1, nfree]]))

        bk_bc = const.tile([128, C], f32)
        dram_bcast(bk_bc[:], bk_d, 128, C)
        sc_bc = const.tile([128, 8], f32)
        dram_bcast(sc_bc[:], sc_d, 128, 8)
        gam_bc = const.tile([128, SH], bf16)
        dram_bcast(gam_bc[:], gb1r_d, 128, SH, off=SH)
        nc.vector.tensor_scalar_mul(gam_bc[:], gam_bc[:], -1.0)

        # Gram streams: xt on gpsimd queue, u2i/g2r on tensor queue
        xt_sb, u2t_sb, g2r_sb = [], [], []
        for c in range(NCH):
            t = xtp.tile([128, CST, 175], bf16, tag="xt", name=f"xt{c}")
            nc.gpsimd.dma_start(
                out=t[:],
                in_=xt_d[:, c * CST * 175:(c + 1) * CST * 175])
            xt_sb.append(t)
            u = utp.tile([128, CST, 89], bf16, tag="u2t", name=f"u2{c}")
            nc.scalar.dma_start(
                out=u[:, :, 0:3],
                in_=u2i_d[:, c * CST * 3:(c + 1) * CST * 3])
            u2t_sb.append(u)
            g = g2p.tile([128, CST, 86], bf16, tag="g2r", name=f"g2{c}")
            nc.scalar.dma_start(
                out=g[:],
                in_=g2r_d[:, c * CST * 86:(c + 1) * CST * 86])
            g2r_sb.append(g)

        # x resident bf16 [128, 2, 16384]
        x_sb = xpool.tile([128, 2, SH], bf16)
        for ct in range(2):
            nc.sync.dma_start(
                out=x_sb[:, ct, :],
                in_=xs_d[ct * 128:(ct + 1) * 128, :])

        # ------------- Gram over 128 stiles (8 chunks) -------------
        bncP_in = dram.tile([PTOT], f32)
        bncP_out = dram.tile([PTOT], f32)

        with tc.tile_pool(name="s1ps", bufs=1, space="PSUM") as stg1ps:
            ptk_ps = stg1ps.tile([89, 175], f32)
            for c in range(NCH):
                nc.vector.tensor_tensor(
                    out=u2t_sb[c][:, :, 3:89], in0=xt_sb[c][:, :, 89:175],
                    in1=g2r_sb[c][:], op=OP.mult)
                for j in range(CST):
                    st = c * CST + j
                    nc.tensor.matmul(
                        ptk_ps[:], lhsT=u2t_sb[c][:, j, :],
                        rhs=xt_sb[c][:, j, :],
                        start=(st == 0), stop=(st == NST - 1))

            ptk_sb = small.tile([89, 86], f32, tag="ptksb", bufs=1)
            nc.scalar.copy(ptk_sb[:], ptk_ps[0:89, 0:86])
            ptk3_sb = small.tile([3, 86], f32, tag="ptk3sb", bufs=1)
            nc.scalar.copy(ptk3_sb[:], ptk_ps[0:3, 89:175])

        # ------------- stats: Sum x (DVE), Sum x^2 (Scalar) -------------
        sums_sb = const.tile([128, 2], f32)
        sqp = const.tile([128, 8], f32)
        sqs_sb = const.tile([128, 2], f32)
        for ct in range(2):
            nc.vector.reduce_sum(
                sums_sb[:, ct:ct + 1], x_sb[:, ct, :], axis=AX.X)
            for cc in range(4):
                scr = osml.tile([128, 4096], bf16, tag="ostg",
                                name=f"sq{ct}{cc}")
                nc.scalar.activation(
                    out=scr[:], in_=x_sb[:, ct, cc * 4096:(cc + 1) * 4096],
                    func=AF.Square, accum_out=sqp[:, 4 * ct + cc:4 * ct + cc + 1])
            nc.vector.reduce_sum(
                sqs_sb[:, ct:ct + 1], sqp[:, 4 * ct:4 * ct + 4], axis=AX.X)

        # ------------- rhs for M2 (independent of AR) -------------
        # rows 0..85 = gamma*x_V (ch 170..255), 86..88 = [ones, -gamma, beta]
        rhs_m2 = rhsp.tile([128, SH], bf16)
        nc.gpsimd.dma_start(out=rhs_m2[0:86, :], in_=x_sb[42:128, 1, :])
        nc.vector.tensor_tensor(
            out=rhs_m2[0:86, :], in0=rhs_m2[0:86, :], in1=gam_bc[0:86, :],
            op=OP.mult)
        nc.gpsimd.dma_start(out=rhs_m2[86:89, :], in_=gb1r_d.ap())

        # ------------- AllReduce within the batch pair -------------
        nc.gpsimd.dma_start(
            out=bncP_in[0:7654].rearrange("(p f) -> p f", f=86),
            in_=ptk_sb[:])
        nc.gpsimd.dma_start(
            out=bncP_in[7654:7912].rearrange("(p f) -> p f", f=86),
            in_=ptk3_sb[:])
        nc.gpsimd.dma_start(
            out=bncP_in[7912:8168].rearrange("(t p) -> p t", p=128),
            in_=sums_sb[:])
        nc.gpsimd.dma_start(
            out=bncP_in[8168:8424].rearrange("(t p) -> p t", p=128),
            in_=sqs_sb[:])
        nc.gpsimd.collective_compute(
            "AllReduce", OP.add,
            replica_groups=[[0, 1], [2, 3], [4, 5], [6, 7]],
            ins=[bncP_in[:].opt()], outs=[bncP_out[:].opt()])

        # ------------- DMA back -------------
        pt_back = const.tile([86, 86], f32)
        nc.sync.dma_start(
            out=pt_back[:],
            in_=bass.AP(tensor=bncP_out.tensor,
                        offset=bncP_out.offset + 3 * 86,
                        ap=[[86, 86], [1, 86]]))
        tga = const.tile([86, 3], f32)
        nc.sync.dma_start(
            out=tga[:],
            in_=bass.AP(tensor=bncP_out.tensor, offset=bncP_out.offset,
                        ap=[[1, 86], [86, 3]]))
        tgk = const.tile([86, 3], f32)
        nc.sync.dma_start(
            out=tgk[:],
            in_=bass.AP(tensor=bncP_out.tensor,
                        offset=bncP_out.offset + 7654,
                        ap=[[1, 86], [86, 3]]))
        # stats cols: [p, {Sx,Sxx} x {A,K,V}]
        sAK = const.tile([86, 6], f32)
        for k, koff in ((0, 7912), (1, 8168)):
            for g, goff in ((0, 0), (1, 85), (2, 170)):
                nc.sync.dma_start(
                    out=sAK[:, 3 * k + g:3 * k + g + 1],
                    in_=bass.AP(tensor=bncP_out.tensor,
                                offset=bncP_out.offset + koff + goff,
                                ap=[[1, 86], [1, 1]]))

        invS = 1.0 / float(S)

        # --- per-channel LayerNorm scalars ---
        mAK = small.tile([86, 3], f32, tag="mAK")
        nc.vector.tensor_scalar(
            out=mAK[:], in0=sAK[:, 0:3], scalar1=invS, scalar2=None,
            op0=OP.mult)
        vAK = small.tile([86, 3], f32, tag="vAK")
        nc.vector.tensor_scalar(
            out=vAK[:], in0=sAK[:, 3:6], scalar1=invS, scalar2=EPS,
            op0=OP.mult, op1=OP.add)
        msq = small.tile([86, 3], f32, tag="msq")
        nc.vector.tensor_mul(msq[:], mAK[:], mAK[:])
        nc.vector.tensor_sub(vAK[:], vAK[:], msq[:])
        nc.scalar.activation(out=vAK[:], in_=vAK[:], func=AF.Sqrt)
        rAK = small.tile([86, 3], f32, tag="rAK")
        nc.vector.reciprocal(rAK[:], vAK[:])
        invrV = small.tile([86, 1], f32, tag="invrV")
        nc.vector.reciprocal(invrV[:], rAK[:, 2:3])
        mvinv_bf = small.tile([86, 2], bf16, tag="mvinv")
        nc.vector.tensor_copy(mvinv_bf[:, 0:1], mAK[:, 2:3])
        nc.vector.tensor_copy(mvinv_bf[:, 1:2], invrV[:])
        rv_ext = small.tile([128, 1], f32, tag="rvext")
        nc.vector.memset(rv_ext[64:128, :], 1.0)
        nc.vector.tensor_copy(rv_ext[0:86, :], rAK[:, 2:3])

        tA, gA, hA = tga[:, 0:1], tga[:, 1:2], tga[:, 2:3]
        tK, gK, hK = tgk[:, 0:1], tgk[:, 1:2], tgk[:, 2:3]
        mA, mK = mAK[:, 0:1], mAK[:, 1:2]
        rA, rK = rAK[:, 0:1], rAK[:, 1:2]
        scG1 = sc_bc[0:86, 0:1]
        scG2 = sc_bc[0:86, 1:2]
        scGb = sc_bc[0:86, 2:3]
        scB1 = sc_bc[0:86, 3:4]
        scBb = sc_bc[0:86, 4:5]

        ntK = small.tile([86, 1], f32, tag="ntK")
        nc.vector.tensor_scalar_mul(ntK[:], tK, -1.0)
        nmK = small.tile([86, 1], f32, tag="nmK")
        nc.vector.tensor_scalar_mul(nmK[:], mK, -1.0)
        g2mK = small.tile([86, 1], f32, tag="g2mK")
        nc.vector.tensor_scalar(
            out=g2mK[:], in0=mK, scalar1=scG2, scalar2=None, op0=OP.mult)
        t3c = small.tile([86, 1], f32, tag="t3c")
        nc.vector.tensor_scalar(
            out=t3c[:], in0=mK, scalar1=scGb, scalar2=None, op0=OP.mult)
        nc.vector.tensor_sub(t3c[:], gK, t3c[:])
        nc.vector.tensor_mul(t3c[:], rK, t3c[:])
        t2c = small.tile([86, 1], f32, tag="t2c")
        nc.vector.tensor_scalar(
            out=t2c[:], in0=mA, scalar1=scGb, scalar2=None, op0=OP.mult)
        nc.vector.tensor_sub(t2c[:], gA, t2c[:])
        nc.vector.tensor_mul(t2c[:], rA, t2c[:])
        syA = small.tile([86, 1], f32, tag="syA")
        nc.vector.tensor_scalar(
            out=syA[:], in0=mA, scalar1=scG1, scalar2=None, op0=OP.mult)
        nc.vector.tensor_sub(syA[:], hA, syA[:])
        nc.vector.tensor_mul(syA[:], rA, syA[:])
        nc.vector.tensor_scalar(
            out=syA[:], in0=syA[:], scalar1=scB1, scalar2=None, op0=OP.add)
        syK = small.tile([86, 1], f32, tag="syK")
        nc.vector.tensor_scalar(
            out=syK[:], in0=mK, scalar1=scG1, scalar2=None, op0=OP.mult)
        nc.vector.tensor_sub(syK[:], hK, syK[:])
        nc.vector.tensor_mul(syK[:], rK, syK[:])
        nc.vector.tensor_scalar(
            out=syK[:], in0=syK[:], scalar1=scB1, scalar2=None, op0=OP.add)

        with tc.tile_pool(name="psG1", bufs=1, space="PSUM") as psG1, \
             tc.tile_pool(name="psG2", bufs=1, space="PSUM") as psG2, \
             tc.tile_pool(name="psLog", bufs=2, space="PSUM") as psLog:

            # rows (mA, tA, rA, term2) -> transpose -> DRAM -> one bcast DMA
            pack = small.tile([86, 4], f32, tag="pack")
            nc.vector.tensor_copy(pack[:, 0:1], mA)
            nc.vector.tensor_copy(pack[:, 1:2], tA)
            nc.vector.tensor_copy(pack[:, 2:3], rA)
            nc.vector.tensor_copy(pack[:, 3:4], t2c[:])
            packT_ps = psG1.tile([4, 86], f32, tag="pT")
            nc.tensor.transpose(packT_ps[:], pack[:], ident[0:86, 0:86])
            packT = small.tile([4, 86], f32, tag="packT")
            nc.scalar.copy(packT[:], packT_ps[:])
            rows_d = dram.tile([4, 86], f32, tag="rowsd")
            nc.gpsimd.dma_start(out=rows_d[:], in_=packT[:])
            bc4 = small.tile([86, 4, 86], f32, tag="bc4")
            nc.gpsimd.dma_start(
                out=bc4[:],
                in_=bass.AP(tensor=rows_d.tensor, offset=rows_d.offset,
                            ap=[[0, 86], [86, 4], [1, 86]]))

            # --- syy ---
            syy = small.tile([86, 97], f32, tag="syy")
            nc.vector.memset(syy[:, 86:96], 0.0)
            nc.vector.scalar_tensor_tensor(
                out=syy[:, 0:86], in0=bc4[:, 0, :], scalar=ntK[:],
                in1=pt_back[:], op0=OP.mult, op1=OP.add)
            nc.vector.scalar_tensor_tensor(
                out=syy[:, 0:86], in0=bc4[:, 1, :], scalar=nmK[:],
                in1=syy[:, 0:86], op0=OP.mult, op1=OP.add)
            nc.vector.scalar_tensor_tensor(
                out=syy[:, 0:86], in0=bc4[:, 0, :], scalar=g2mK[:],
                in1=syy[:, 0:86], op0=OP.mult, op1=OP.add)
            nc.vector.scalar_tensor_tensor(
                out=syy[:, 0:86], in0=bc4[:, 2, :], scalar=rK,
                in1=syy[:, 0:86], op0=OP.mult, op1=OP.mult)
            nc.vector.tensor_add(syy[:, 0:86], syy[:, 0:86], bc4[:, 3, :])
            nc.vector.tensor_scalar(
                out=syy[:, 0:86], in0=syy[:, 0:86], scalar1=t3c[:],
                scalar2=scBb, op0=OP.add, op1=OP.add)
            nc.vector.tensor_copy(syy[:, 96:97], syK[:])

            # --- logits + softmax (recip folded into att) ---
            u_ps = psG2.tile([97, C], f32, tag="uP")
            nc.tensor.matmul(u_ps[:], lhsT=syy[:], rhs=ekt_sb[:],
                             start=True, stop=True)
            u_ext = small.tile([128, C], f32, tag="uext")
            nc.vector.memset(u_ext[64:128, :], 0.0)
            nc.vector.scalar_tensor_tensor(
                out=u_ext[0:86, :], in0=bk_bc[0:86, :], scalar=syA[:],
                in1=u_ps[0:86, :], op0=OP.mult, op1=OP.add)
            nc.vector.tensor_scalar_mul(
                u_ext[96:97, :], bk_bc[96:97, :], float(S))
            nc.vector.tensor_add(u_ext[96:97, :], u_ext[96:97, :],
                                 u_ps[96:97, :])

            att_nrm = []
            recip2 = small.tile([128, 2], f32, tag="recip2")
            z2 = small.tile([128, 2], f32, tag="z2")
            for it in range(2):
                log_ps = psLog.tile([128, 512], f32, tag="lg", name=f"lg{it}")
                nc.tensor.matmul(
                    log_ps[:, 0:C], lhsT=eqt_sb[:, it * 128:(it + 1) * 128],
                    rhs=u_ext[0:97, :], start=True, stop=True)
                rmax = small.tile([128, 1], f32, tag="rmax", name=f"rm{it}")
                nc.vector.reduce_max(rmax[:], log_ps[:, 0:C], axis=AX.X)
                nbias = small.tile([128, 1], f32, tag="nbias", name=f"nb{it}")
                nc.vector.tensor_scalar_mul(nbias[:], rmax[:], -SCALE)
                a_sb = small.tile([128, C], bf16, tag=f"attsb{it}",
                                  name=f"att{it}")
                nc.scalar.activation(
                    out=a_sb[:], in_=log_ps[:, 0:C], func=AF.Exp,
                    bias=nbias[:], scale=SCALE, accum_out=z2[:, it:it + 1])
                nc.vector.reciprocal(recip2[:, it:it + 1], z2[:, it:it + 1])
                a_nr = small.tile([128, C], bf16, tag=f"anrm{it}",
                                  name=f"an{it}")
                nc.scalar.activation(
                    out=a_nr[:], in_=a_sb[:], func=AF.Copy,
                    scale=recip2[:, it:it + 1])
                att_nrm.append(a_nr)

        # --- NT: lhs_m2 [89 rows, 256 q-ch] ---
        psNtc = ctx.enter_context(tc.tile_pool(name="psNtc", bufs=1,
                                               space="PSUM"))
        psAt = ctx.enter_context(tc.tile_pool(name="psAt", bufs=2,
                                              space="PSUM"))
        psO = ctx.enter_context(tc.tile_pool(name="psO", bufs=2,
                                             space="PSUM"))

        ntc_ps = psNtc.tile([128, C], f32, tag="ntc")
        for jt in range(2):
            at_ps = psAt.tile([128, C], bf16, tag="atp", name=f"atp{jt}")
            for it in range(2):
                nc.tensor.transpose(
                    at_ps[:, it * 128:(it + 1) * 128],
                    att_nrm[it][:, jt * 128:(jt + 1) * 128],
                    ident_bf[:])
            at_bf = small.tile([128, C], bf16, tag=f"atbf{jt}", name=f"atb{jt}")
            nc.scalar.copy(at_bf[:], at_ps[:])
            nc.tensor.matmul(
                ntc_ps[0:87, :], lhsT=w0_sb[:, jt, :], rhs=at_bf[:],
                start=(jt == 0), stop=(jt == 1))

        lhs_m2 = small.tile([128, C], bf16, tag="lhsm2")
        rv = rv_ext
        nc.scalar.activation(
            out=lhs_m2[0:64, :], in_=ntc_ps[0:64, :], func=AF.Copy,
            scale=rv[0:64, :])
        nc.scalar.activation(
            out=lhs_m2[64:87, :], in_=ntc_ps[64:87, :], func=AF.Copy,
            scale=rv[64:87, :])
        nc.tensor.matmul(
            ntc_ps[64:66, :], lhsT=mvinv_bf[:],
            rhs=lhs_m2[0:86, :], start=True, stop=True)
        c12_sb = small.tile([128, C], bf16, tag="c12sb")
        nc.scalar.copy(c12_sb[64:66, :], ntc_ps[64:66, :])
        nc.gpsimd.dma_start(out=lhs_m2[87:89, :], in_=c12_sb[64:66, :])

        # --- M2: out = x + att_nrm @ v ---
        nadd = 0
        for it in range(2):
            for ch in range(4):
                ostg = osml.tile([128, 4096], bf16, tag="ostg",
                                 name=f"o{it}{ch}")
                for j in range(4):
                    off = ch * 4096 + j * 1024
                    o_ps = psO.tile([128, 1024], f32, tag="oP",
                                    name=f"op{it}{ch}{j}")
                    for h in range(2):
                        nc.tensor.matmul(
                            o_ps[:, h * 512:(h + 1) * 512],
                            lhsT=lhs_m2[0:89, it * 128:(it + 1) * 128],
                            rhs=rhs_m2[0:89, off + h * 512:off + (h + 1) * 512],
                            start=True, stop=True)
                    nadd += 1
                    nc.vector.tensor_tensor(
                        out=ostg[:, j * 1024:(j + 1) * 1024], in0=o_ps[:],
                        in1=x_sb[:, it, off:off + 1024], op=OP.add)
                nc.sync.dma_start(
                    out=out_d[it * 128:(it + 1) * 128,
                              ch * 4096:(ch + 1) * 4096],
                    in_=ostg[:])

    nc.compile()
    return nc


def _host_prep(x, gamma, beta, w_qkv, b_qkv):
    xf = np.asarray(x, np.float32).reshape(B, C, S)
    gam = np.asarray(gamma, np.float32).reshape(-1)
    bet = np.asarray(beta, np.float32).reshape(-1)
    w_qkv = np.asarray(w_qkv, np.float32)
    b_qkv = np.asarray(b_qkv, np.float32)
    w_q, w_k, w_v = w_qkv[:C], w_qkv[C:2 * C], w_qkv[2 * C:]
    b_q, b_k, b_v = b_qkv[:C], b_qkv[C:2 * C], b_qkv[2 * C:]

    ii = np.arange(C)
    eqt = np.zeros((97, C), np.float32)
    eqt[ii // 3, ii] = w_q
    eqt[96] = b_q
    ekt = np.zeros((86, C), np.float32)
    ekt[(C + ii) // 3 - 85, ii] = w_k
    w0 = np.zeros((C, 87), np.float32)
    w0[ii, (2 * C + ii) // 3 - 170] = w_v
    w0[:, 86] = b_v
    w0 = w0.astype(_BF)

    sc = np.zeros((1, 8), np.float32)
    sc[0, :5] = [gam.sum(), (gam * gam).sum(), (gam * bet).sum(),
                 bet.sum(), (bet * bet).sum()]

    in_maps = []
    for r in range(NCORES):
        b, half = r // 2, r % 2
        sl = slice(half * SH, (half + 1) * SH)
        gl = gam[sl]
        bl = bet[sl]
        gb1r = np.stack([np.ones(SH, np.float32), -gl, bl], 0)

        xl = xf[b][:, sl]                       # [256, 16384]
        xtl = np.ascontiguousarray(xl.T)        # [16384, 256]
        blocks = np.empty((SH, 175), np.float32)
        blocks[:, 0:86] = xtl[:, 0:86]
        blocks[:, 86] = gl * gl
        blocks[:, 87] = gl * bl
        blocks[:, 88] = gl
        blocks[:, 89:175] = xtl[:, 85:171]
        xt = blocks.reshape(NST, 128, 175).transpose(1, 0, 2)
        xt = np.ascontiguousarray(xt.reshape(128, NST * 175)).astype(_BF)

        u2i = blocks[:, 86:89].reshape(NST, 128, 3).transpose(1, 0, 2)
        u2i = np.ascontiguousarray(u2i.reshape(128, NST * 3)).astype(_BF)

        g2c = (gl * gl).reshape(NST, 128).T     # [128, NST]
        g2r = np.repeat(g2c[:, :, None], 86, axis=2)
        g2r = np.ascontiguousarray(g2r.reshape(128, NST * 86)).astype(_BF)

        in_maps.append({
            "xs": np.ascontiguousarray(xl).astype(_BF),
            "xt": xt,
            "u2i": u2i,
            "g2r": g2r,
            "gb1r": gb1r.astype(_BF),
            "eqt": eqt,
            "ekt": ekt,
            "w0": w0,
            "bk": b_k.reshape(1, C).copy(),
            "sc": sc,
        })
    return in_maps


def kernel(x, gamma, beta, w_qkv, b_qkv):
    from concourse.bass_utils import run_bass_kernel_spmd

    if "nc" not in _cache:
        _cache["nc"] = _build_program()
    nc = _cache["nc"]

    in_maps = _host_prep(x, gamma, beta, w_qkv, b_qkv)
    res = run_bass_kernel_spmd(nc, in_maps, core_ids=list(range(NCORES)))
    out = np.empty((B, C, S), np.float32)
    for r in range(NCORES):
        b, half = r // 2, r % 2
        out[b][:, half * SH:(half + 1) * SH] = np.asarray(
            res.results[r]["out"]).astype(np.float32)
    return out.reshape(np.asarray(x).shape)


if __name__ == "__main__":
    rng = np.random.default_rng(0)
    inputs = {
        "x": rng.standard_normal((B, C, 32, 32, 32)).astype(np.float32),
        "gamma": (1 + 0.1 * rng.standard_normal((32, 32, 32))).astype(np.float32),
        "beta": (0.1 * rng.standard_normal((32, 32, 32))).astype(np.float32),
        "w_qkv": (0.5 * rng.standard_normal(3 * C)).astype(np.float32),
        "b_qkv": (0.05 * rng.standard_normal(3 * C)).astype(np.float32),
    }
    o = kernel(**inputs)
    print("out", o.shape, o.dtype, float(np.abs(o).mean()))


# revision 16
# speedup vs baseline: 1.4627x; 1.0140x over previous
"""Channel-self-attention (LayerNorm + grouped-1x1-qkv + channel softmax attn
+ residual) on 8 TRN2 NeuronCores.

Strategy (v3): pair-sharding — 2 cores per batch, each core owns one
spatial half (16384 of 32768). One ~34 KB 2-rank Mesh AllReduce per core.

Per core:
 - x half-shard [256, 16384] bf16 resident in SBUF (channel-major)
 - host also sends x TRANSPOSED (spatial-major, bf16) packed per 128-row
   stile as [x_A(86) | g2 gb g (3) | x_K(86)] so the Gram matmul needs NO
   on-chip transposes:
     lhsT = [g2 gb g | g2*x_K]   (g2*x_K built by 8 bulk chunk DVE mults
                                  against a host-replicated gamma^2 tile)
     rhs  = the raw packed stile
     out  = [89,175]: rows 0..2 x cols 0..85 = tgh_A, rows 3..88 = P^T,
            rows 0..2 x cols 89..174 = tgh_K
 - stats: Sum x via DVE reduce, Sum x^2 via Scalar Square+accum (idle
   engine), replacing bn_stats
 - ONE AllReduce (Gram + tgh + stats, 33.7 KB) within the batch pair
 - logits from the Gram expansion of the LayerNorm algebra; softmax
   normalization folded into att before the transpose, so the epilogue is
   a plain  out = x + att_nrm @ v  residual add (split DVE/GpSimd), with
   bf16 output upcast on host
"""
import sys

sys.path.insert(0, "/opt/trn_rl_repo")

import numpy as np
import ml_dtypes

B, C = 4, 256
S = 32 * 32 * 32          # 32768 global spatial
NCORES = 8
SH = S // 2               # 16384 per-core spatial half
NST = SH // 128           # 128 stiles
NCH = 8                   # Gram stream chunks
CST = NST // NCH          # 16 stiles per chunk
EPS = 1e-5
SCALE = float(S) ** -0.5

_BF = ml_dtypes.bfloat16

_cache = {}


def _build_program():
    from contextlib import ExitStack
    import concourse.bass as bass
    import concourse.bacc as bacc
    import concourse.tile as tile
    from concourse import mybir, masks

    f32 = mybir.dt.float32
    bf16 = mybir.dt.bfloat16
    AF = mybir.ActivationFunctionType
    OP = mybir.AluOpType
    AX = mybir.AxisListType

    nc = bacc.Bacc(
        "TRN2",
        target_bir_lowering=False,
        debug=False,
        enable_asserts=False,
        num_devices=NCORES,
    )

    # ---------------- DRAM I/O ----------------
    xs_d = nc.dram_tensor("xs", [C, SH], bf16, kind="ExternalInput")
    xt_d = nc.dram_tensor("xt", [128, NST * 175], bf16, kind="ExternalInput")
    g2e_d = nc.dram_tensor("g2e", [128, NST * 89], bf16, kind="ExternalInput")
    gb1r_d = nc.dram_tensor("gb1r", [3, SH], bf16, kind="ExternalInput")
    eqt_d = nc.dram_tensor("eqt", [97, C], f32, kind="ExternalInput")
    ekt_d = nc.dram_tensor("ekt", [86, C], f32, kind="ExternalInput")
    w0_d = nc.dram_tensor("w0", [2 * 128, 87], bf16, kind="ExternalInput")
    bk_d = nc.dram_tensor("bk", [1, C], f32, kind="ExternalInput")
    sc_d = nc.dram_tensor("sc", [1, 8], f32, kind="ExternalInput")
    out_d = nc.dram_tensor("out", [C, SH], bf16, kind="ExternalOutput")

    # AR payload layout (f32 words):
    #   [0 : 7654)        M[0:89, 0:86] row-major  (tgh_A rows 0..2, P^T 3..88)
    #   [7654 : 7912)     M[0:3, 89:175] row-major (tgh_K)
    #   [7912 : 8168)     Sum x   per channel (flat idx = channel)
    #   [8168 : 8424)     Sum x^2 per channel
    PTOT = 8424

    with tile.TileContext(nc) as tc, ExitStack() as ctx:
        const = ctx.enter_context(tc.tile_pool(name="const", bufs=1))
        xpool = ctx.enter_context(tc.tile_pool(name="xpool", bufs=1))
        xtp = ctx.enter_context(tc.tile_pool(name="xtp", bufs=2))
        utp = ctx.enter_context(tc.tile_pool(name="utp", bufs=2))
        g2p = ctx.enter_context(tc.tile_pool(name="g2p", bufs=2))
        rhsp = ctx.enter_context(tc.tile_pool(name="rhsp", bufs=1))
        osml = ctx.enter_context(tc.tile_pool(name="osml", bufs=2))
        small = ctx.enter_context(tc.tile_pool(name="small", bufs=2))
        dram = ctx.enter_context(tc.tile_pool(name="dram", bufs=1, space="DRAM"))

        # ------------- constants / inputs to SBUF -------------
        ident = const.tile([128, 128], f32)
        masks.make_identity(nc, ident[:])
        ident_bf = const.tile([128, 128], bf16)
        masks.make_identity(nc, ident_bf[:])
        eqt_sb = const.tile([97, C], f32)
        nc.sync.dma_start(out=eqt_sb[:], in_=eqt_d.ap())
        ekt_sb = const.tile([86, C], f32)
        nc.sync.dma_start(out=ekt_sb[:], in_=ekt_d.ap())
        w0_sb = const.tile([128, 2, 87], bf16)
        for jt in range(2):
            nc.sync.dma_start(out=w0_sb[:, jt, :], in_=w0_d[jt * 128:(jt + 1) * 128, :])

        def dram_bcast(dst, src_d, nparts, nfree, off=0):
            nc.gpsimd.dma_start(
                out=dst,
                in_=bass.AP(tensor=src_d, offset=off,
                            ap=[[0, nparts], [1, nfree]]))

        bk_bc = const.tile([128, C], f32)
        dram_bcast(bk_bc[:], bk_d, 128, C)
        sc_bc = const.tile([128, 8], f32)
        dram_bcast(sc_bc[:], sc_d, 128, 8)
        gam_bc = const.tile([128, SH], bf16)
        dram_bcast(gam_bc[:], gb1r_d, 128, SH, off=SH)
        nc.vector.tensor_scalar_mul(gam_bc[:], gam_bc[:], -1.0)

        # Gram streams: xt on gpsimd queue, g2e on sync queue (before x)
        xt_sb, u2t_sb, g2e_sb = [], [], []
        for c in range(NCH):
            t = xtp.tile([128, CST, 175], bf16, tag="xt", name=f"xt{c}")
            nc.gpsimd.dma_start(
                out=t[:],
                in_=xt_d[:, c * CST * 175:(c + 1) * CST * 175])
            xt_sb.append(t)
            u = utp.tile([128, CST, 89], bf16, tag="u2t", name=f"u2{c}")
            u2t_sb.append(u)
            g = g2p.tile([128, CST, 89], bf16, tag="g2e", name=f"g2{c}")
            nc.sync.dma_start(
                out=g[:],
                in_=g2e_d[:, c * CST * 89:(c + 1) * CST * 89])
            g2e_sb.append(g)

        # x resident bf16 [128, 2, 16384]
        x_sb = xpool.tile([128, 2, SH], bf16)
        for ct in range(2):
            nc.sync.dma_start(
                out=x_sb[:, ct, :],
                in_=xs_d[ct * 128:(ct + 1) * 128, :])

        # ------------- Gram over 128 stiles (8 chunks) -------------
        bncP_in = dram.tile([PTOT], f32)
        bncP_out = dram.tile([PTOT], f32)

        with tc.tile_pool(name="s1ps", bufs=1, space="PSUM") as stg1ps:
            ptk_ps = stg1ps.tile([89, 175], f32)
            for c in range(NCH):
                nc.vector.tensor_tensor(
                    out=u2t_sb[c][:], in0=xt_sb[c][:, :, 86:175],
                    in1=g2e_sb[c][:], op=OP.mult)
                for j in range(CST):
                    st = c * CST + j
                    nc.tensor.matmul(
                        ptk_ps[:], lhsT=u2t_sb[c][:, j, :],
                        rhs=xt_sb[c][:, j, :],
                        start=(st == 0), stop=(st == NST - 1))

            # ------- stats: Sum x (DVE), Sum x^2 (Scalar) -------
            sums_sb = const.tile([128, 2], f32)
            sqp = const.tile([128, 8], f32)
            sqs_sb = const.tile([128, 2], f32)
            for ct in range(2):
                nc.vector.reduce_sum(
                    sums_sb[:, ct:ct + 1], x_sb[:, ct, :], axis=AX.X)
                for cc in range(4):
                    scr = osml.tile([128, 4096], bf16, tag="ostg",
                                    name=f"sq{ct}{cc}")
                    nc.scalar.activation(
                        out=scr[:], in_=x_sb[:, ct, cc * 4096:(cc + 1) * 4096],
                        func=AF.Square,
                        accum_out=sqp[:, 4 * ct + cc:4 * ct + cc + 1])
                nc.vector.reduce_sum(
                    sqs_sb[:, ct:ct + 1], sqp[:, 4 * ct:4 * ct + 4], axis=AX.X)

            ptk_sb = small.tile([89, 86], f32, tag="ptksb", bufs=1)
            nc.scalar.copy(ptk_sb[:], ptk_ps[0:89, 0:86])
            ptk3_sb = small.tile([3, 86], f32, tag="ptk3sb", bufs=1)
            nc.scalar.copy(ptk3_sb[:], ptk_ps[0:3, 89:175])

        # ------------- rhs for M2 (independent of AR) -------------
        # rows 0..85 = gamma*x_V (ch 170..255), 86..88 = [ones, -gamma, beta]
        rhs_m2 = rhsp.tile([128, SH], bf16)
        nc.gpsimd.dma_start(out=rhs_m2[0:86, :], in_=x_sb[42:128, 1, :])
        nc.vector.tensor_tensor(
            out=rhs_m2[0:86, :], in0=rhs_m2[0:86, :], in1=gam_bc[0:86, :],
            op=OP.mult)
        nc.gpsimd.dma_start(out=rhs_m2[86:89, :], in_=gb1r_d.ap())

        # ------------- AllReduce within the batch pair -------------
        nc.gpsimd.dma_start(
            out=bncP_in[0:7654].rearrange("(p f) -> p f", f=86),
            in_=ptk_sb[:])
        nc.gpsimd.dma_start(
            out=bncP_in[7654:7912].rearrange("(p f) -> p f", f=86),
            in_=ptk3_sb[:])
        nc.gpsimd.dma_start(
            out=bncP_in[7912:8168].rearrange("(t p) -> p t", p=128),
            in_=sums_sb[:])
        nc.gpsimd.dma_start(
            out=bncP_in[8168:8424].rearrange("(t p) -> p t", p=128),
            in_=sqs_sb[:])
        nc.gpsimd.collective_compute(
            "AllReduce", OP.add,
            replica_groups=[[0, 1], [2, 3], [4, 5], [6, 7]],
            ins=[bncP_in[:].opt()], outs=[bncP_out[:].opt()])

        # ------------- DMA back -------------
        pt_back = const.tile([86, 86], f32)
        nc.sync.dma_start(
            out=pt_back[:],
            in_=bass.AP(tensor=bncP_out.tensor,
                        offset=bncP_out.offset + 3 * 86,
                        ap=[[86, 86], [1, 86]]))
        tga = const.tile([86, 3], f32)
        nc.sync.dma_start(
            out=tga[:],
            in_=bass.AP(tensor=bncP_out.tensor, offset=bncP_out.offset,
                        ap=[[1, 86], [86, 3]]))
        tgk = const.tile([86, 3], f32)
        nc.sync.dma_start(
            out=tgk[:],
            in_=bass.AP(tensor=bncP_out.tensor,
                        offset=bncP_out.offset + 7654,
                        ap=[[1, 86], [86, 3]]))
        # stats cols: [p, {Sx,Sxx} x {A,K,V}]
        sAK = const.tile([86, 6], f32)
        for k, koff in ((0, 7912), (1, 8168)):
            for g, goff in ((0, 0), (1, 85), (2, 170)):
                nc.sync.dma_start(
                    out=sAK[:, 3 * k + g:3 * k + g + 1],
                    in_=bass.AP(tensor=bncP_out.tensor,
                                offset=bncP_out.offset + koff + goff,
                                ap=[[1, 86], [1, 1]]))

        invS = 1.0 / float(S)

        # --- per-channel LayerNorm scalars ---
        mAK = small.tile([86, 3], f32, tag="mAK")
        nc.vector.tensor_scalar(
            out=mAK[:], in0=sAK[:, 0:3], scalar1=invS, scalar2=None,
            op0=OP.mult)
        vAK = small.tile([86, 3], f32, tag="vAK")
        nc.vector.tensor_scalar(
            out=vAK[:], in0=sAK[:, 3:6], scalar1=invS, scalar2=EPS,
            op0=OP.mult, op1=OP.add)
        msq = small.tile([86, 3], f32, tag="msq")
        nc.vector.tensor_mul(msq[:], mAK[:], mAK[:])
        nc.vector.tensor_sub(vAK[:], vAK[:], msq[:])
        nc.scalar.activation(out=vAK[:], in_=vAK[:], func=AF.Sqrt)
        rAK = small.tile([86, 3], f32, tag="rAK")
        nc.vector.reciprocal(rAK[:], vAK[:])
        invrV = small.tile([86, 1], f32, tag="invrV")
        nc.vector.reciprocal(invrV[:], rAK[:, 2:3])
        mvinv_bf = small.tile([86, 2], bf16, tag="mvinv")
        nc.vector.tensor_copy(mvinv_bf[:, 0:1], mAK[:, 2:3])
        nc.vector.tensor_copy(mvinv_bf[:, 1:2], invrV[:])
        rv_ext = small.tile([128, 1], f32, tag="rvext")
        nc.vector.memset(rv_ext[64:128, :], 1.0)
        nc.vector.tensor_copy(rv_ext[0:86, :], rAK[:, 2:3])

        tA, gA, hA = tga[:, 0:1], tga[:, 1:2], tga[:, 2:3]
        tK, gK, hK = tgk[:, 0:1], tgk[:, 1:2], tgk[:, 2:3]
        mA, mK = mAK[:, 0:1], mAK[:, 1:2]
        rA, rK = rAK[:, 0:1], rAK[:, 1:2]
        scG1 = sc_bc[0:86, 0:1]
        scG2 = sc_bc[0:86, 1:2]
        scGb = sc_bc[0:86, 2:3]
        scB1 = sc_bc[0:86, 3:4]
        scBb = sc_bc[0:86, 4:5]

        ntK = small.tile([86, 1], f32, tag="ntK")
        nc.vector.tensor_scalar_mul(ntK[:], tK, -1.0)
        nmK = small.tile([86, 1], f32, tag="nmK")
        nc.vector.tensor_scalar_mul(nmK[:], mK, -1.0)
        g2mK = small.tile([86, 1], f32, tag="g2mK")
        nc.vector.tensor_scalar(
            out=g2mK[:], in0=mK, scalar1=scG2, scalar2=None, op0=OP.mult)
        t3c = small.tile([86, 1], f32, tag="t3c")
        nc.vector.tensor_scalar(
            out=t3c[:], in0=mK, scalar1=scGb, scalar2=None, op0=OP.mult)
        nc.vector.tensor_sub(t3c[:], gK, t3c[:])
        nc.vector.tensor_mul(t3c[:], rK, t3c[:])
        t2c = small.tile([86, 1], f32, tag="t2c")
        nc.vector.tensor_scalar(
            out=t2c[:], in0=mA, scalar1=scGb, scalar2=None, op0=OP.mult)
        nc.vector.tensor_sub(t2c[:], gA, t2c[:])
        nc.vector.tensor_mul(t2c[:], rA, t2c[:])
        syA = small.tile([86, 1], f32, tag="syA")
        nc.vector.tensor_scalar(
            out=syA[:], in0=mA, scalar1=scG1, scalar2=None, op0=OP.mult)
        nc.vector.tensor_sub(syA[:], hA, syA[:])
        nc.vector.tensor_mul(syA[:], rA, syA[:])
        nc.vector.tensor_scalar(
            out=syA[:], in0=syA[:], scalar1=scB1, scalar2=None, op0=OP.add)
        syK = small.tile([86, 1], f32, tag="syK")
        nc.vector.tensor_scalar(
            out=syK[:], in0=mK, scalar1=scG1, scalar2=None, op0=OP.mult)
        nc.vector.tensor_sub(syK[:], hK, syK[:])
        nc.vector.tensor_mul(syK[:], rK, syK[:])
        nc.vector.tensor_scalar(
            out=syK[:], in0=syK[:], scalar1=scB1, scalar2=None, op0=OP.add)

        with tc.tile_pool(name="psG1", bufs=1, space="PSUM") as psG1, \
             tc.tile_pool(name="psG2", bufs=1, space="PSUM") as psG2, \
             tc.tile_pool(name="psLog", bufs=2, space="PSUM") as psLog:

            # rows (mA, tA, rA, term2) -> transpose -> DRAM -> one bcast DMA
            pack = small.tile([86, 4], f32, tag="pack")
            nc.vector.tensor_copy(pack[:, 0:1], mA)
            nc.vector.tensor_copy(pack[:, 1:2], tA)
            nc.vector.tensor_copy(pack[:, 2:3], rA)
            nc.vector.tensor_copy(pack[:, 3:4], t2c[:])
            packT_ps = psG1.tile([4, 86], f32, tag="pT")
            nc.tensor.transpose(packT_ps[:], pack[:], ident[0:86, 0:86])
            packT = small.tile([4, 86], f32, tag="packT")
            nc.scalar.copy(packT[:], packT_ps[:])
            rows_d = dram.tile([4, 86], f32, tag="rowsd")
            nc.gpsimd.dma_start(out=rows_d[:], in_=packT[:])
            bc4 = small.tile([86, 4, 86], f32, tag="bc4")
            nc.gpsimd.dma_start(
                out=bc4[:],
                in_=bass.AP(tensor=rows_d.tensor, offset=rows_d.offset,
                            ap=[[0, 86], [86, 4], [1, 86]]))

            # --- syy ---
            syy = small.tile([86, 97], f32, tag="syy")
            nc.vector.memset(syy[:, 86:96], 0.0)
            nc.vector.scalar_tensor_tensor(
                out=syy[:, 0:86], in0=bc4[:, 0, :], scalar=ntK[:],
                in1=pt_back[:], op0=OP.mult, op1=OP.add)
            nc.vector.scalar_tensor_tensor(
                out=syy[:, 0:86], in0=bc4[:, 1, :], scalar=nmK[:],
                in1=syy[:, 0:86], op0=OP.mult, op1=OP.add)
            nc.vector.scalar_tensor_tensor(
                out=syy[:, 0:86], in0=bc4[:, 0, :], scalar=g2mK[:],
                in1=syy[:, 0:86], op0=OP.mult, op1=OP.add)
            nc.vector.scalar_tensor_tensor(
                out=syy[:, 0:86], in0=bc4[:, 2, :], scalar=rK,
                in1=syy[:, 0:86], op0=OP.mult, op1=OP.mult)
            nc.vector.tensor_add(syy[:, 0:86], syy[:, 0:86], bc4[:, 3, :])
            nc.vector.tensor_scalar(
                out=syy[:, 0:86], in0=syy[:, 0:86], scalar1=t3c[:],
                scalar2=scBb, op0=OP.add, op1=OP.add)
            nc.vector.tensor_copy(syy[:, 96:97], syK[:])

            # --- logits + softmax (recip folded into att) ---
            u_ps = psG2.tile([97, C], f32, tag="uP")
            nc.tensor.matmul(u_ps[:], lhsT=syy[:], rhs=ekt_sb[:],
                             start=True, stop=True)
            u_ext = small.tile([128, C], f32, tag="uext")
            nc.vector.memset(u_ext[64:128, :], 0.0)
            nc.vector.scalar_tensor_tensor(
                out=u_ext[0:86, :], in0=bk_bc[0:86, :], scalar=syA[:],
                in1=u_ps[0:86, :], op0=OP.mult, op1=OP.add)
            nc.vector.tensor_scalar_mul(
                u_ext[96:97, :], bk_bc[96:97, :], float(S))
            nc.vector.tensor_add(u_ext[96:97, :], u_ext[96:97, :],
                                 u_ps[96:97, :])

            att_nrm = []
            recip2 = small.tile([128, 2], f32, tag="recip2")
            z2 = small.tile([128, 2], f32, tag="z2")
            for it in range(2):
                log_ps = psLog.tile([128, 512], f32, tag="lg", name=f"lg{it}")
                nc.tensor.matmul(
                    log_ps[:, 0:C], lhsT=eqt_sb[:, it * 128:(it + 1) * 128],
                    rhs=u_ext[0:97, :], start=True, stop=True)
                rmax = small.tile([128, 1], f32, tag="rmax", name=f"rm{it}")
                nc.vector.reduce_max(rmax[:], log_ps[:, 0:C], axis=AX.X)
                nbias = small.tile([128, 1], f32, tag="nbias", name=f"nb{it}")
                nc.vector.tensor_scalar_mul(nbias[:], rmax[:], -SCALE)
                a_sb = small.tile([128, C], bf16, tag=f"attsb{it}",
                                  name=f"att{it}")
                nc.scalar.activation(
                    out=a_sb[:], in_=log_ps[:, 0:C], func=AF.Exp,
                    bias=nbias[:], scale=SCALE, accum_out=z2[:, it:it + 1])
                nc.vector.reciprocal(recip2[:, it:it + 1], z2[:, it:it + 1])
                a_nr = small.tile([128, C], bf16, tag=f"anrm{it}",
                                  name=f"an{it}")
                nc.scalar.activation(
                    out=a_nr[:], in_=a_sb[:], func=AF.Copy,
                    scale=recip2[:, it:it + 1])
                att_nrm.append(a_nr)

        # --- NT: lhs_m2 [89 rows, 256 q-ch] ---
        psNtc = ctx.enter_context(tc.tile_pool(name="psNtc", bufs=1,
                                               space="PSUM"))
        psAt = ctx.enter_context(tc.tile_pool(name="psAt", bufs=2,
                                              space="PSUM"))
        psO = ctx.enter_context(tc.tile_pool(name="psO", bufs=2,
                                             space="PSUM"))

        ntc_ps = psNtc.tile([128, C], f32, tag="ntc")
        for jt in range(2):
            at_ps = psAt.tile([128, C], bf16, tag="atp", name=f"atp{jt}")
            for it in range(2):
                nc.tensor.transpose(
                    at_ps[:, it * 128:(it + 1) * 128],
                    att_nrm[it][:, jt * 128:(jt + 1) * 128],
                    ident_bf[:])
            at_bf = small.tile([128, C], bf16, tag=f"atbf{jt}", name=f"atb{jt}")
            nc.scalar.copy(at_bf[:], at_ps[:])
            nc.tensor.matmul(
                ntc_ps[0:87, :], lhsT=w0_sb[:, jt, :], rhs=at_bf[:],
                start=(jt == 0), stop=(jt == 1))

        lhs_m2 = small.tile([128, C], bf16, tag="lhsm2")
        rv = rv_ext
        nc.scalar.activation(
            out=lhs_m2[0:64, :], in_=ntc_ps[0:64, :], func=AF.Copy,
            scale=rv[0:64, :])
        nc.scalar.activation(
            out=lhs_m2[64:87, :], in_=ntc_ps[64:87, :], func=AF.Copy,
            scale=rv[64:87, :])
        nc.tensor.matmul(
            ntc_ps[64:66, :], lhsT=mvinv_bf[:],
            rhs=lhs_m2[0:86, :], start=True, stop=True)
        c12_sb = small.tile([128, C], bf16, tag="c12sb")
        nc.scalar.copy(c12_sb[64:66, :], ntc_ps[64:66, :])
        nc.gpsimd.dma_start(out=lhs_m2[87:89, :], in_=c12_sb[64:66, :])

        # --- M2: out = x + att_nrm @ v ---
        nadd = 0
        for it in range(2):
            for ch in range(4):
                ostg = osml.tile([128, 4096], bf16, tag="ostg",
                                 name=f"o{it}{ch}")
                for j in range(4):
                    off = ch * 4096 + j * 1024
                    o_ps = psO.tile([128, 1024], f32, tag="oP",
                                    name=f"op{it}{ch}{j}")
                    for h in range(2):
                        nc.tensor.matmul(
                            o_ps[:, h * 512:(h + 1) * 512],
                            lhsT=lhs_m2[0:89, it * 128:(it + 1) * 128],
                            rhs=rhs_m2[0:89, off + h * 512:off + (h + 1) * 512],
                            start=True, stop=True)
                    nadd += 1
                    nc.vector.tensor_tensor(
                        out=ostg[:, j * 1024:(j + 1) * 1024], in0=o_ps[:],
                        in1=x_sb[:, it, off:off + 1024], op=OP.add)
                nc.sync.dma_start(
                    out=out_d[it * 128:(it + 1) * 128,
                              ch * 4096:(ch + 1) * 4096],
                    in_=ostg[:])

    nc.compile()
    return nc


def _host_prep(x, gamma, beta, w_qkv, b_qkv):
    xf = np.asarray(x, np.float32).reshape(B, C, S)
    gam = np.asarray(gamma, np.float32).reshape(-1)
    bet = np.asarray(beta, np.float32).reshape(-1)
    w_qkv = np.asarray(w_qkv, np.float32)
    b_qkv = np.asarray(b_qkv, np.float32)
    w_q, w_k, w_v = w_qkv[:C], w_qkv[C:2 * C], w_qkv[2 * C:]
    b_q, b_k, b_v = b_qkv[:C], b_qkv[C:2 * C], b_qkv[2 * C:]

    ii = np.arange(C)
    eqt = np.zeros((97, C), np.float32)
    eqt[ii // 3, ii] = w_q
    eqt[96] = b_q
    ekt = np.zeros((86, C), np.float32)
    ekt[(C + ii) // 3 - 85, ii] = w_k
    w0 = np.zeros((C, 87), np.float32)
    w0[ii, (2 * C + ii) // 3 - 170] = w_v
    w0[:, 86] = b_v
    w0 = w0.astype(_BF)

    sc = np.zeros((1, 8), np.float32)
    sc[0, :5] = [gam.sum(), (gam * gam).sum(), (gam * bet).sum(),
                 bet.sum(), (bet * bet).sum()]

    in_maps = []
    for r in range(NCORES):
        b, half = r // 2, r % 2
        sl = slice(half * SH, (half + 1) * SH)
        gl = gam[sl]
        bl = bet[sl]
        gb1r = np.stack([np.ones(SH, np.float32), -gl, bl], 0)

        xl = xf[b][:, sl]                       # [256, 16384]
        xtl = np.ascontiguousarray(xl.T)        # [16384, 256]
        blocks = np.empty((SH, 175), np.float32)
        blocks[:, 0:86] = xtl[:, 0:86]
        blocks[:, 86] = gl * gl
        blocks[:, 87] = gl * bl
        blocks[:, 88] = gl
        blocks[:, 89:175] = xtl[:, 85:171]
        xt = blocks.reshape(NST, 128, 175).transpose(1, 0, 2)
        xt = np.ascontiguousarray(xt.reshape(128, NST * 175)).astype(_BF)

        g2c = (gl * gl).reshape(NST, 128).T     # [128, NST]
        g2e = np.empty((128, NST, 89), np.float32)
        g2e[:, :, 0:3] = 1.0
        g2e[:, :, 3:89] = g2c[:, :, None]
        g2e = np.ascontiguousarray(g2e.reshape(128, NST * 89)).astype(_BF)

        in_maps.append({
            "xs": np.ascontiguousarray(xl).astype(_BF),
            "xt": xt,
            "g2e": g2e,
            "gb1r": gb1r.astype(_BF),
            "eqt": eqt,
            "ekt": ekt,
            "w0": w0,
            "bk": b_k.reshape(1, C).copy(),
            "sc": sc,
        })
    return in_maps


def kernel(x, gamma, beta, w_qkv, b_qkv):
    from concourse.bass_utils import run_bass_kernel_spmd

    if "nc" not in _cache:
        _cache["nc"] = _build_program()
    nc = _cache["nc"]

    in_maps = _host_prep(x, gamma, beta, w_qkv, b_qkv)
    res = run_bass_kernel_spmd(nc, in_maps, core_ids=list(range(NCORES)))
    out = np.empty((B, C, S), np.float32)
    for r in range(NCORES):
        b, half = r // 2, r % 2
        out[b][:, half * SH:(half + 1) * SH] = np.asarray(
            res.results[r]["out"]).astype(np.float32)
    return out.reshape(np.asarray(x).shape)


if __name__ == "__main__":
    rng = np.random.default_rng(0)
    inputs = {
        "x": rng.standard_normal((B, C, 32, 32, 32)).astype(np.float32),
        "gamma": (1 + 0.1 * rng.standard_normal((32, 32, 32))).astype(np.float32),
        "beta": (0.1 * rng.standard_normal((32, 32, 32))).astype(np.float32),
        "w_qkv": (0.5 * rng.standard_normal(3 * C)).astype(np.float32),
        "b_qkv": (0.05 * rng.standard_normal(3 * C)).astype(np.float32),
    }
    o = kernel(**inputs)
    print("out", o.shape, o.dtype, float(np.abs(o).mean()))


# revision 22
# speedup vs baseline: 1.5291x; 1.0453x over previous
"""Channel-self-attention (LayerNorm + grouped-1x1-qkv + channel softmax attn
+ residual) on 8 TRN2 NeuronCores.

Strategy (v3): pair-sharding — 2 cores per batch, each core owns one
spatial half (16384 of 32768). One ~34 KB 2-rank Mesh AllReduce per core.

Per core:
 - x half-shard [256, 16384] bf16 resident in SBUF (channel-major)
 - host also sends x TRANSPOSED (spatial-major, bf16) packed per 128-row
   stile as [x_A(86) | g2 gb g (3) | x_K(86)] so the Gram matmul needs NO
   on-chip transposes:
     lhsT = [g2 gb g | g2*x_K]   (g2*x_K built by 8 bulk chunk DVE mults
                                  against a host-replicated gamma^2 tile)
     rhs  = the raw packed stile
     out  = [89,175]: rows 0..2 x cols 0..85 = tgh_A, rows 3..88 = P^T,
            rows 0..2 x cols 89..174 = tgh_K
 - stats: Sum x via DVE reduce, Sum x^2 via Scalar Square+accum (idle
   engine), replacing bn_stats
 - ONE AllReduce (Gram + tgh + stats, 33.7 KB) within the batch pair
 - logits from the Gram expansion of the LayerNorm algebra; softmax
   normalization folded into att before the transpose, so the epilogue is
   a plain  out = x + att_nrm @ v  residual add (split DVE/GpSimd), with
   bf16 output upcast on host
"""
import sys

sys.path.insert(0, "/opt/trn_rl_repo")

import numpy as np
import ml_dtypes

B, C = 4, 256
S = 32 * 32 * 32          # 32768 global spatial
NCORES = 8
SH = S // 2               # 16384 per-core spatial half
NST = SH // 128           # 128 stiles
NCH = 8                   # Gram stream chunks
CST = NST // NCH          # 16 stiles per chunk
EPS = 1e-5
SCALE = float(S) ** -0.5

_BF = ml_dtypes.bfloat16

_cache = {}


def _build_program():
    from contextlib import ExitStack
    import concourse.bass as bass
    import concourse.bacc as bacc
    import concourse.tile as tile
    from concourse import mybir, masks

    f32 = mybir.dt.float32
    bf16 = mybir.dt.bfloat16
    AF = mybir.ActivationFunctionType
    OP = mybir.AluOpType
    AX = mybir.AxisListType

    nc = bacc.Bacc(
        "TRN2",
        target_bir_lowering=False,
        debug=False,
        enable_asserts=False,
        num_devices=NCORES,
    )

    # ---------------- DRAM I/O ----------------
    xs_d = nc.dram_tensor("xs", [C, SH], bf16, kind="ExternalInput")
    xt_d = nc.dram_tensor("xt", [128, NST * 176], bf16, kind="ExternalInput")
    g2e_d = nc.dram_tensor("g2e", [128, NST * 90], bf16, kind="ExternalInput")
    gb1r_d = nc.dram_tensor("gb1r", [3, SH], bf16, kind="ExternalInput")
    eqt_d = nc.dram_tensor("eqt", [97, C], f32, kind="ExternalInput")
    ekt_d = nc.dram_tensor("ekt", [86, C], f32, kind="ExternalInput")
    w0_d = nc.dram_tensor("w0", [2 * 128, 87], bf16, kind="ExternalInput")
    bk_d = nc.dram_tensor("bk", [1, C], f32, kind="ExternalInput")
    sc_d = nc.dram_tensor("sc", [1, 8], f32, kind="ExternalInput")
    out_d = nc.dram_tensor("out", [C, SH], bf16, kind="ExternalOutput")

    # AR payload layout (f32 words). M = [90,175] Gram PSUM; lhsT col 89
    # is ones so M row 89 = per-channel Sum x for A (cols 0:86) / K (89:175).
    #   [0 : 7740)        M[0:90, 0:86] row-major (tgh_A 0:3, P^T 3:89, SxA 89)
    #   [7740 : 7998)     M[0:3, 89:175] row-major (tgh_K)
    #   [7998 : 8084)     M[89, 89:175]  (Sum x_K)
    #   [8084 : 8170)     Sum x_V (DVE reduce over partitions 42:128 of ct1)
    #   [8170 : 8426)     Sum x^2 per channel
    PTOT = 8426

    with tile.TileContext(nc) as tc, ExitStack() as ctx:
        const = ctx.enter_context(tc.tile_pool(name="const", bufs=1))
        xpool = ctx.enter_context(tc.tile_pool(name="xpool", bufs=1))
        xtp = ctx.enter_context(tc.tile_pool(name="xtp", bufs=2))
        utp = ctx.enter_context(tc.tile_pool(name="utp", bufs=2))
        g2p = ctx.enter_context(tc.tile_pool(name="g2p", bufs=1))
        rhsp = ctx.enter_context(tc.tile_pool(name="rhsp", bufs=1))
        osml = ctx.enter_context(tc.tile_pool(name="osml", bufs=2))
        small = ctx.enter_context(tc.tile_pool(name="small", bufs=2))
        dram = ctx.enter_context(tc.tile_pool(name="dram", bufs=1, space="DRAM"))

        # ------------- constants / inputs to SBUF -------------
        ident = const.tile([128, 128], f32)
        masks.make_identity(nc, ident[:])
        ident_bf = const.tile([128, 128], bf16)
        masks.make_identity(nc, ident_bf[:])
        eqt_sb = const.tile([97, C], f32)
        nc.sync.dma_start(out=eqt_sb[:], in_=eqt_d.ap())
        ekt_sb = const.tile([86, C], f32)
        nc.sync.dma_start(out=ekt_sb[:], in_=ekt_d.ap())
        w0_sb = const.tile([128, 2, 87], bf16)
        for jt in range(2):
            nc.sync.dma_start(out=w0_sb[:, jt, :], in_=w0_d[jt * 128:(jt + 1) * 128, :])

        def dram_bcast(dst, src_d, nparts, nfree, off=0):
            nc.gpsimd.dma_start(
                out=dst,
                in_=bass.AP(tensor=src_d, offset=off,
                            ap=[[0, nparts], [1, nfree]]))

        bk_bc = const.tile([128, C], f32)
        dram_bcast(bk_bc[:], bk_d, 128, C)
        sc_bc = const.tile([128, 8], f32)
        dram_bcast(sc_bc[:], sc_d, 128, 8)
        gam_bc = const.tile([128, SH], bf16)
        dram_bcast(gam_bc[:], gb1r_d, 128, SH, off=SH)
        nc.vector.tensor_scalar_mul(gam_bc[:], gam_bc[:], -1.0)

        # Gram streams: xt chunks on gpsimd queue; g2e resident via scalar
        g2e_sb = g2p.tile([128, NST, 90], bf16)
        nc.scalar.dma_start(out=g2e_sb[:], in_=g2e_d.ap())
        xt_sb, u2t_sb = [], []
        for c in range(NCH):
            t = xtp.tile([128, CST, 176], bf16, tag="xt", name=f"xt{c}")
            nc.gpsimd.dma_start(
                out=t[:],
                in_=xt_d[:, c * CST * 176:(c + 1) * CST * 176])
            xt_sb.append(t)
            u = utp.tile([128, CST, 90], bf16, tag="u2t", name=f"u2{c}")
            u2t_sb.append(u)

        # x resident bf16 [128, 2, 16384]
        x_sb = xpool.tile([128, 2, SH], bf16)
        for ct in range(2):
            nc.sync.dma_start(
                out=x_sb[:, ct, :],
                in_=xs_d[ct * 128:(ct + 1) * 128, :])

        # ------------- Gram over 128 stiles (8 chunks) -------------
        bncP_in = dram.tile([PTOT], f32)
        bncP_out = dram.tile([PTOT], f32)

        with tc.tile_pool(name="s1ps", bufs=1, space="PSUM") as stg1ps:
            ptk_ps = stg1ps.tile([90, 175], f32)
            for c in range(NCH):
                nc.vector.tensor_tensor(
                    out=u2t_sb[c][:], in0=xt_sb[c][:, :, 86:176],
                    in1=g2e_sb[:, c * CST:(c + 1) * CST, :], op=OP.mult)
                for j in range(CST):
                    st = c * CST + j
                    nc.tensor.matmul(
                        ptk_ps[:], lhsT=u2t_sb[c][:, j, :],
                        rhs=xt_sb[c][:, j, 0:175],
                        start=(st == 0), stop=(st == NST - 1))

            # ------- stats: Sum x_V (DVE), Sum x^2 (Scalar) -------
            sumsV = const.tile([128, 1], f32)
            sqp = const.tile([128, 8], f32)
            sqs_sb = const.tile([128, 2], f32)
            nc.vector.reduce_sum(
                sumsV[:], x_sb[:, 1, :], axis=AX.X)
            for ct in range(2):
                for cc in range(4):
                    scr = osml.tile([128, 4096], bf16, tag="sqscr", bufs=1,
                                    name=f"sq{ct}{cc}")
                    nc.scalar.activation(
                        out=scr[:], in_=x_sb[:, ct, cc * 4096:(cc + 1) * 4096],
                        func=AF.Square,
                        accum_out=sqp[:, 4 * ct + cc:4 * ct + cc + 1])
                nc.vector.reduce_sum(
                    sqs_sb[:, ct:ct + 1], sqp[:, 4 * ct:4 * ct + 4], axis=AX.X)

            ptk_sb = small.tile([90, 86], f32, tag="ptksb", bufs=1)
            nc.scalar.copy(ptk_sb[:], ptk_ps[0:90, 0:86])
            ptk3_sb = small.tile([90, 86], f32, tag="ptk3sb", bufs=1)
            nc.scalar.copy(ptk3_sb[:], ptk_ps[0:90, 89:175])

        # ------------- rhs for M2 (independent of AR) -------------
        # rows 0..85 = gamma*x_V (ch 170..255), 86..88 = [ones, -gamma, beta]
        rhs_m2 = rhsp.tile([128, SH], bf16)
        nc.gpsimd.dma_start(out=rhs_m2[0:86, :], in_=x_sb[42:128, 1, :])
        nc.vector.tensor_tensor(
            out=rhs_m2[0:86, :], in0=rhs_m2[0:86, :], in1=gam_bc[0:86, :],
            op=OP.mult)
        nc.gpsimd.dma_start(out=rhs_m2[86:89, :], in_=gb1r_d.ap())

        # ------------- AllReduce within the batch pair -------------
        nc.gpsimd.dma_start(
            out=bncP_in[0:7740].rearrange("(p f) -> p f", f=86),
            in_=ptk_sb[:])
        nc.gpsimd.dma_start(
            out=bncP_in[7740:7998].rearrange("(p f) -> p f", f=86),
            in_=ptk3_sb[0:3, :])
        nc.gpsimd.dma_start(
            out=bncP_in[7998:8084].rearrange("(p f) -> p f", f=86),
            in_=ptk3_sb[89:90, :])
        nc.gpsimd.dma_start(
            out=bncP_in[8084:8170].rearrange("(p f) -> p f", f=1),
            in_=sumsV[42:128, :])
        nc.gpsimd.dma_start(
            out=bncP_in[8170:8426].rearrange("(t p) -> p t", p=128),
            in_=sqs_sb[:])
        nc.gpsimd.collective_compute(
            "AllReduce", OP.add,
            replica_groups=[[0, 1], [2, 3], [4, 5], [6, 7]],
            ins=[bncP_in[:].opt()], outs=[bncP_out[:].opt()])

        # ------------- DMA back -------------
        pt_back = const.tile([86, 86], f32)
        nc.sync.dma_start(
            out=pt_back[:],
            in_=bass.AP(tensor=bncP_out.tensor,
                        offset=bncP_out.offset + 3 * 86,
                        ap=[[86, 86], [1, 86]]))  # P^T rows 3..88 of block1
        tga = const.tile([86, 3], f32)
        nc.sync.dma_start(
            out=tga[:],
            in_=bass.AP(tensor=bncP_out.tensor, offset=bncP_out.offset,
                        ap=[[1, 86], [86, 3]]))
        tgk = const.tile([86, 3], f32)
        nc.sync.dma_start(
            out=tgk[:],
            in_=bass.AP(tensor=bncP_out.tensor,
                        offset=bncP_out.offset + 7740,
                        ap=[[1, 86], [86, 3]]))
        # stats cols: [p, {Sx,Sxx} x {A,K,V}]
        sAK = const.tile([86, 6], f32)
        sx_srcs = [(0, 89, [[90, 86], [1, 1]]),      # Sx_A = block1 col 89
                   (1, 7998, [[1, 86], [1, 1]]),     # Sx_K
                   (2, 8084, [[1, 86], [1, 1]])]     # Sx_V
        for g, koff, ap in sx_srcs:
            nc.sync.dma_start(
                out=sAK[:, g:g + 1],
                in_=bass.AP(tensor=bncP_out.tensor,
                            offset=bncP_out.offset + koff, ap=ap))
        for g, goff in ((0, 0), (1, 85), (2, 170)):
            nc.sync.dma_start(
                out=sAK[:, 3 + g:4 + g],
                in_=bass.AP(tensor=bncP_out.tensor,
                            offset=bncP_out.offset + 8170 + goff,
                            ap=[[1, 86], [1, 1]]))

        invS = 1.0 / float(S)

        # --- per-channel LayerNorm scalars ---
        mAK = small.tile([86, 3], f32, tag="mAK")
        nc.vector.tensor_scalar(
            out=mAK[:], in0=sAK[:, 0:3], scalar1=invS, scalar2=None,
            op0=OP.mult)
        vAK = small.tile([86, 3], f32, tag="vAK")
        nc.vector.tensor_scalar(
            out=vAK[:], in0=sAK[:, 3:6], scalar1=invS, scalar2=EPS,
            op0=OP.mult, op1=OP.add)
        msq = small.tile([86, 3], f32, tag="msq")
        nc.vector.tensor_mul(msq[:], mAK[:], mAK[:])
        nc.vector.tensor_sub(vAK[:], vAK[:], msq[:])
        nc.scalar.activation(out=vAK[:], in_=vAK[:], func=AF.Sqrt)
        rAK = small.tile([86, 3], f32, tag="rAK")
        nc.vector.reciprocal(rAK[:], vAK[:])
        invrV = small.tile([86, 1], f32, tag="invrV")
        nc.vector.reciprocal(invrV[:], rAK[:, 2:3])
        mvinv_bf = small.tile([86, 2], bf16, tag="mvinv")
        nc.vector.tensor_copy(mvinv_bf[:, 0:1], mAK[:, 2:3])
        nc.vector.tensor_copy(mvinv_bf[:, 1:2], invrV[:])
        rv_ext = small.tile([128, 1], f32, tag="rvext")
        nc.vector.memset(rv_ext[64:128, :], 1.0)
        nc.vector.tensor_copy(rv_ext[0:86, :], rAK[:, 2:3])

        tA, gA, hA = tga[:, 0:1], tga[:, 1:2], tga[:, 2:3]
        tK, gK, hK = tgk[:, 0:1], tgk[:, 1:2], tgk[:, 2:3]
        mA, mK = mAK[:, 0:1], mAK[:, 1:2]
        rA, rK = rAK[:, 0:1], rAK[:, 1:2]
        scG1 = sc_bc[0:86, 0:1]
        scG2 = sc_bc[0:86, 1:2]
        scGb = sc_bc[0:86, 2:3]
        scB1 = sc_bc[0:86, 3:4]
        scBb = sc_bc[0:86, 4:5]

        ntK = small.tile([86, 1], f32, tag="ntK")
        nc.vector.tensor_scalar_mul(ntK[:], tK, -1.0)
        nmK = small.tile([86, 1], f32, tag="nmK")
        nc.vector.tensor_scalar_mul(nmK[:], mK, -1.0)
        g2mK = small.tile([86, 1], f32, tag="g2mK")
        nc.vector.tensor_scalar(
            out=g2mK[:], in0=mK, scalar1=scG2, scalar2=None, op0=OP.mult)
        t3c = small.tile([86, 1], f32, tag="t3c")
        nc.vector.tensor_scalar(
            out=t3c[:], in0=mK, scalar1=scGb, scalar2=None, op0=OP.mult)
        nc.vector.tensor_sub(t3c[:], gK, t3c[:])
        nc.vector.tensor_mul(t3c[:], rK, t3c[:])
        t2c = small.tile([86, 1], f32, tag="t2c")
        nc.vector.tensor_scalar(
            out=t2c[:], in0=mA, scalar1=scGb, scalar2=None, op0=OP.mult)
        nc.vector.tensor_sub(t2c[:], gA, t2c[:])
        nc.vector.tensor_mul(t2c[:], rA, t2c[:])
        syA = small.tile([86, 1], f32, tag="syA")
        nc.vector.tensor_scalar(
            out=syA[:], in0=mA, scalar1=scG1, scalar2=None, op0=OP.mult)
        nc.vector.tensor_sub(syA[:], hA, syA[:])
        nc.vector.tensor_mul(syA[:], rA, syA[:])
        nc.vector.tensor_scalar(
            out=syA[:], in0=syA[:], scalar1=scB1, scalar2=None, op0=OP.add)
        syK = small.tile([86, 1], f32, tag="syK")
        nc.vector.tensor_scalar(
            out=syK[:], in0=mK, scalar1=scG1, scalar2=None, op0=OP.mult)
        nc.vector.tensor_sub(syK[:], hK, syK[:])
        nc.vector.tensor_mul(syK[:], rK, syK[:])
        nc.vector.tensor_scalar(
            out=syK[:], in0=syK[:], scalar1=scB1, scalar2=None, op0=OP.add)

        with tc.tile_pool(name="psG1", bufs=1, space="PSUM") as psG1, \
             tc.tile_pool(name="psG2", bufs=1, space="PSUM") as psG2, \
             tc.tile_pool(name="psLog", bufs=2, space="PSUM") as psLog:

            # rows (mA, tA, rA, term2) -> transpose -> DRAM -> one bcast DMA
            pack = small.tile([86, 4], f32, tag="pack")
            nc.vector.tensor_copy(pack[:, 0:1], mA)
            nc.vector.tensor_copy(pack[:, 1:2], tA)
            nc.vector.tensor_copy(pack[:, 2:3], rA)
            nc.vector.tensor_copy(pack[:, 3:4], t2c[:])
            packT_ps = psG1.tile([4, 86], f32, tag="pT")
            nc.tensor.transpose(packT_ps[:], pack[:], ident[0:86, 0:86])
            packT = small.tile([4, 86], f32, tag="packT")
            nc.scalar.copy(packT[:], packT_ps[:])
            rows_d = dram.tile([4, 86], f32, tag="rowsd")
            nc.gpsimd.dma_start(out=rows_d[:], in_=packT[:])
            bc4 = small.tile([86, 4, 86], f32, tag="bc4")
            nc.gpsimd.dma_start(
                out=bc4[:],
                in_=bass.AP(tensor=rows_d.tensor, offset=rows_d.offset,
                            ap=[[0, 86], [86, 4], [1, 86]]))

            # --- syy ---
            syy = small.tile([86, 97], f32, tag="syy")
            nc.vector.memset(syy[:, 86:96], 0.0)
            nc.vector.scalar_tensor_tensor(
                out=syy[:, 0:86], in0=bc4[:, 0, :], scalar=ntK[:],
                in1=pt_back[:], op0=OP.mult, op1=OP.add)
            nc.vector.scalar_tensor_tensor(
                out=syy[:, 0:86], in0=bc4[:, 1, :], scalar=nmK[:],
                in1=syy[:, 0:86], op0=OP.mult, op1=OP.add)
            nc.vector.scalar_tensor_tensor(
                out=syy[:, 0:86], in0=bc4[:, 0, :], scalar=g2mK[:],
                in1=syy[:, 0:86], op0=OP.mult, op1=OP.add)
            nc.vector.scalar_tensor_tensor(
                out=syy[:, 0:86], in0=bc4[:, 2, :], scalar=rK,
                in1=syy[:, 0:86], op0=OP.mult, op1=OP.mult)
            nc.vector.tensor_add(syy[:, 0:86], syy[:, 0:86], bc4[:, 3, :])
            nc.vector.tensor_scalar(
                out=syy[:, 0:86], in0=syy[:, 0:86], scalar1=t3c[:],
                scalar2=scBb, op0=OP.add, op1=OP.add)
            nc.vector.tensor_copy(syy[:, 96:97], syK[:])

            # --- logits + softmax (recip folded into att) ---
            u_ps = psG2.tile([97, C], f32, tag="uP")
            nc.tensor.matmul(u_ps[:], lhsT=syy[:], rhs=ekt_sb[:],
                             start=True, stop=True)
            u_ext = small.tile([128, C], f32, tag="uext")
            nc.vector.memset(u_ext[64:128, :], 0.0)
            nc.vector.scalar_tensor_tensor(
                out=u_ext[0:86, :], in0=bk_bc[0:86, :], scalar=syA[:],
                in1=u_ps[0:86, :], op0=OP.mult, op1=OP.add)
            nc.vector.tensor_scalar_mul(
                u_ext[96:97, :], bk_bc[96:97, :], float(S))
            nc.vector.tensor_add(u_ext[96:97, :], u_ext[96:97, :],
                                 u_ps[96:97, :])

            att_nrm = []
            recip2 = small.tile([128, 2], f32, tag="recip2")
            z2 = small.tile([128, 2], f32, tag="z2")
            for it in range(2):
                log_ps = psLog.tile([128, 512], f32, tag="lg", name=f"lg{it}")
                nc.tensor.matmul(
                    log_ps[:, 0:C], lhsT=eqt_sb[:, it * 128:(it + 1) * 128],
                    rhs=u_ext[0:97, :], start=True, stop=True)
                rmax = small.tile([128, 1], f32, tag="rmax", name=f"rm{it}")
                nc.vector.reduce_max(rmax[:], log_ps[:, 0:C], axis=AX.X)
                nbias = small.tile([128, 1], f32, tag="nbias", name=f"nb{it}")
                nc.vector.tensor_scalar_mul(nbias[:], rmax[:], -SCALE)
                a_sb = small.tile([128, C], bf16, tag=f"attsb{it}",
                                  name=f"att{it}")
                nc.scalar.activation(
                    out=a_sb[:], in_=log_ps[:, 0:C], func=AF.Exp,
                    bias=nbias[:], scale=SCALE, accum_out=z2[:, it:it + 1])
                nc.vector.reciprocal(recip2[:, it:it + 1], z2[:, it:it + 1])
                a_nr = small.tile([128, C], bf16, tag=f"anrm{it}",
                                  name=f"an{it}")
                nc.scalar.activation(
                    out=a_nr[:], in_=a_sb[:], func=AF.Copy,
                    scale=recip2[:, it:it + 1])
                att_nrm.append(a_nr)

        # --- NT: lhs_m2 [89 rows, 256 q-ch] ---
        psNtc = ctx.enter_context(tc.tile_pool(name="psNtc", bufs=1,
                                               space="PSUM"))
        psAt = ctx.enter_context(tc.tile_pool(name="psAt", bufs=2,
                                              space="PSUM"))
        psO = ctx.enter_context(tc.tile_pool(name="psO", bufs=2,
                                             space="PSUM"))

        ntc_ps = psNtc.tile([128, C], f32, tag="ntc")
        for jt in range(2):
            at_ps = psAt.tile([128, C], bf16, tag="atp", name=f"atp{jt}")
            for it in range(2):
                nc.tensor.transpose(
                    at_ps[:, it * 128:(it + 1) * 128],
                    att_nrm[it][:, jt * 128:(jt + 1) * 128],
                    ident_bf[:])
            at_bf = small.tile([128, C], bf16, tag=f"atbf{jt}", name=f"atb{jt}")
            nc.scalar.copy(at_bf[:], at_ps[:])
            nc.tensor.matmul(
                ntc_ps[0:87, :], lhsT=w0_sb[:, jt, :], rhs=at_bf[:],
                start=(jt == 0), stop=(jt == 1))

        lhs_m2 = small.tile([128, C], bf16, tag="lhsm2")
        rv = rv_ext
        nc.scalar.activation(
            out=lhs_m2[0:64, :], in_=ntc_ps[0:64, :], func=AF.Copy,
            scale=rv[0:64, :])
        nc.scalar.activation(
            out=lhs_m2[64:87, :], in_=ntc_ps[64:87, :], func=AF.Copy,
            scale=rv[64:87, :])
        nc.tensor.matmul(
            ntc_ps[64:66, :], lhsT=mvinv_bf[:],
            rhs=lhs_m2[0:86, :], start=True, stop=True)
        c12_sb = small.tile([128, C], bf16, tag="c12sb")
        nc.scalar.copy(c12_sb[64:66, :], ntc_ps[64:66, :])
        nc.gpsimd.dma_start(out=lhs_m2[87:89, :], in_=c12_sb[64:66, :])

        # --- M2: out = x + att_nrm @ v ---
        nadd = 0
        for it in range(2):
            for ch in range(8):
                ostg = osml.tile([128, 2048], bf16, tag="ostg",
                                 name=f"o{it}{ch}")
                for j in range(2):
                    off = ch * 2048 + j * 1024
                    o_ps = psO.tile([128, 1024], f32, tag="oP",
                                    name=f"op{it}{ch}{j}")
                    for h in range(2):
                        nc.tensor.matmul(
                            o_ps[:, h * 512:(h + 1) * 512],
                            lhsT=lhs_m2[0:89, it * 128:(it + 1) * 128],
                            rhs=rhs_m2[0:89, off + h * 512:off + (h + 1) * 512],
                            start=True, stop=True)
                    nadd += 1
                    nc.vector.tensor_tensor(
                        out=ostg[:, j * 1024:(j + 1) * 1024], in0=o_ps[:],
                        in1=x_sb[:, it, off:off + 1024], op=OP.add)
                nc.sync.dma_start(
                    out=out_d[it * 128:(it + 1) * 128,
                              ch * 2048:(ch + 1) * 2048],
                    in_=ostg[:])

    nc.compile()
    return nc


def _host_prep(x, gamma, beta, w_qkv, b_qkv):
    xf = np.asarray(x, np.float32).reshape(B, C, S)
    gam = np.asarray(gamma, np.float32).reshape(-1)
    bet = np.asarray(beta, np.float32).reshape(-1)
    w_qkv = np.asarray(w_qkv, np.float32)
    b_qkv = np.asarray(b_qkv, np.float32)
    w_q, w_k, w_v = w_qkv[:C], w_qkv[C:2 * C], w_qkv[2 * C:]
    b_q, b_k, b_v = b_qkv[:C], b_qkv[C:2 * C], b_qkv[2 * C:]

    ii = np.arange(C)
    eqt = np.zeros((97, C), np.float32)
    eqt[ii // 3, ii] = w_q
    eqt[96] = b_q
    ekt = np.zeros((86, C), np.float32)
    ekt[(C + ii) // 3 - 85, ii] = w_k
    w0 = np.zeros((C, 87), np.float32)
    w0[ii, (2 * C + ii) // 3 - 170] = w_v
    w0[:, 86] = b_v
    w0 = w0.astype(_BF)

    sc = np.zeros((1, 8), np.float32)
    sc[0, :5] = [gam.sum(), (gam * gam).sum(), (gam * bet).sum(),
                 bet.sum(), (bet * bet).sum()]

    in_maps = []
    for r in range(NCORES):
        b, half = r // 2, r % 2
        sl = slice(half * SH, (half + 1) * SH)
        gl = gam[sl]
        bl = bet[sl]
        gb1r = np.stack([np.ones(SH, np.float32), -gl, bl], 0)

        xl = xf[b][:, sl]                       # [256, 16384]
        xtl = np.ascontiguousarray(xl.T)        # [16384, 256]
        blocks = np.empty((SH, 176), np.float32)
        blocks[:, 0:86] = xtl[:, 0:86]
        blocks[:, 86] = gl * gl
        blocks[:, 87] = gl * bl
        blocks[:, 88] = gl
        blocks[:, 89:175] = xtl[:, 85:171]
        blocks[:, 175] = 1.0
        xt = blocks.reshape(NST, 128, 176).transpose(1, 0, 2)
        xt = np.ascontiguousarray(xt.reshape(128, NST * 176)).astype(_BF)

        g2c = (gl * gl).reshape(NST, 128).T     # [128, NST]
        g2e = np.empty((128, NST, 90), np.float32)
        g2e[:, :, 0:3] = 1.0
        g2e[:, :, 3:89] = g2c[:, :, None]
        g2e[:, :, 89] = 1.0
        g2e = np.ascontiguousarray(g2e.reshape(128, NST * 90)).astype(_BF)

        in_maps.append({
            "xs": np.ascontiguousarray(xl).astype(_BF),
            "xt": xt,
            "g2e": g2e,
            "gb1r": gb1r.astype(_BF),
            "eqt": eqt,
            "ekt": ekt,
            "w0": w0,
            "bk": b_k.reshape(1, C).copy(),
            "sc": sc,
        })
    return in_maps


def kernel(x, gamma, beta, w_qkv, b_qkv):
    from concourse.bass_utils import run_bass_kernel_spmd

    if "nc" not in _cache:
        _cache["nc"] = _build_program()
    nc = _cache["nc"]

    in_maps = _host_prep(x, gamma, beta, w_qkv, b_qkv)
    res = run_bass_kernel_spmd(nc, in_maps, core_ids=list(range(NCORES)))
    out = np.empty((B, C, S), np.float32)
    for r in range(NCORES):
        b, half = r // 2, r % 2
        out[b][:, half * SH:(half + 1) * SH] = np.asarray(
            res.results[r]["out"]).astype(np.float32)
    return out.reshape(np.asarray(x).shape)


if __name__ == "__main__":
    rng = np.random.default_rng(0)
    inputs = {
        "x": rng.standard_normal((B, C, 32, 32, 32)).astype(np.float32),
        "gamma": (1 + 0.1 * rng.standard_normal((32, 32, 32))).astype(np.float32),
        "beta": (0.1 * rng.standard_normal((32, 32, 32))).astype(np.float32),
        "w_qkv": (0.5 * rng.standard_normal(3 * C)).astype(np.float32),
        "b_qkv": (0.05 * rng.standard_normal(3 * C)).astype(np.float32),
    }
    o = kernel(**inputs)
    print("out", o.shape, o.dtype, float(np.abs(o).mean()))


# revision 23
# speedup vs baseline: 1.5632x; 1.0223x over previous
"""Channel-self-attention (LayerNorm + grouped-1x1-qkv + channel softmax attn
+ residual) on 8 TRN2 NeuronCores.

Strategy (v3): pair-sharding — 2 cores per batch, each core owns one
spatial half (16384 of 32768). One ~34 KB 2-rank Mesh AllReduce per core.

Per core:
 - x half-shard [256, 16384] bf16 resident in SBUF (channel-major)
 - host also sends x TRANSPOSED (spatial-major, bf16) packed per 128-row
   stile as [x_A(86) | g2 gb g (3) | x_K(86)] so the Gram matmul needs NO
   on-chip transposes:
     lhsT = [g2 gb g | g2*x_K]   (g2*x_K built by 8 bulk chunk DVE mults
                                  against a host-replicated gamma^2 tile)
     rhs  = the raw packed stile
     out  = [89,175]: rows 0..2 x cols 0..85 = tgh_A, rows 3..88 = P^T,
            rows 0..2 x cols 89..174 = tgh_K
 - stats: Sum x via DVE reduce, Sum x^2 via Scalar Square+accum (idle
   engine), replacing bn_stats
 - ONE AllReduce (Gram + tgh + stats, 33.7 KB) within the batch pair
 - logits from the Gram expansion of the LayerNorm algebra; softmax
   normalization folded into att before the transpose, so the epilogue is
   a plain  out = x + att_nrm @ v  residual add (split DVE/GpSimd), with
   bf16 output upcast on host
"""
import sys

sys.path.insert(0, "/opt/trn_rl_repo")

import numpy as np
import ml_dtypes

B, C = 4, 256
S = 32 * 32 * 32          # 32768 global spatial
NCORES = 8
SH = S // 2               # 16384 per-core spatial half
NST = SH // 128           # 128 stiles
NCH = 8                   # Gram stream chunks
CST = NST // NCH          # 16 stiles per chunk
EPS = 1e-5
SCALE = float(S) ** -0.5

_BF = ml_dtypes.bfloat16

_cache = {}


def _build_program():
    from contextlib import ExitStack
    import concourse.bass as bass
    import concourse.bacc as bacc
    import concourse.tile as tile
    from concourse import mybir, masks

    f32 = mybir.dt.float32
    bf16 = mybir.dt.bfloat16
    AF = mybir.ActivationFunctionType
    OP = mybir.AluOpType
    AX = mybir.AxisListType

    nc = bacc.Bacc(
        "TRN2",
        target_bir_lowering=False,
        debug=False,
        enable_asserts=False,
        num_devices=NCORES,
    )

    # ---------------- DRAM I/O ----------------
    xs_d = nc.dram_tensor("xs", [C, SH], bf16, kind="ExternalInput")
    xt_d = nc.dram_tensor("xt", [128, NST * 176], bf16, kind="ExternalInput")
    g2e_d = nc.dram_tensor("g2e", [128, NST * 90], bf16, kind="ExternalInput")
    gb1r_d = nc.dram_tensor("gb1r", [3, SH], bf16, kind="ExternalInput")
    eqt_d = nc.dram_tensor("eqt", [97, C], f32, kind="ExternalInput")
    ekt_d = nc.dram_tensor("ekt", [86, C], f32, kind="ExternalInput")
    w0_d = nc.dram_tensor("w0", [2 * 128, 87], bf16, kind="ExternalInput")
    bk_d = nc.dram_tensor("bk", [1, C], f32, kind="ExternalInput")
    sc_d = nc.dram_tensor("sc", [1, 8], f32, kind="ExternalInput")
    out_d = nc.dram_tensor("out", [C, SH], bf16, kind="ExternalOutput")

    # AR payload layout (f32 words). M = [90,175] Gram PSUM; lhsT col 89
    # is ones so M row 89 = per-channel Sum x for A (cols 0:86) / K (89:175).
    #   [0 : 7740)        M[0:90, 0:86] row-major (tgh_A 0:3, P^T 3:89, SxA 89)
    #   [7740 : 7998)     M[0:3, 89:175] row-major (tgh_K)
    #   [7998 : 8084)     M[89, 89:175]  (Sum x_K)
    #   [8084 : 8170)     Sum x_V (DVE reduce over partitions 42:128 of ct1)
    #   [8170 : 8426)     Sum x^2 per channel
    PTOT = 8426

    with tile.TileContext(nc) as tc, ExitStack() as ctx:
        const = ctx.enter_context(tc.tile_pool(name="const", bufs=1))
        xpool = ctx.enter_context(tc.tile_pool(name="xpool", bufs=1))
        xtp = ctx.enter_context(tc.tile_pool(name="xtp", bufs=2))
        utp = ctx.enter_context(tc.tile_pool(name="utp", bufs=2))
        g2p = ctx.enter_context(tc.tile_pool(name="g2p", bufs=1))
        rhsp = ctx.enter_context(tc.tile_pool(name="rhsp", bufs=1))
        osml = ctx.enter_context(tc.tile_pool(name="osml", bufs=2))
        small = ctx.enter_context(tc.tile_pool(name="small", bufs=2))
        dram = ctx.enter_context(tc.tile_pool(name="dram", bufs=1, space="DRAM"))

        # ------------- constants / inputs to SBUF -------------
        ident = const.tile([128, 128], f32)
        masks.make_identity(nc, ident[:])
        ident_bf = const.tile([128, 128], bf16)
        masks.make_identity(nc, ident_bf[:])
        eqt_sb = const.tile([97, C], f32)
        nc.sync.dma_start(out=eqt_sb[:], in_=eqt_d.ap())
        ekt_sb = const.tile([86, C], f32)
        nc.sync.dma_start(out=ekt_sb[:], in_=ekt_d.ap())
        w0_sb = const.tile([128, 2, 87], bf16)
        for jt in range(2):
            nc.sync.dma_start(out=w0_sb[:, jt, :], in_=w0_d[jt * 128:(jt + 1) * 128, :])

        def dram_bcast(dst, src_d, nparts, nfree, off=0):
            nc.gpsimd.dma_start(
                out=dst,
                in_=bass.AP(tensor=src_d, offset=off,
                            ap=[[0, nparts], [1, nfree]]))

        bk_bc = const.tile([128, C], f32)
        dram_bcast(bk_bc[:], bk_d, 128, C)
        sc_bc = const.tile([128, 8], f32)
        dram_bcast(sc_bc[:], sc_d, 128, 8)
        gam_bc = const.tile([128, SH], bf16)
        dram_bcast(gam_bc[:], gb1r_d, 128, SH, off=SH)
        nc.vector.tensor_scalar_mul(gam_bc[:], gam_bc[:], -1.0)

        # Gram streams: xt chunks on gpsimd queue; g2e resident via scalar
        g2e_sb = g2p.tile([128, NST, 90], bf16)
        nc.scalar.dma_start(out=g2e_sb[:], in_=g2e_d.ap())
        xt_sb, u2t_sb = [], []
        for c in range(NCH):
            t = xtp.tile([128, CST, 176], bf16, tag="xt", name=f"xt{c}")
            nc.gpsimd.dma_start(
                out=t[:],
                in_=xt_d[:, c * CST * 176:(c + 1) * CST * 176])
            xt_sb.append(t)
            u = utp.tile([128, CST, 90], bf16, tag="u2t", name=f"u2{c}")
            u2t_sb.append(u)

        # x resident bf16 [128, 2, 16384]
        x_sb = xpool.tile([128, 2, SH], bf16)
        for ct in range(2):
            nc.sync.dma_start(
                out=x_sb[:, ct, :],
                in_=xs_d[ct * 128:(ct + 1) * 128, :])

        # ------------- Gram over 128 stiles (8 chunks) -------------
        bncP_in = dram.tile([PTOT], f32)
        bncP_out = dram.tile([PTOT], f32)

        with tc.tile_pool(name="s1ps", bufs=1, space="PSUM") as stg1ps:
            ptk_ps = stg1ps.tile([90, 175], f32)
            with tc.high_priority():
                for c in range(NCH):
                    nc.vector.tensor_tensor(
                        out=u2t_sb[c][:], in0=xt_sb[c][:, :, 86:176],
                        in1=g2e_sb[:, c * CST:(c + 1) * CST, :], op=OP.mult)
                    for j in range(CST):
                        st = c * CST + j
                        nc.tensor.matmul(
                            ptk_ps[:], lhsT=u2t_sb[c][:, j, :],
                            rhs=xt_sb[c][:, j, 0:175],
                            start=(st == 0), stop=(st == NST - 1))

            # ------- stats: Sum x_V (DVE), Sum x^2 (Scalar) -------
            sumsV = const.tile([128, 1], f32)
            sqp = const.tile([128, 8], f32)
            sqs_sb = const.tile([128, 2], f32)
            nc.vector.reduce_sum(
                sumsV[:], x_sb[:, 1, :], axis=AX.X)
            for ct in range(2):
                for cc in range(4):
                    scr = osml.tile([128, 4096], bf16, tag="sqscr", bufs=1,
                                    name=f"sq{ct}{cc}")
                    nc.scalar.activation(
                        out=scr[:], in_=x_sb[:, ct, cc * 4096:(cc + 1) * 4096],
                        func=AF.Square,
                        accum_out=sqp[:, 4 * ct + cc:4 * ct + cc + 1])
                nc.vector.reduce_sum(
                    sqs_sb[:, ct:ct + 1], sqp[:, 4 * ct:4 * ct + 4], axis=AX.X)

            ptk_sb = small.tile([90, 86], f32, tag="ptksb", bufs=1)
            nc.scalar.copy(ptk_sb[:], ptk_ps[0:90, 0:86])
            ptk3_sb = small.tile([90, 86], f32, tag="ptk3sb", bufs=1)
            nc.scalar.copy(ptk3_sb[:], ptk_ps[0:90, 89:175])

        # ------------- rhs for M2 (independent of AR) -------------
        # rows 0..85 = gamma*x_V (ch 170..255), 86..88 = [ones, -gamma, beta]
        rhs_m2 = rhsp.tile([128, SH], bf16)
        nc.gpsimd.dma_start(out=rhs_m2[0:86, :], in_=x_sb[42:128, 1, :])
        nc.vector.tensor_tensor(
            out=rhs_m2[0:86, :], in0=rhs_m2[0:86, :], in1=gam_bc[0:86, :],
            op=OP.mult)
        nc.gpsimd.dma_start(out=rhs_m2[86:89, :], in_=gb1r_d.ap())

        # ------------- AllReduce within the batch pair -------------
        nc.gpsimd.dma_start(
            out=bncP_in[0:7740].rearrange("(p f) -> p f", f=86),
            in_=ptk_sb[:])
        nc.gpsimd.dma_start(
            out=bncP_in[7740:7998].rearrange("(p f) -> p f", f=86),
            in_=ptk3_sb[0:3, :])
        nc.gpsimd.dma_start(
            out=bncP_in[7998:8084].rearrange("(p f) -> p f", f=86),
            in_=ptk3_sb[89:90, :])
        nc.gpsimd.dma_start(
            out=bncP_in[8084:8170].rearrange("(p f) -> p f", f=1),
            in_=sumsV[42:128, :])
        nc.gpsimd.dma_start(
            out=bncP_in[8170:8426].rearrange("(t p) -> p t", p=128),
            in_=sqs_sb[:])
        nc.gpsimd.collective_compute(
            "AllReduce", OP.add,
            replica_groups=[[0, 1], [2, 3], [4, 5], [6, 7]],
            ins=[bncP_in[:].opt()], outs=[bncP_out[:].opt()])

        # ------------- DMA back -------------
        pt_back = const.tile([86, 86], f32)
        nc.sync.dma_start(
            out=pt_back[:],
            in_=bass.AP(tensor=bncP_out.tensor,
                        offset=bncP_out.offset + 3 * 86,
                        ap=[[86, 86], [1, 86]]))  # P^T rows 3..88 of block1
        tga = const.tile([86, 3], f32)
        nc.sync.dma_start(
            out=tga[:],
            in_=bass.AP(tensor=bncP_out.tensor, offset=bncP_out.offset,
                        ap=[[1, 86], [86, 3]]))
        tgk = const.tile([86, 3], f32)
        nc.sync.dma_start(
            out=tgk[:],
            in_=bass.AP(tensor=bncP_out.tensor,
                        offset=bncP_out.offset + 7740,
                        ap=[[1, 86], [86, 3]]))
        # stats cols: [p, {Sx,Sxx} x {A,K,V}]
        sAK = const.tile([86, 6], f32)
        sx_srcs = [(0, 89, [[90, 86], [1, 1]]),      # Sx_A = block1 col 89
                   (1, 7998, [[1, 86], [1, 1]]),     # Sx_K
                   (2, 8084, [[1, 86], [1, 1]])]     # Sx_V
        for g, koff, ap in sx_srcs:
            nc.sync.dma_start(
                out=sAK[:, g:g + 1],
                in_=bass.AP(tensor=bncP_out.tensor,
                            offset=bncP_out.offset + koff, ap=ap))
        for g, goff in ((0, 0), (1, 85), (2, 170)):
            nc.sync.dma_start(
                out=sAK[:, 3 + g:4 + g],
                in_=bass.AP(tensor=bncP_out.tensor,
                            offset=bncP_out.offset + 8170 + goff,
                            ap=[[1, 86], [1, 1]]))

        invS = 1.0 / float(S)

        # --- per-channel LayerNorm scalars ---
        mAK = small.tile([86, 3], f32, tag="mAK")
        nc.vector.tensor_scalar(
            out=mAK[:], in0=sAK[:, 0:3], scalar1=invS, scalar2=None,
            op0=OP.mult)
        vAK = small.tile([86, 3], f32, tag="vAK")
        nc.vector.tensor_scalar(
            out=vAK[:], in0=sAK[:, 3:6], scalar1=invS, scalar2=EPS,
            op0=OP.mult, op1=OP.add)
        msq = small.tile([86, 3], f32, tag="msq")
        nc.vector.tensor_mul(msq[:], mAK[:], mAK[:])
        nc.vector.tensor_sub(vAK[:], vAK[:], msq[:])
        nc.scalar.activation(out=vAK[:], in_=vAK[:], func=AF.Sqrt)
        rAK = small.tile([86, 3], f32, tag="rAK")
        nc.vector.reciprocal(rAK[:], vAK[:])
        invrV = small.tile([86, 1], f32, tag="invrV")
        nc.vector.reciprocal(invrV[:], rAK[:, 2:3])
        mvinv_bf = small.tile([86, 2], bf16, tag="mvinv")
        nc.vector.tensor_copy(mvinv_bf[:, 0:1], mAK[:, 2:3])
        nc.vector.tensor_copy(mvinv_bf[:, 1:2], invrV[:])
        rv_ext = small.tile([128, 1], f32, tag="rvext")
        nc.vector.memset(rv_ext[64:128, :], 1.0)
        nc.vector.tensor_copy(rv_ext[0:86, :], rAK[:, 2:3])

        tA, gA, hA = tga[:, 0:1], tga[:, 1:2], tga[:, 2:3]
        tK, gK, hK = tgk[:, 0:1], tgk[:, 1:2], tgk[:, 2:3]
        mA, mK = mAK[:, 0:1], mAK[:, 1:2]
        rA, rK = rAK[:, 0:1], rAK[:, 1:2]
        scG1 = sc_bc[0:86, 0:1]
        scG2 = sc_bc[0:86, 1:2]
        scGb = sc_bc[0:86, 2:3]
        scB1 = sc_bc[0:86, 3:4]
        scBb = sc_bc[0:86, 4:5]

        ntK = small.tile([86, 1], f32, tag="ntK")
        nc.vector.tensor_scalar_mul(ntK[:], tK, -1.0)
        nmK = small.tile([86, 1], f32, tag="nmK")
        nc.vector.tensor_scalar_mul(nmK[:], mK, -1.0)
        g2mK = small.tile([86, 1], f32, tag="g2mK")
        nc.vector.tensor_scalar(
            out=g2mK[:], in0=mK, scalar1=scG2, scalar2=None, op0=OP.mult)
        t3c = small.tile([86, 1], f32, tag="t3c")
        nc.vector.tensor_scalar(
            out=t3c[:], in0=mK, scalar1=scGb, scalar2=None, op0=OP.mult)
        nc.vector.tensor_sub(t3c[:], gK, t3c[:])
        nc.vector.tensor_mul(t3c[:], rK, t3c[:])
        t2c = small.tile([86, 1], f32, tag="t2c")
        nc.vector.tensor_scalar(
            out=t2c[:], in0=mA, scalar1=scGb, scalar2=None, op0=OP.mult)
        nc.vector.tensor_sub(t2c[:], gA, t2c[:])
        nc.vector.tensor_mul(t2c[:], rA, t2c[:])
        syA = small.tile([86, 1], f32, tag="syA")
        nc.vector.tensor_scalar(
            out=syA[:], in0=mA, scalar1=scG1, scalar2=None, op0=OP.mult)
        nc.vector.tensor_sub(syA[:], hA, syA[:])
        nc.vector.tensor_mul(syA[:], rA, syA[:])
        nc.vector.tensor_scalar(
            out=syA[:], in0=syA[:], scalar1=scB1, scalar2=None, op0=OP.add)
        syK = small.tile([86, 1], f32, tag="syK")
        nc.vector.tensor_scalar(
            out=syK[:], in0=mK, scalar1=scG1, scalar2=None, op0=OP.mult)
        nc.vector.tensor_sub(syK[:], hK, syK[:])
        nc.vector.tensor_mul(syK[:], rK, syK[:])
        nc.vector.tensor_scalar(
            out=syK[:], in0=syK[:], scalar1=scB1, scalar2=None, op0=OP.add)

        with tc.tile_pool(name="psG1", bufs=1, space="PSUM") as psG1, \
             tc.tile_pool(name="psG2", bufs=1, space="PSUM") as psG2, \
             tc.tile_pool(name="psLog", bufs=2, space="PSUM") as psLog:

            # rows (mA, tA, rA, term2) -> transpose -> DRAM -> one bcast DMA
            pack = small.tile([86, 4], f32, tag="pack")
            nc.vector.tensor_copy(pack[:, 0:1], mA)
            nc.vector.tensor_copy(pack[:, 1:2], tA)
            nc.vector.tensor_copy(pack[:, 2:3], rA)
            nc.vector.tensor_copy(pack[:, 3:4], t2c[:])
            packT_ps = psG1.tile([4, 86], f32, tag="pT")
            nc.tensor.transpose(packT_ps[:], pack[:], ident[0:86, 0:86])
            packT = small.tile([4, 86], f32, tag="packT")
            nc.scalar.copy(packT[:], packT_ps[:])
            rows_d = dram.tile([4, 86], f32, tag="rowsd")
            nc.gpsimd.dma_start(out=rows_d[:], in_=packT[:])
            bc4 = small.tile([86, 4, 86], f32, tag="bc4")
            nc.gpsimd.dma_start(
                out=bc4[:],
                in_=bass.AP(tensor=rows_d.tensor, offset=rows_d.offset,
                            ap=[[0, 86], [86, 4], [1, 86]]))

            # --- syy ---
            syy = small.tile([86, 97], f32, tag="syy")
            nc.vector.memset(syy[:, 86:96], 0.0)
            nc.vector.scalar_tensor_tensor(
                out=syy[:, 0:86], in0=bc4[:, 0, :], scalar=ntK[:],
                in1=pt_back[:], op0=OP.mult, op1=OP.add)
            nc.vector.scalar_tensor_tensor(
                out=syy[:, 0:86], in0=bc4[:, 1, :], scalar=nmK[:],
                in1=syy[:, 0:86], op0=OP.mult, op1=OP.add)
            nc.vector.scalar_tensor_tensor(
                out=syy[:, 0:86], in0=bc4[:, 0, :], scalar=g2mK[:],
                in1=syy[:, 0:86], op0=OP.mult, op1=OP.add)
            nc.vector.scalar_tensor_tensor(
                out=syy[:, 0:86], in0=bc4[:, 2, :], scalar=rK,
                in1=syy[:, 0:86], op0=OP.mult, op1=OP.mult)
            nc.vector.tensor_add(syy[:, 0:86], syy[:, 0:86], bc4[:, 3, :])
            nc.vector.tensor_scalar(
                out=syy[:, 0:86], in0=syy[:, 0:86], scalar1=t3c[:],
                scalar2=scBb, op0=OP.add, op1=OP.add)
            nc.vector.tensor_copy(syy[:, 96:97], syK[:])

            # --- logits + softmax (recip folded into att) ---
            u_ps = psG2.tile([97, C], f32, tag="uP")
            nc.tensor.matmul(u_ps[:], lhsT=syy[:], rhs=ekt_sb[:],
                             start=True, stop=True)
            u_ext = small.tile([128, C], f32, tag="uext")
            nc.vector.memset(u_ext[64:128, :], 0.0)
            nc.vector.scalar_tensor_tensor(
                out=u_ext[0:86, :], in0=bk_bc[0:86, :], scalar=syA[:],
                in1=u_ps[0:86, :], op0=OP.mult, op1=OP.add)
            nc.vector.tensor_scalar_mul(
                u_ext[96:97, :], bk_bc[96:97, :], float(S))
            nc.vector.tensor_add(u_ext[96:97, :], u_ext[96:97, :],
                                 u_ps[96:97, :])

            att_nrm = []
            recip2 = small.tile([128, 2], f32, tag="recip2")
            z2 = small.tile([128, 2], f32, tag="z2")
            for it in range(2):
                log_ps = psLog.tile([128, 512], f32, tag="lg", name=f"lg{it}")
                nc.tensor.matmul(
                    log_ps[:, 0:C], lhsT=eqt_sb[:, it * 128:(it + 1) * 128],
                    rhs=u_ext[0:97, :], start=True, stop=True)
                rmax = small.tile([128, 1], f32, tag="rmax", name=f"rm{it}")
                nc.vector.reduce_max(rmax[:], log_ps[:, 0:C], axis=AX.X)
                nbias = small.tile([128, 1], f32, tag="nbias", name=f"nb{it}")
                nc.vector.tensor_scalar_mul(nbias[:], rmax[:], -SCALE)
                a_sb = small.tile([128, C], bf16, tag=f"attsb{it}",
                                  name=f"att{it}")
                nc.scalar.activation(
                    out=a_sb[:], in_=log_ps[:, 0:C], func=AF.Exp,
                    bias=nbias[:], scale=SCALE, accum_out=z2[:, it:it + 1])
                nc.vector.reciprocal(recip2[:, it:it + 1], z2[:, it:it + 1])
                a_nr = small.tile([128, C], bf16, tag=f"anrm{it}",
                                  name=f"an{it}")
                nc.scalar.activation(
                    out=a_nr[:], in_=a_sb[:], func=AF.Copy,
                    scale=recip2[:, it:it + 1])
                att_nrm.append(a_nr)

        # --- NT: lhs_m2 [89 rows, 256 q-ch] ---
        psNtc = ctx.enter_context(tc.tile_pool(name="psNtc", bufs=1,
                                               space="PSUM"))
        psAt = ctx.enter_context(tc.tile_pool(name="psAt", bufs=2,
                                              space="PSUM"))
        psO = ctx.enter_context(tc.tile_pool(name="psO", bufs=2,
                                             space="PSUM"))

        ntc_ps = psNtc.tile([128, C], f32, tag="ntc")
        for jt in range(2):
            at_ps = psAt.tile([128, C], bf16, tag="atp", name=f"atp{jt}")
            for it in range(2):
                nc.tensor.transpose(
                    at_ps[:, it * 128:(it + 1) * 128],
                    att_nrm[it][:, jt * 128:(jt + 1) * 128],
                    ident_bf[:])
            at_bf = small.tile([128, C], bf16, tag=f"atbf{jt}", name=f"atb{jt}")
            nc.scalar.copy(at_bf[:], at_ps[:])
            nc.tensor.matmul(
                ntc_ps[0:87, :], lhsT=w0_sb[:, jt, :], rhs=at_bf[:],
                start=(jt == 0), stop=(jt == 1))

        lhs_m2 = small.tile([128, C], bf16, tag="lhsm2")
        rv = rv_ext
        nc.scalar.activation(
            out=lhs_m2[0:64, :], in_=ntc_ps[0:64, :], func=AF.Copy,
            scale=rv[0:64, :])
        nc.scalar.activation(
            out=lhs_m2[64:87, :], in_=ntc_ps[64:87, :], func=AF.Copy,
            scale=rv[64:87, :])
        nc.tensor.matmul(
            ntc_ps[64:66, :], lhsT=mvinv_bf[:],
            rhs=lhs_m2[0:86, :], start=True, stop=True)
        c12_sb = small.tile([128, C], bf16, tag="c12sb")
        nc.scalar.copy(c12_sb[64:66, :], ntc_ps[64:66, :])
        nc.gpsimd.dma_start(out=lhs_m2[87:89, :], in_=c12_sb[64:66, :])

        # --- M2: out = x + att_nrm @ v ---
        nadd = 0
        for it in range(2):
            for ch in range(8):
                ostg = osml.tile([128, 2048], bf16, tag="ostg",
                                 name=f"o{it}{ch}")
                for j in range(2):
                    off = ch * 2048 + j * 1024
                    o_ps = psO.tile([128, 1024], f32, tag="oP",
                                    name=f"op{it}{ch}{j}")
                    for h in range(2):
                        nc.tensor.matmul(
                            o_ps[:, h * 512:(h + 1) * 512],
                            lhsT=lhs_m2[0:89, it * 128:(it + 1) * 128],
                            rhs=rhs_m2[0:89, off + h * 512:off + (h + 1) * 512],
                            start=True, stop=True)
                    nadd += 1
                    nc.vector.tensor_tensor(
                        out=ostg[:, j * 1024:(j + 1) * 1024], in0=o_ps[:],
                        in1=x_sb[:, it, off:off + 1024], op=OP.add)
                nc.sync.dma_start(
                    out=out_d[it * 128:(it + 1) * 128,
                              ch * 2048:(ch + 1) * 2048],
                    in_=ostg[:])

    nc.compile()
    return nc


def _host_prep(x, gamma, beta, w_qkv, b_qkv):
    xf = np.asarray(x, np.float32).reshape(B, C, S)
    gam = np.asarray(gamma, np.float32).reshape(-1)
    bet = np.asarray(beta, np.float32).reshape(-1)
    w_qkv = np.asarray(w_qkv, np.float32)
    b_qkv = np.asarray(b_qkv, np.float32)
    w_q, w_k, w_v = w_qkv[:C], w_qkv[C:2 * C], w_qkv[2 * C:]
    b_q, b_k, b_v = b_qkv[:C], b_qkv[C:2 * C], b_qkv[2 * C:]

    ii = np.arange(C)
    eqt = np.zeros((97, C), np.float32)
    eqt[ii // 3, ii] = w_q
    eqt[96] = b_q
    ekt = np.zeros((86, C), np.float32)
    ekt[(C + ii) // 3 - 85, ii] = w_k
    w0 = np.zeros((C, 87), np.float32)
    w0[ii, (2 * C + ii) // 3 - 170] = w_v
    w0[:, 86] = b_v
    w0 = w0.astype(_BF)

    sc = np.zeros((1, 8), np.float32)
    sc[0, :5] = [gam.sum(), (gam * gam).sum(), (gam * bet).sum(),
                 bet.sum(), (bet * bet).sum()]

    in_maps = []
    for r in range(NCORES):
        b, half = r // 2, r % 2
        sl = slice(half * SH, (half + 1) * SH)
        gl = gam[sl]
        bl = bet[sl]
        gb1r = np.stack([np.ones(SH, np.float32), -gl, bl], 0)

        xl = xf[b][:, sl]                       # [256, 16384]
        xtl = np.ascontiguousarray(xl.T)        # [16384, 256]
        blocks = np.empty((SH, 176), np.float32)
        blocks[:, 0:86] = xtl[:, 0:86]
        blocks[:, 86] = gl * gl
        blocks[:, 87] = gl * bl
        blocks[:, 88] = gl
        blocks[:, 89:175] = xtl[:, 85:171]
        blocks[:, 175] = 1.0
        xt = blocks.reshape(NST, 128, 176).transpose(1, 0, 2)
        xt = np.ascontiguousarray(xt.reshape(128, NST * 176)).astype(_BF)

        g2c = (gl * gl).reshape(NST, 128).T     # [128, NST]
        g2e = np.empty((128, NST, 90), np.float32)
        g2e[:, :, 0:3] = 1.0
        g2e[:, :, 3:89] = g2c[:, :, None]
        g2e[:, :, 89] = 1.0
        g2e = np.ascontiguousarray(g2e.reshape(128, NST * 90)).astype(_BF)

        in_maps.append({
            "xs": np.ascontiguousarray(xl).astype(_BF),
            "xt": xt,
            "g2e": g2e,
            "gb1r": gb1r.astype(_BF),
            "eqt": eqt,
            "ekt": ekt,
            "w0": w0,
            "bk": b_k.reshape(1, C).copy(),
            "sc": sc,
        })
    return in_maps


def kernel(x, gamma, beta, w_qkv, b_qkv):
    from concourse.bass_utils import run_bass_kernel_spmd

    if "nc" not in _cache:
        _cache["nc"] = _build_program()
    nc = _cache["nc"]

    in_maps = _host_prep(x, gamma, beta, w_qkv, b_qkv)
    res = run_bass_kernel_spmd(nc, in_maps, core_ids=list(range(NCORES)))
    out = np.empty((B, C, S), np.float32)
    for r in range(NCORES):
        b, half = r // 2, r % 2
        out[b][:, half * SH:(half + 1) * SH] = np.asarray(
            res.results[r]["out"]).astype(np.float32)
    return out.reshape(np.asarray(x).shape)


if __name__ == "__main__":
    rng = np.random.default_rng(0)
    inputs = {
        "x": rng.standard_normal((B, C, 32, 32, 32)).astype(np.float32),
        "gamma": (1 + 0.1 * rng.standard_normal((32, 32, 32))).astype(np.float32),
        "beta": (0.1 * rng.standard_normal((32, 32, 32))).astype(np.float32),
        "w_qkv": (0.5 * rng.standard_normal(3 * C)).astype(np.float32),
        "b_qkv": (0.05 * rng.standard_normal(3 * C)).astype(np.float32),
    }
    o = kernel(**inputs)
    print("out", o.shape, o.dtype, float(np.abs(o).mean()))
